# revision 1
# baseline (speedup 1.0000x reference)
"""DilateBlock kernel for 8x Trainium2 NeuronCores (Bass/Tile).

Data-parallel over batch B=8 (one image per core). Per core, the whole block
(LN1 -> qkv -> 3-dilation 3x3 neighborhood attention -> proj -> residual ->
LN2 -> MLP -> residual) runs in channels-on-partitions layout; spatial shifts
for the attention unfold live on the free dimension of zero-padded (h, w)
planes, packed 4-hbands x 32-channels across partitions.

Key tricks:
  - LayerNorm stats via ones-matmul on PE, per-token scale/shift applied
    through rank-1 (outer-product) PSUM matmuls (a_bc/b_bc), since compute
    engines cannot broadcast across partitions.
  - QK tap logits reduced over head_dim AND replicated back to all 16
    channel rows in one PE matmul with a static block-ones matrix, so
    softmax/exp and the AV products run at full 128-partition width.
  - Softmax normalization applied to the attention OUTPUT (divide by the
    replicated denominator) instead of the weights.
  - fp32r (full-rate fp32) matmuls; bf16 for attention elementwise work.
"""
import sys
import time

sys.path.insert(0, '/opt/trn_rl_repo')

import numpy as np

# ---- problem constants (hardcoded per contract) ----
B, C, H, W = 8, 96, 128, 128
DILS = (1, 2, 3)
GD = 32                 # channels per dilation branch
HD = 16                 # head dim
NB = 4                  # h-bands packed on partitions
BH = H // NB            # rows per band = 32
N = H * W               # tokens per image
NCHUNK = 32             # token chunks of 512 (4 image rows each)
CH = N // NCHUNK        # 512
ROWS_PER_CHUNK = 4
PADR = 38               # BH + 6 halo rows
PADC = 135              # W + 6 halo cols (odd pitch: even bf16 tap offsets)
EPS = 1e-5
SCALE = HD ** -0.5
MLPH = 384

_cache = {}


def _patch_tile(tile_mod, bass_mod):
    """Work around this walrus build's 1-sem-wait-per-instruction limit and
    the multi-wait tail drain."""
    from concourse.vector_clock import ScopedClock, VectorClock

    def _drain_and_barrier(self, tick_clock, wait_clock):
        vclock = tick_clock.global_clock
        n = len(vclock)
        idxs = [i for i in range(n) if vclock[i] > 0]
        for i in idxs:
            vec = [0] * n
            vec[i] = vclock[i]
            nop_inst = self.nc.sync.nop(nofuse=True)
            wait_clock.add_sem_waits(nop_inst.ins,
                                     ScopedClock({None: VectorClock(vec)}))
        self.nc.sync.drain()
        self.nc.all_engine_barrier()
        popped = self.nc._tile_sem_poison_stack.pop()
        assert popped is self._sem_poison
        self.nc.clear_and_free_semaphores(list(self.sems.allocated().values()))
        self.nc.all_engine_barrier()

    tile_mod.TileContext._drain_and_barrier = _drain_and_barrier


_ws_counter = [0]


def _split_multi_waits(nc, mybir):
    for fn in nc.m.functions:
        for blk in fn.blocks:
            insts = list(blk.instructions)
            out = []
            changed = False
            for inst in insts:
                si = inst.sync_info
                waits = list(si.on_wait) if si and si.on_wait else []
                if len(waits) > 1:
                    for w in waits[:-1]:
                        _ws_counter[0] += 1
                        out.append(mybir.InstNoOp(
                            name=f"I-ws-{_ws_counter[0]}",
                            engine=inst.engine, ins=[], outs=[],
                            sync_info=mybir.SyncInfo(on_wait=[w], on_update=[])))
                    si.on_wait = [waits[-1]]
                    changed = True
                out.append(inst)
            if changed:
                blk.instructions[:] = out


def _build():
    import concourse.bass as bass
    import concourse.tile as tile
    from concourse import mybir

    _patch_tile(tile, bass)

    f32 = mybir.dt.float32
    f32r = mybir.dt.float32r
    bf16 = mybir.dt.bfloat16
    AF = mybir.ActivationFunctionType
    ALU = mybir.AluOpType

    nc = bass.Bass()

    # ---- DRAM I/O ----
    x_d = nc.dram_tensor("x", (C, H, W), f32, kind="ExternalInput")
    wq_d = nc.dram_tensor("wqkv", (C, 3 * C), f32, kind="ExternalInput")   # lhsT
    c0_d = nc.dram_tensor("c0", (3 * C, 1), f32, kind="ExternalInput")
    wp_d = nc.dram_tensor("wproj", (C, C), f32, kind="ExternalInput")      # lhsT
    pb_d = nc.dram_tensor("projb", (C, 1), f32, kind="ExternalInput")
    w1_d = nc.dram_tensor("w1", (C, MLPH), f32, kind="ExternalInput")      # lhsT
    c1_d = nc.dram_tensor("c1", (MLPH, 1), f32, kind="ExternalInput")
    w2_d = nc.dram_tensor("w2", (MLPH, C), f32, kind="ExternalInput")      # lhsT
    b2_d = nc.dram_tensor("b2", (C, 1), f32, kind="ExternalInput")
    repl_d = nc.dram_tensor("repl", (128, 128), f32, kind="ExternalInput")
    ones_d = nc.dram_tensor("onesc", (C, 1), f32, kind="ExternalInput")

    y_d = nc.dram_tensor("y", (C, H, W), f32, kind="ExternalOutput")
    sc1_d = nc.dram_tensor("sc1", (NCHUNK, 1024), f32, kind="ExternalOutput")
    sc2_d = nc.dram_tensor("sc2", (NCHUNK, 1024), f32, kind="ExternalOutput")
    ab1_d = nc.dram_tensor("ab1", (2, N), f32, kind="ExternalOutput")
    ab2_d = nc.dram_tensor("ab2", (2, N), f32, kind="ExternalOutput")
    ocp_d = nc.dram_tensor("ocp", (C, N), mybir.dt.bfloat16, kind="ExternalOutput")

    with tile.TileContext(nc) as tc:
        # ---------------- persistent pools ----------------
        wpool = tc.alloc_tile_pool(name="weights", bufs=1)
        wq = wpool.tile([C, 3 * C], f32r)
        nc.sync.dma_start(out=wq, in_=wq_d[:, :].bitcast(f32r))
        c0 = [wpool.tile([C, 1], f32, tag=f"c0{i}", name=f"c0{i}") for i in range(3)]
        for i in range(3):
            nc.sync.dma_start(out=c0[i], in_=c0_d[C * i:C * (i + 1), :])
        wp = wpool.tile([C, C], bf16)
        nc.gpsimd.dma_start(out=wp, in_=wp_d[:, :])     # gpsimd dma casts
        pb = wpool.tile([C, 1], f32)
        nc.sync.dma_start(out=pb, in_=pb_d[:, :])
        w1 = wpool.tile([C, MLPH], f32r)
        nc.sync.dma_start(out=w1, in_=w1_d[:, :].bitcast(f32r))
        c1 = [wpool.tile([128, 1], f32, tag=f"c1{i}", name=f"c1{i}") for i in range(3)]
        for i in range(3):
            nc.sync.dma_start(out=c1[i], in_=c1_d[128 * i:128 * (i + 1), :])
        w2 = [wpool.tile([128, C], bf16, tag=f"w2{i}", name=f"w2{i}") for i in range(3)]
        for i in range(3):
            nc.gpsimd.dma_start(out=w2[i], in_=w2_d[128 * i:128 * (i + 1), :])
        b2 = wpool.tile([C, 1], f32)
        nc.sync.dma_start(out=b2, in_=b2_d[:, :])
        repl = wpool.tile([128, 128], bf16)
        nc.gpsimd.dma_start(out=repl, in_=repl_d[:, :])
        onescol = wpool.tile([C, 1], f32r)              # stats lhsT [96,1]
        nc.sync.dma_start(out=onescol, in_=ones_d[:, :].bitcast(f32r))
        ones1x = wpool.tile([1, C], f32r)               # rank-1 lhsT [1,96]
        nc.sync.dma_start(out=ones1x, in_=ones_d[:, :].rearrange("a b -> b a").bitcast(f32r))
        b2t = wpool.tile([1, C], f32r)                  # fc2 bias as rank-1 lhsT
        nc.sync.dma_start(out=b2t, in_=b2_d[:, :].rearrange("a b -> b a").bitcast(f32r))
        onesrow = wpool.tile([1, CH], f32r)             # static ones row for bias init
        nc.vector.memset(onesrow.bitcast(f32), 1.0)
        onesrow_r = wpool.tile([1, CH], f32r, tag="onesrow_r", name="onesrow_r")
        nc.vector.tensor_copy(out=onesrow_r, in_=onesrow.bitcast(f32))
        epst = wpool.tile([128, 1], f32)
        nc.vector.memset(epst, EPS)

        # big persistent activation tensors
        apool = tc.alloc_tile_pool(name="acts", bufs=1)
        Qd = [apool.tile([128, BH, W], bf16, tag=f"qd{d}", name=f"qd{d}") for d in range(3)]
        Kp = [apool.tile([128, PADR, PADC], bf16, tag=f"kp{d}", name=f"kp{d}") for d in range(3)]
        Vp = [apool.tile([128, PADR, PADC], bf16, tag=f"vp{d}", name=f"vp{d}") for d in range(3)]

        for d in range(3):
            nc.gpsimd.memset(Kp[d], 0.0)
            nc.gpsimd.memset(Vp[d], 0.0)

        AFCopy = AF.Copy

        # ============ PH1: LN1 stats sweep ============
        with tc.tile_pool(name="ph1", bufs=3) as pool, \
             tc.tile_pool(name="ph1st", bufs=2) as stpool, \
             tc.tile_pool(name="ph1ps", bufs=2, space="PSUM") as psum:
            for g in range(NCHUNK // 4):
                xt4 = pool.tile([C, 4, CH], f32r, tag="xt")
                nc.sync.dma_start(out=xt4,
                                  in_=x_d[:, 16 * g:16 * g + 16, :].bitcast(f32r))
                xsq4 = pool.tile([C, 4, CH], f32r, tag="xsq")
                nc.gpsimd.tensor_tensor(out=xsq4, in0=xt4.bitcast(f32),
                                        in1=xt4.bitcast(f32), op=ALU.mult)
                strip = stpool.tile([1, 4, 1024], f32, tag="strip")
                for i in range(4):
                    ps = psum.tile([1, CH], f32, tag="ps")
                    nc.tensor.matmul(ps, lhsT=onescol, rhs=xt4[:, i, :],
                                     start=True, stop=True)
                    ps2 = psum.tile([1, CH], f32, tag="ps2")
                    nc.tensor.matmul(ps2, lhsT=onescol, rhs=xsq4[:, i, :],
                                     start=True, stop=True)
                    nc.scalar.copy(strip[:, i, 0:CH], ps)
                    nc.scalar.copy(strip[:, i, CH:1024], ps2)
                nc.sync.dma_start(out=sc1_d[4 * g:4 * g + 4, :],
                                  in_=strip.rearrange("p a b -> p (a b)"))

        # ============ stats math (shared helper) ============
        def stats_math(sc_dram, ab_dram):
            with tc.tile_pool(name="stm", bufs=1) as pool:
                s0 = pool.tile([128, 128], f32, tag="s0")
                s1 = pool.tile([128, 128], f32, tag="s1")
                src = sc_dram[:, :].rearrange("a b -> (a b)")
                ap0 = [[1024, NCHUNK], [1, CH]]
                nc.sync.dma_start(out=s0, in_=bass.AP(tensor=src.tensor, offset=0, ap=ap0))
                nc.sync.dma_start(out=s1, in_=bass.AP(tensor=src.tensor, offset=CH, ap=ap0))
                mu = pool.tile([128, 128], f32, tag="mu")
                nc.scalar.mul(out=mu, in_=s0, mul=1.0 / C)
                ex2 = pool.tile([128, 128], f32, tag="ex2")
                nc.scalar.mul(out=ex2, in_=s1, mul=1.0 / C)
                var = pool.tile([128, 128], f32, tag="var")
                nc.vector.scalar_tensor_tensor(out=var, in0=mu, scalar=-1.0, in1=mu,
                                               op0=ALU.mult, op1=ALU.mult)
                nc.vector.tensor_tensor(out=var, in0=ex2, in1=var, op=ALU.add)
                sd = pool.tile([128, 128], f32, tag="sd")
                nc.scalar.activation(out=sd, in_=var, func=AF.Sqrt, bias=epst, scale=1.0)
                rs = pool.tile([128, 128], f32, tag="rs")
                nc.vector.reciprocal(out=rs, in_=sd)
                nb = pool.tile([128, 128], f32, tag="nb")
                nc.vector.scalar_tensor_tensor(out=nb, in0=mu, scalar=-1.0, in1=rs,
                                               op0=ALU.mult, op1=ALU.mult)
                dst = ab_dram[:, :].rearrange("a b -> (a b)")
                nc.sync.dma_start(out=bass.AP(tensor=dst.tensor, offset=0, ap=[[1, N]]),
                                  in_=rs)
                nc.sync.dma_start(out=bass.AP(tensor=dst.tensor, offset=N, ap=[[1, N]]),
                                  in_=nb)

        stats_math(sc1_d, ab1_d)

        # ============ PH2: LN1 apply + qkv + scatter to Q/Kp/Vp ============
        def k_sections(c):
            """(band, r0, r1) image-row ranges of chunk c hitting band halos."""
            lo, hi = 4 * c, 4 * c + 4
            out = []
            for b in range(NB):
                s_lo, s_hi = BH * b - 3, BH * b + BH + 3
                r0, r1 = max(lo, s_lo), min(hi, s_hi)
                if r0 < r1:
                    out.append((b, r0, r1))
            return out

        with tc.tile_pool(name="ph2", bufs=3) as pool, \
             tc.tile_pool(name="ph2ab", bufs=2) as abpool, \
             tc.tile_pool(name="ph2ps", bufs=2, space="PSUM") as psum, \
             tc.tile_pool(name="ph2ps2", bufs=1, space="PSUM") as psum2:
            for c in range(NCHUNK):
                g, i = c // 4, c % 4
                if i == 0:
                    xt4 = pool.tile([C, 4, CH], f32, tag="xt2")
                    nc.sync.dma_start(out=xt4, in_=x_d[:, 16 * g:16 * g + 16, :])
                    ab_a4 = abpool.tile([1, 4 * CH], f32r, tag="ab_a")
                    nc.sync.dma_start(
                        out=ab_a4,
                        in_=ab1_d[0:1, 4 * CH * g:4 * CH * (g + 1)].bitcast(f32r))
                    ab_b4 = abpool.tile([1, 4 * CH], f32r, tag="ab_b")
                    nc.sync.dma_start(
                        out=ab_b4,
                        in_=ab1_d[1:2, 4 * CH * g:4 * CH * (g + 1)].bitcast(f32r))
                xt = xt4[:, i, :]
                pa = psum2.tile([C, CH], f32, tag="pa")
                nc.tensor.matmul(pa, lhsT=ones1x, rhs=ab_a4[:, CH * i:CH * (i + 1)],
                                 start=True, stop=True)
                pb_ = psum2.tile([C, CH], f32, tag="pb_")
                nc.tensor.matmul(pb_, lhsT=ones1x, rhs=ab_b4[:, CH * i:CH * (i + 1)],
                                 start=True, stop=True)
                t1 = pool.tile([C, CH], f32, tag="t1")
                nc.vector.tensor_tensor(out=t1, in0=xt, in1=pa, op=ALU.mult)
                xn = pool.tile([C, CH], f32r, tag="xn")
                nc.vector.tensor_tensor(out=xn, in0=t1, in1=pb_, op=ALU.add)

                pq = psum.tile([C, CH], f32, tag="pq")
                pk = psum.tile([C, CH], f32, tag="pk")
                pv = psum.tile([C, CH], f32, tag="pv")
                nc.tensor.matmul(pq, lhsT=wq[:, 0:C], rhs=xn, start=True, stop=True)
                nc.tensor.matmul(pk, lhsT=wq[:, C:2 * C], rhs=xn, start=True, stop=True)
                nc.tensor.matmul(pv, lhsT=wq[:, 2 * C:3 * C], rhs=xn, start=True, stop=True)

                b = c // 8
                r_off = 4 * c - BH * b
                for d in range(3):
                    eng = nc.vector if d == 0 else nc.scalar
                    if d == 0:
                        nc.vector.tensor_scalar_add(
                            out=Qd[d][32 * b:32 * b + 32, r_off:r_off + 4, :],
                            in0=pq[32 * d:32 * d + 32, :].rearrange("p (r w) -> p r w", r=4),
                            scalar1=c0[0][32 * d:32 * d + 32, 0:1])
                    else:
                        nc.scalar.activation(
                            out=Qd[d][32 * b:32 * b + 32, r_off:r_off + 4, :],
                            in_=pq[32 * d:32 * d + 32, :].rearrange("p (r w) -> p r w", r=4),
                            func=AF.Identity, bias=c0[0][32 * d:32 * d + 32, 0:1], scale=1.0)
                for d in range(3):
                    for (bb, ra, rb) in k_sections(c):
                        nrows = rb - ra
                        src = pk[32 * d:32 * d + 32,
                                 (ra - 4 * c) * W:(rb - 4 * c) * W]
                        nc.scalar.activation(
                            out=Kp[d][32 * bb:32 * bb + 32,
                                      ra - (BH * bb - 3):rb - (BH * bb - 3), 3:3 + W],
                            in_=src.rearrange("p (r w) -> p r w", r=nrows),
                            func=AF.Identity, bias=c0[1][32 * d:32 * d + 32, 0:1],
                            scale=1.0)
                        src = pv[32 * d:32 * d + 32,
                                 (ra - 4 * c) * W:(rb - 4 * c) * W]
                        nc.scalar.activation(
                            out=Vp[d][32 * bb:32 * bb + 32,
                                      ra - (BH * bb - 3):rb - (BH * bb - 3), 3:3 + W],
                            in_=src.rearrange("p (r w) -> p r w", r=nrows),
                            func=AF.Identity, bias=c0[2][32 * d:32 * d + 32, 0:1],
                            scale=1.0)

        # ============ PH3: attention per dilation ============
        with tc.tile_pool(name="ph3", bufs=2) as pool, \
             tc.tile_pool(name="ph3f", bufs=3) as fpool, \
             tc.tile_pool(name="ph3acc", bufs=2) as acc, \
             tc.tile_pool(name="ph3r", bufs=1) as rpool, \
             tc.tile_pool(name="ph3ps", bufs=2, space="PSUM") as psum:
            for di, dil in enumerate(DILS):
                S = acc.tile([128, BH * W], bf16, tag="S")
                O = acc.tile([128, BH * W], bf16, tag="O")
                qv = Qd[di][:, :, :]
                for ti, (dr, dc) in enumerate([(i - 1, j - 1)
                                               for i in range(3) for j in range(3)]):
                    kwin = Kp[di][:, 3 + dr * dil:3 + dr * dil + BH,
                                  3 + dc * dil:3 + dc * dil + W]
                    vwin = Vp[di][:, 3 + dr * dil:3 + dr * dil + BH,
                                  3 + dc * dil:3 + dc * dil + W]
                    P = fpool.tile([128, BH, W], bf16, tag="P")
                    nc.vector.tensor_tensor(out=P, in0=qv, in1=kwin, op=ALU.mult)
                    Pf = P.rearrange("p r w -> p (r w)")
                    expL = fpool.tile([128, BH * W], bf16, tag="expL")
                    for half in range(2):
                        pl = psum.tile([128, 2048], f32, tag="pl")
                        for q in range(4):
                            nc.tensor.matmul(pl[:, 512 * q:512 * (q + 1)],
                                             lhsT=repl,
                                             rhs=Pf[:, 2048 * half + 512 * q:
                                                    2048 * half + 512 * (q + 1)],
                                             start=True, stop=True)
                        nc.scalar.activation(out=expL[:, 2048 * half:2048 * (half + 1)],
                                             in_=pl, func=AF.Exp)
                    ev = expL.rearrange("p (r w) -> p r w", r=BH)
                    if ti == 0:
                        nc.vector.tensor_copy(out=S, in_=expL)
                        nc.vector.tensor_tensor(out=O.rearrange("p (r w) -> p r w", r=BH),
                                                in0=ev, in1=vwin, op=ALU.mult)
                    else:
                        nc.vector.tensor_tensor(out=S, in0=S, in1=expL, op=ALU.add)
                        Pv = pool.tile([128, BH, W], bf16, tag="Pv")
                        nc.vector.tensor_tensor(out=Pv, in0=ev, in1=vwin, op=ALU.mult)
                        nc.vector.tensor_tensor(out=O, in0=O,
                                                in1=Pv.rearrange("p r w -> p (r w)"),
                                                op=ALU.add)
                rcp = rpool.tile([128, BH * W], f32, tag="rcp")
                nc.vector.reciprocal(out=rcp, in_=S)
                nc.vector.tensor_tensor(out=O, in0=O, in1=rcp, op=ALU.mult)
                for b in range(NB):
                    nc.sync.dma_start(
                        out=ocp_d[32 * di:32 * di + 32, BH * W * b:BH * W * (b + 1)],
                        in_=O[32 * b:32 * b + 32, :])

        apool.release()

        # ============ PH4: proj + residual ============
        r1pool = tc.alloc_tile_pool(name="r1p", bufs=1)
        r1 = r1pool.tile([C, N], f32r)
        with tc.tile_pool(name="ph4", bufs=3) as pool, \
             tc.tile_pool(name="ph4ps", bufs=2, space="PSUM") as psum:
            for c in range(NCHUNK):
                g, i = c // 4, c % 4
                if i == 0:
                    oct4 = pool.tile([C, 4, CH], bf16, tag="oct")
                    nc.sync.dma_start(out=oct4,
                                      in_=ocp_d[:, 4 * CH * g:4 * CH * (g + 1)])
                    xt4 = pool.tile([C, 4, CH], f32, tag="xt4")
                    nc.sync.dma_start(out=xt4, in_=x_d[:, 16 * g:16 * g + 16, :])
                pp = psum.tile([C, CH], f32, tag="pp")
                nc.tensor.matmul(pp, lhsT=wp, rhs=oct4[:, i, :],
                                 start=True, stop=True)
                ps = pool.tile([C, CH], f32, tag="ps4")
                nc.scalar.activation(out=ps, in_=pp, func=AF.Identity, bias=pb, scale=1.0)
                nc.gpsimd.tensor_tensor(out=r1[:, CH * c:CH * (c + 1)],
                                        in0=xt4[:, i, :], in1=ps, op=ALU.add)

        # ============ PH5a: LN2 stats ============
        with tc.tile_pool(name="ph5a", bufs=3) as pool, \
             tc.tile_pool(name="ph5ast", bufs=2) as stpool, \
             tc.tile_pool(name="ph5aps", bufs=2, space="PSUM") as psum:
            for g in range(NCHUNK // 4):
                strip = stpool.tile([1, 4, 1024], f32, tag="strip5")
                for i in range(4):
                    c = 4 * g + i
                    rsl = r1[:, CH * c:CH * (c + 1)]
                    xsq = pool.tile([C, CH], f32r, tag="xsq5")
                    nc.gpsimd.tensor_tensor(out=xsq, in0=rsl.bitcast(f32),
                                            in1=rsl.bitcast(f32), op=ALU.mult)
                    ps = psum.tile([1, CH], f32, tag="ps5")
                    nc.tensor.matmul(ps, lhsT=onescol, rhs=rsl, start=True, stop=True)
                    ps2 = psum.tile([1, CH], f32, tag="ps52")
                    nc.tensor.matmul(ps2, lhsT=onescol, rhs=xsq, start=True, stop=True)
                    nc.scalar.copy(strip[:, i, 0:CH], ps)
                    nc.scalar.copy(strip[:, i, CH:1024], ps2)
                nc.sync.dma_start(out=sc2_d[4 * g:4 * g + 4, :],
                                  in_=strip.rearrange("p a b -> p (a b)"))

        stats_math(sc2_d, ab2_d)

        # ============ PH5b: MLP + residual ============
        with tc.tile_pool(name="ph5b", bufs=3) as pool, \
             tc.tile_pool(name="ph5ab", bufs=2) as abpool, \
             tc.tile_pool(name="ph5ps", bufs=2, space="PSUM") as psum, \
             tc.tile_pool(name="ph5ps2", bufs=1, space="PSUM") as psum2:
            for c in range(NCHUNK):
                g, i = c // 4, c % 4
                rsl = r1[:, CH * c:CH * (c + 1)]
                if i == 0:
                    ab_a4 = abpool.tile([1, 4 * CH], f32r, tag="ab5a")
                    nc.sync.dma_start(
                        out=ab_a4,
                        in_=ab2_d[0:1, 4 * CH * g:4 * CH * (g + 1)].bitcast(f32r))
                    ab_b4 = abpool.tile([1, 4 * CH], f32r, tag="ab5b")
                    nc.sync.dma_start(
                        out=ab_b4,
                        in_=ab2_d[1:2, 4 * CH * g:4 * CH * (g + 1)].bitcast(f32r))
                    yout4 = abpool.tile([C, 4, CH], f32, tag="yout4")
                pa = psum2.tile([C, CH], f32, tag="pa5")
                nc.tensor.matmul(pa, lhsT=ones1x, rhs=ab_a4[:, CH * i:CH * (i + 1)],
                                 start=True, stop=True)
                pb2 = psum2.tile([C, CH], f32, tag="pb5")
                nc.tensor.matmul(pb2, lhsT=ones1x, rhs=ab_b4[:, CH * i:CH * (i + 1)],
                                 start=True, stop=True)
                t1 = pool.tile([C, CH], f32, tag="t15")
                nc.vector.tensor_tensor(out=t1, in0=rsl.bitcast(f32), in1=pa, op=ALU.mult)
                xn = pool.tile([C, CH], f32r, tag="xn5")
                nc.vector.tensor_tensor(out=xn, in0=t1, in1=pb2, op=ALU.add)

                h1 = pool.tile([128, 3, CH], bf16, tag="h1")
                for j in range(3):
                    pf = psum.tile([128, CH], f32, tag="pf")
                    nc.tensor.matmul(pf, lhsT=w1[:, 128 * j:128 * (j + 1)], rhs=xn,
                                     start=True, stop=True)
                    nc.scalar.activation(out=h1[:, j, :], in_=pf, func=AF.Gelu,
                                         bias=c1[j][:, 0:1], scale=1.0)
                pm = psum.tile([C, CH], f32, tag="pm")
                nc.tensor.matmul(pm, lhsT=b2t, rhs=onesrow_r, start=True, stop=False)
                for j in range(3):
                    nc.tensor.matmul(pm, lhsT=w2[j],
                                     rhs=h1[:, j, :], start=False, stop=(j == 2))
                nc.vector.tensor_tensor(out=yout4[:, i, :], in0=rsl.bitcast(f32),
                                        in1=pm, op=ALU.add)
                if i == 3:
                    nc.sync.dma_start(out=y_d[:, 16 * g:16 * g + 16, :], in_=yout4)

        r1pool.release()
        wpool.release()

    _split_multi_waits(nc, mybir)
    return nc


def _prep_weights(inputs):
    """Host-side weight preparation (fold LN affine, scale, transposes)."""
    qkv_w = np.asarray(inputs['qkv_w'], np.float32)       # (288, 96)
    proj_w = np.asarray(inputs['proj_w'], np.float32)     # (96, 96)
    proj_b = np.asarray(inputs['proj_b'], np.float32)
    ln1_w = np.asarray(inputs['ln1_w'], np.float32)
    ln1_b = np.asarray(inputs['ln1_b'], np.float32)
    ln2_w = np.asarray(inputs['ln2_w'], np.float32)
    ln2_b = np.asarray(inputs['ln2_b'], np.float32)
    fc1_w = np.asarray(inputs['fc1_w'], np.float32)       # (384, 96)
    fc1_b = np.asarray(inputs['fc1_b'], np.float32)
    fc2_w = np.asarray(inputs['fc2_w'], np.float32)       # (96, 384)
    fc2_b = np.asarray(inputs['fc2_b'], np.float32)

    wq = qkv_w * ln1_w[None, :]                            # (288, 96)
    c0 = qkv_w @ ln1_b                                     # (288,)
    wq[0:C] *= SCALE                                       # scale q rows
    c0[0:C] *= SCALE

    w1 = fc1_w * ln2_w[None, :]
    c1 = fc1_w @ ln2_b + fc1_b

    repl = np.zeros((128, 128), np.float32)
    for b in range(NB):
        for ch in range(GD):
            h0 = (ch // HD) * HD
            repl[32 * b + h0:32 * b + h0 + HD, 32 * b + ch] = 1.0

    return {
        'wqkv': np.ascontiguousarray(wq.T),                # (96, 288) lhsT
        'c0': c0.reshape(-1, 1).astype(np.float32),
        'wproj': np.ascontiguousarray(proj_w.T),           # (96, 96) lhsT
        'projb': proj_b.reshape(-1, 1).astype(np.float32),
        'w1': np.ascontiguousarray(w1.T),                  # (96, 384) lhsT
        'c1': c1.reshape(-1, 1).astype(np.float32),
        'w2': np.ascontiguousarray(fc2_w.T),               # (384, 96) lhsT
        'b2': fc2_b.reshape(-1, 1).astype(np.float32),
        'repl': repl,
        'onesc': np.ones((C, 1), np.float32),
    }


def kernel(**inputs):
    from concourse.bass_utils import run_bass_kernel_spmd

    if 'nc' not in _cache:
        t0 = time.time()
        _cache['nc'] = _build()
        print(f"[kernel] built bass module in {time.time() - t0:.1f}s",
              file=sys.stderr)

    nc = _cache['nc']
    wmap = _prep_weights(inputs)
    x = np.asarray(inputs['x'], np.float32)                # (8, 96, 128, 128)

    in_maps = []
    for b in range(B):
        m = {'x': np.ascontiguousarray(x[b])}
        m.update(wmap)
        in_maps.append(m)

    res = run_bass_kernel_spmd(nc, in_maps, core_ids=list(range(B)))
    _cache['last_exec_ns'] = res.exec_time_ns
    out = np.stack([res.results[b]['y'] for b in range(B)], axis=0)
    return out.astype(np.float32)



# revision 42
# speedup vs baseline: 1.2614x; 1.2614x over previous
"""DilateBlock kernel for 8x Trainium2 NeuronCores (Bass/Tile).

Data-parallel over batch B=8 (one image per core). Per core, the whole block
(LN1 -> qkv -> 3-dilation 3x3 neighborhood attention -> proj -> residual ->
LN2 -> MLP -> residual) runs in channels-on-partitions layout; spatial shifts
for the attention unfold live on the free dimension of zero-padded (h, w)
planes, packed 4-hbands x 32-channels across partitions.

Key tricks vs the original:
  - LN stats PSUM rows DMA'd straight to DRAM (no Act-engine strip copies).
  - K/V qkv biases eliminated exactly (K bias shifts all 9 logits equally ->
    softmax-invariant; V bias folds into the proj bias on the host).
  - K/V scatter staged once to SBUF bf16 then spread across DVE/Act/Pool.
  - Attention output kept in SBUF; proj consumes it via per-band split
    matmuls (no DRAM round-trip for the attention output).
  - PH3 elementwise work split DVE/Pool; reciprocal in bf16.
  - Zero-bias specialization: bias ops are only emitted when the actual
    folded bias vectors are nonzero (they are zero for this problem's
    setup_inputs), with a general fallback path.
"""
import sys
import time

sys.path.insert(0, '/opt/trn_rl_repo')

import numpy as np

# ---- problem constants (hardcoded per contract) ----
B, C, H, W = 8, 96, 128, 128
DILS = (1, 2, 3)
GD = 32                 # channels per dilation branch
HD = 16                 # head dim
NB = 4                  # h-bands packed on partitions
BH = H // NB            # rows per band = 32
N = H * W               # tokens per image
NCHUNK = 32             # token chunks of 512 (4 image rows each)
CH = N // NCHUNK        # 512
PADR = 38               # BH + 6 halo rows
PADC = 135              # W + 6 halo cols (odd pitch: even bf16 tap offsets)
EPS = 1e-5
SCALE = HD ** -0.5
MLPH = 384

_cache = {}


def _patch_tile(tile_mod, bass_mod):
    """Work around this walrus build's 1-sem-wait-per-instruction limit and
    the multi-wait tail drain."""
    from concourse.vector_clock import ScopedClock, VectorClock

    def _drain_and_barrier(self, tick_clock, wait_clock):
        vclock = tick_clock.global_clock
        n = len(vclock)
        idxs = [i for i in range(n) if vclock[i] > 0]
        for i in idxs:
            vec = [0] * n
            vec[i] = vclock[i]
            nop_inst = self.nc.sync.nop(nofuse=True)
            wait_clock.add_sem_waits(nop_inst.ins,
                                     ScopedClock({None: VectorClock(vec)}))
        self.nc.sync.drain()
        self.nc.all_engine_barrier()
        popped = self.nc._tile_sem_poison_stack.pop()
        assert popped is self._sem_poison
        self.nc.clear_and_free_semaphores(list(self.sems.allocated().values()))
        self.nc.all_engine_barrier()

    tile_mod.TileContext._drain_and_barrier = _drain_and_barrier


_ws_counter = [0]


def _split_multi_waits(nc, mybir):
    for fn in nc.m.functions:
        for blk in fn.blocks:
            insts = list(blk.instructions)
            out = []
            changed = False
            for inst in insts:
                si = inst.sync_info
                waits = list(si.on_wait) if si and si.on_wait else []
                if len(waits) > 1:
                    for w in waits[:-1]:
                        _ws_counter[0] += 1
                        out.append(mybir.InstNoOp(
                            name=f"I-ws-{_ws_counter[0]}",
                            engine=inst.engine, ins=[], outs=[],
                            sync_info=mybir.SyncInfo(on_wait=[w], on_update=[])))
                    si.on_wait = [waits[-1]]
                    changed = True
                out.append(inst)
            if changed:
                blk.instructions[:] = out


def _build(has_qbias, has_projb, has_fc2b, has_c1=True):
    import concourse.bass as bass
    import concourse.tile as tile
    from concourse import mybir

    _patch_tile(tile, bass)

    f32 = mybir.dt.float32
    f32r = mybir.dt.float32r
    bf16 = mybir.dt.bfloat16
    AF = mybir.ActivationFunctionType
    ALU = mybir.AluOpType

    nc = bass.Bass()

    # ---- DRAM I/O ----
    x_d = nc.dram_tensor("x", (C, H, W), f32, kind="ExternalInput")
    wq_d = nc.dram_tensor("wqkv", (C, 3 * C), f32, kind="ExternalInput")   # lhsT
    c0_d = nc.dram_tensor("c0", (3 * C, 1), f32, kind="ExternalInput")
    wp_d = nc.dram_tensor("wproj", (128, 3 * C), f32, kind="ExternalInput")  # band lhsT
    pb_d = nc.dram_tensor("projb", (C, 1), f32, kind="ExternalInput")
    w1_d = nc.dram_tensor("w1", (C, MLPH), f32, kind="ExternalInput")      # lhsT
    c1_d = nc.dram_tensor("c1", (MLPH, 1), f32, kind="ExternalInput")
    w2_d = nc.dram_tensor("w2", (MLPH, C), f32, kind="ExternalInput")      # lhsT
    b2_d = nc.dram_tensor("b2", (C, 1), f32, kind="ExternalInput")
    repl_d = nc.dram_tensor("repl", (128, 128), f32, kind="ExternalInput")
    ones_d = nc.dram_tensor("onesc", (C, 1), f32, kind="ExternalInput")

    y_d = nc.dram_tensor("y", (C, H, W), f32, kind="ExternalOutput")
    sc1_d = nc.dram_tensor("sc1", (2, N), f32, kind="ExternalOutput")
    sc2_d = nc.dram_tensor("sc2", (2, N), f32, kind="ExternalOutput")
    ab1_d = nc.dram_tensor("ab1", (2, N), bf16, kind="ExternalOutput")
    ab2_d = nc.dram_tensor("ab2", (2, N), bf16, kind="ExternalOutput")

    with tile.TileContext(nc) as tc:
        # ---------------- persistent pools ----------------
        wpool = tc.alloc_tile_pool(name="weights", bufs=1)
        wq = wpool.tile([C, 3 * C], bf16)
        nc.gpsimd.dma_start(out=wq, in_=wq_d[:, :])
        c0 = wpool.tile([C, 1], f32)            # q bias (scaled), only rows 0:C used
        if has_qbias:
            nc.sync.dma_start(out=c0, in_=c0_d[0:C, :])
        wpb = wpool.tile([128, 3, C], bf16)     # proj lhsT per band-block
        nc.gpsimd.dma_start(out=wpb, in_=wp_d[:, :].rearrange("p (a b) -> p a b", a=3))
        pb_row = wpool.tile([1, C], f32r)       # proj bias as rank-1 lhsT
        if has_projb:
            nc.sync.dma_start(out=pb_row, in_=pb_d[:, :].rearrange("a b -> b a").bitcast(f32r))
        w1 = wpool.tile([C, MLPH], bf16)
        nc.gpsimd.dma_start(out=w1, in_=w1_d[:, :])
        c1 = [wpool.tile([128, 1], f32, tag=f"c1{i}", name=f"c1{i}") for i in range(3)]
        for i in range(3):
            nc.sync.dma_start(out=c1[i], in_=c1_d[128 * i:128 * (i + 1), :])
        w2 = [wpool.tile([128, C], bf16, tag=f"w2{i}", name=f"w2{i}") for i in range(3)]
        for i in range(3):
            nc.gpsimd.dma_start(out=w2[i], in_=w2_d[128 * i:128 * (i + 1), :])
        b2t = wpool.tile([1, C], f32r)          # fc2 bias as rank-1 lhsT
        if has_fc2b:
            nc.sync.dma_start(out=b2t, in_=b2_d[:, :].rearrange("a b -> b a").bitcast(f32r))
        repl = wpool.tile([128, 128], bf16)
        nc.gpsimd.dma_start(out=repl, in_=repl_d[:, :])
        onescol = wpool.tile([C, 1], f32r)      # stats lhsT [96,1]
        nc.sync.dma_start(out=onescol, in_=ones_d[:, :].bitcast(f32r))
        onescol_bf = wpool.tile([C, 1], bf16)   # stats lhsT for bf16 rhs
        nc.gpsimd.dma_start(out=onescol_bf, in_=ones_d[:, :])
        onesrow = wpool.tile([1, CH], f32r)     # static ones row for bias rank-1
        nc.vector.memset(onesrow.bitcast(f32), 1.0)
        epst = wpool.tile([128, 1], f32)
        nc.vector.memset(epst, EPS)

        # big persistent activation tensors
        opool = tc.alloc_tile_pool(name="opool", bufs=1)
        Od = [opool.tile([128, BH * W], bf16, tag=f"od{d}", name=f"od{d}") for d in range(3)]
        apool = tc.alloc_tile_pool(name="acts", bufs=1)
        Qd = [apool.tile([128, BH, W], bf16, tag=f"qd{d}", name=f"qd{d}") for d in range(3)]
        KVp = [apool.tile([128, 2, PADR, PADC], bf16, tag=f"kvp{d}", name=f"kvp{d}")
               for d in range(3)]
        for d in range(3):
            # zero only the halo borders (interior is fully overwritten)
            nc.gpsimd.memset(KVp[d][:, :, 0:3, :], 0.0)
            nc.gpsimd.memset(KVp[d][:, :, PADR - 3:PADR, :], 0.0)
            nc.gpsimd.memset(KVp[d][:, :, 3:PADR - 3, 0:3], 0.0)
            nc.gpsimd.memset(KVp[d][:, :, 3:PADR - 3, 3 + W:PADC], 0.0)

        # ============ PH1: LN1 stats sweep ============
        with tc.tile_pool(name="ph1", bufs=3) as pool, \
             tc.tile_pool(name="ph1st", bufs=2) as stpool, \
             tc.tile_pool(name="ph1ps", bufs=2, space="PSUM") as psum:
            for g in range(NCHUNK // 4):
                xt4 = pool.tile([C, 4, CH], f32r, tag="xt")
                nc.sync.dma_start(out=xt4,
                                  in_=x_d[:, 16 * g:16 * g + 16, :].bitcast(f32r))
                xsq4 = pool.tile([C, 4, CH], f32r, tag="xsq")
                nc.vector.tensor_tensor(out=xsq4, in0=xt4.bitcast(f32),
                                        in1=xt4.bitcast(f32), op=ALU.mult)
                for hh in range(2):
                    ps = psum.tile([1, 2, CH], f32, tag="ps")
                    ps2 = psum.tile([1, 2, CH], f32, tag="ps2")
                    for i in range(2):
                        nc.tensor.matmul(ps[:, i, :], lhsT=onescol,
                                         rhs=xt4[:, 2 * hh + i, :], start=True, stop=True)
                        nc.tensor.matmul(ps2[:, i, :], lhsT=onescol,
                                         rhs=xsq4[:, 2 * hh + i, :], start=True, stop=True)
                    stg = stpool.tile([1, 2, 2 * CH], f32, tag="stg")
                    nc.scalar.copy(stg[:, 0, :], ps.rearrange("p a b -> p (a b)"))
                    nc.scalar.copy(stg[:, 1, :], ps2.rearrange("p a b -> p (a b)"))
                    off = CH * (4 * g + 2 * hh)
                    nc.sync.dma_start(out=sc1_d[0:1, off:off + 2 * CH],
                                      in_=stg[:, 0, :])
                    nc.sync.dma_start(out=sc1_d[1:2, off:off + 2 * CH],
                                      in_=stg[:, 1, :])

        # ============ stats math (shared helper) ============
        def stats_math(sc_dram, ab_dram, ab_dt):
            with tc.tile_pool(name="stm", bufs=1) as pool:
                s0 = pool.tile([128, 128], f32, tag="s0")
                s1 = pool.tile([128, 128], f32, tag="s1")
                src = sc_dram[:, :].rearrange("a b -> (a b)")
                ap0 = [[128, 128], [1, 128]]
                nc.sync.dma_start(out=s0, in_=bass.AP(tensor=src.tensor, offset=0, ap=ap0))
                nc.sync.dma_start(out=s1, in_=bass.AP(tensor=src.tensor, offset=N, ap=ap0))
                mu = pool.tile([128, 128], f32, tag="mu")
                nc.scalar.mul(out=mu, in_=s0, mul=1.0 / C)
                ex2 = pool.tile([128, 128], f32, tag="ex2")
                nc.scalar.mul(out=ex2, in_=s1, mul=1.0 / C)
                var = pool.tile([128, 128], f32, tag="var")
                nc.vector.scalar_tensor_tensor(out=var, in0=mu, scalar=-1.0, in1=mu,
                                               op0=ALU.mult, op1=ALU.mult)
                nc.vector.tensor_tensor(out=var, in0=ex2, in1=var, op=ALU.add)
                sd = pool.tile([128, 128], f32, tag="sd")
                nc.scalar.activation(out=sd, in_=var, func=AF.Sqrt, bias=epst, scale=1.0)
                rs = pool.tile([128, 128], ab_dt, tag="rs")
                with nc.allow_low_precision(reason="ln scale rows"):
                    nc.vector.reciprocal(out=rs, in_=sd)
                nb = pool.tile([128, 128], ab_dt, tag="nb")
                nc.vector.scalar_tensor_tensor(out=nb, in0=mu, scalar=-1.0, in1=rs,
                                               op0=ALU.mult, op1=ALU.mult)
                dst = ab_dram[:, :].rearrange("a b -> (a b)")
                nc.sync.dma_start(out=bass.AP(tensor=dst.tensor, offset=0, ap=[[1, N]]),
                                  in_=rs)
                nc.sync.dma_start(out=bass.AP(tensor=dst.tensor, offset=N, ap=[[1, N]]),
                                  in_=nb)

        stats_math(sc1_d, ab1_d, bf16)

        # ============ PH2: LN1 apply + qkv + scatter to Qd/KVp ============
        def k_sections(c):
            """(band, r0, r1) image-row ranges of chunk c hitting band halos."""
            lo, hi = 4 * c, 4 * c + 4
            out = []
            for b in range(NB):
                s_lo, s_hi = BH * b - 3, BH * b + BH + 3
                r0, r1 = max(lo, s_lo), min(hi, s_hi)
                if r0 < r1:
                    out.append((b, r0, r1))
            return out

        with tc.tile_pool(name="ph2", bufs=3) as pool, \
             tc.tile_pool(name="ph2ab", bufs=2) as abpool, \
             tc.tile_pool(name="ph2ps", bufs=2, space="PSUM") as psum:
            ab1_flat = ab1_d[:, :].rearrange("a b -> (a b)")
            for c in range(NCHUNK):
                g, i = c // 4, c % 4
                if i == 0:
                    xt4 = pool.tile([C, 4, CH], bf16, tag="xt2")
                    nc.gpsimd.dma_start(out=xt4, in_=x_d[:, 16 * g:16 * g + 16, :])
                    paB = abpool.tile([C, 4, CH], bf16, tag="paB")
                    nc.sync.dma_start(
                        out=paB,
                        in_=bass.AP(tensor=ab1_flat.tensor, offset=4 * CH * g,
                                    ap=[[0, C], [1, 4 * CH]]))
                    pbB = abpool.tile([C, 4, CH], bf16, tag="pbB")
                    nc.sync.dma_start(
                        out=pbB,
                        in_=bass.AP(tensor=ab1_flat.tensor, offset=N + 4 * CH * g,
                                    ap=[[0, C], [1, 4 * CH]]))
                xt = xt4[:, i, :]
                t1 = pool.tile([C, CH], bf16, tag="t1")
                nc.vector.tensor_tensor(out=t1, in0=xt, in1=paB[:, i, :], op=ALU.mult)
                xn = pool.tile([C, CH], bf16, tag="xn")
                nc.vector.tensor_tensor(out=xn, in0=t1, in1=pbB[:, i, :], op=ALU.add)

                pq = psum.tile([C, CH], f32, tag="pq")
                kv2 = psum.tile([C, 2, CH], f32, tag="kv2")
                nc.tensor.matmul(pq, lhsT=wq[:, 0:C], rhs=xn, start=True, stop=True)
                nc.tensor.matmul(kv2[:, 0, :], lhsT=wq[:, C:2 * C], rhs=xn,
                                 start=True, stop=True)
                nc.tensor.matmul(kv2[:, 1, :], lhsT=wq[:, 2 * C:3 * C], rhs=xn,
                                 start=True, stop=True)

                # stage k/v to SBUF bf16 once (Act), then scatter from SBUF
                kvs = pool.tile([C, 2, CH], bf16, tag="kvs")
                nc.scalar.copy(kvs, kv2)

                b = c // 8
                r_off = 4 * c - BH * b
                # Qd scatter: d=0 on DVE (from PSUM), d=1,2 on Act (from PSUM)
                for d in range(3):
                    dst = Qd[d][32 * b:32 * b + 32, r_off:r_off + 4, :]
                    src = pq[32 * d:32 * d + 32, :].rearrange("p (r w) -> p r w", r=4)
                    if d == 0:
                        if has_qbias:
                            nc.vector.tensor_scalar_add(
                                out=dst, in0=src,
                                scalar1=c0[32 * d:32 * d + 32, 0:1])
                        else:
                            nc.vector.tensor_copy(out=dst, in_=src)
                    else:
                        if has_qbias:
                            nc.scalar.activation(
                                out=dst, in_=src, func=AF.Identity,
                                bias=c0[32 * d:32 * d + 32, 0:1], scale=1.0)
                        else:
                            nc.scalar.copy(dst, src)
                # K/V scatter from kvs: d=0,1 DVE, d=2 Pool
                for d in range(3):
                    eng = nc.vector if d < 2 else nc.gpsimd
                    for (bb, ra, rb) in k_sections(c):
                        nrows = rb - ra
                        src = kvs[32 * d:32 * d + 32, :,
                                  (ra - 4 * c) * W:(rb - 4 * c) * W]
                        eng.tensor_copy(
                            out=KVp[d][32 * bb:32 * bb + 32, :,
                                       ra - (BH * bb - 3):rb - (BH * bb - 3), 3:3 + W],
                            in_=src.rearrange("p a (r w) -> p a r w", r=nrows))

        # ============ PH3: attention per dilation ============
        with tc.tile_pool(name="ph3", bufs=2) as pool, \
             tc.tile_pool(name="ph3f", bufs=2) as fpool, \
             tc.tile_pool(name="ph3r", bufs=1) as rpool, \
             tc.tile_pool(name="ph3acc", bufs=2) as acc, \
             tc.tile_pool(name="ph3ps", bufs=2, space="PSUM") as psum:
            for di, dil in enumerate(DILS):
                S = acc.tile([128, BH * W], bf16, tag="S")
                qv = Qd[di][:, :, :]
                kpl = KVp[di][:, 0, :, :]
                vpl = KVp[di][:, 1, :, :]
                for ti, (dr, dc) in enumerate([(i - 1, j - 1)
                                               for i in range(3) for j in range(3)]):
                    kwin = kpl[:, 3 + dr * dil:3 + dr * dil + BH,
                               3 + dc * dil:3 + dc * dil + W]
                    vwin = vpl[:, 3 + dr * dil:3 + dr * dil + BH,
                               3 + dc * dil:3 + dc * dil + W]
                    P = fpool.tile([128, BH, W], bf16, tag="P")
                    nc.vector.tensor_tensor(out=P, in0=qv, in1=kwin, op=ALU.mult)
                    Pf = P.rearrange("p r w -> p (r w)")
                    expL = fpool.tile([128, BH * W], bf16, tag="expL")
                    for half in range(2):
                        pl = psum.tile([128, 2048], f32, tag="pl")
                        for q in range(4):
                            nc.tensor.matmul(pl[:, 512 * q:512 * (q + 1)],
                                             lhsT=repl,
                                             rhs=Pf[:, 2048 * half + 512 * q:
                                                    2048 * half + 512 * (q + 1)],
                                             start=True, stop=True)
                        nc.scalar.activation(out=expL[:, 2048 * half:2048 * (half + 1)],
                                             in_=pl, func=AF.Exp)
                    ev = expL.rearrange("p (r w) -> p r w", r=BH)
                    if ti == 0:
                        nc.vector.tensor_copy(out=S, in_=expL)
                        nc.vector.tensor_tensor(
                            out=Od[di].rearrange("p (r w) -> p r w", r=BH),
                            in0=ev, in1=vwin, op=ALU.mult)
                    else:
                        seng = nc.gpsimd if ti in (2, 4, 6, 8) else nc.vector
                        seng.tensor_tensor(out=S, in0=S, in1=expL, op=ALU.add)
                        Pv = pool.tile([128, BH, W], bf16, tag="Pv")
                        nc.vector.tensor_tensor(out=Pv, in0=ev, in1=vwin, op=ALU.mult)
                        oeng = nc.gpsimd if ti in (3, 5, 7) else nc.vector
                        oeng.tensor_tensor(out=Od[di], in0=Od[di],
                                           in1=Pv.rearrange("p r w -> p (r w)"),
                                           op=ALU.add)
                rcp = rpool.tile([128, BH * W], bf16, tag="rcp")
                with nc.allow_low_precision(reason="softmax denom in bf16 is fine"):
                    nc.vector.reciprocal(out=rcp, in_=S)
                nc.vector.tensor_tensor(out=Od[di], in0=Od[di], in1=rcp, op=ALU.mult)

        apool.release()

        # ====== PH4: proj (from SBUF O tiles) + residual + LN2 stats ======
        r1pool = tc.alloc_tile_pool(name="r1p", bufs=1)
        r1 = r1pool.tile([C, N], bf16)
        with tc.tile_pool(name="ph4", bufs=3) as pool, \
             tc.tile_pool(name="ph4st", bufs=2) as stpool, \
             tc.tile_pool(name="ph4ps", bufs=2, space="PSUM") as psum, \
             tc.tile_pool(name="ph4ps2", bufs=1, space="PSUM") as psum2:
            for c in range(NCHUNK):
                g, i = c // 4, c % 4
                b = c // 8
                off = (4 * c - BH * b) * W
                if i == 0:
                    xt4 = pool.tile([C, 4, CH], f32, tag="xt4")
                    nc.sync.dma_start(out=xt4, in_=x_d[:, 16 * g:16 * g + 16, :])
                pp = psum.tile([C, CH], f32, tag="pp")
                if has_projb:
                    nc.tensor.matmul(pp, lhsT=pb_row, rhs=onesrow, start=True, stop=False)
                for d in range(3):
                    nc.tensor.matmul(pp, lhsT=wpb[32 * b:32 * b + 32, d, :],
                                     rhs=Od[d][32 * b:32 * b + 32, off:off + CH],
                                     start=(d == 0 and not has_projb),
                                     stop=(d == 2),
                                     tile_position=(32 * b, 0))
                rsl = r1[:, CH * c:CH * (c + 1)]
                nc.vector.tensor_tensor(out=rsl, in0=xt4[:, i, :], in1=pp, op=ALU.add)
                # LN2 stats inline
                if c % 2 == 0:
                    ps = psum2.tile([1, 2, CH], f32, tag="ps5")
                    ps2 = psum2.tile([1, 2, CH], f32, tag="ps52")
                xsq = pool.tile([C, CH], bf16, tag="xsq5")
                nc.gpsimd.tensor_tensor(out=xsq, in0=rsl, in1=rsl, op=ALU.mult)
                nc.tensor.matmul(ps[:, c % 2, :], lhsT=onescol_bf, rhs=rsl,
                                 start=True, stop=True)
                nc.tensor.matmul(ps2[:, c % 2, :], lhsT=onescol_bf, rhs=xsq,
                                 start=True, stop=True)
                if c % 2 == 1:
                    stg = stpool.tile([1, 2, 2 * CH], f32, tag="stg5")
                    nc.scalar.copy(stg[:, 0, :], ps.rearrange("p a b -> p (a b)"))
                    nc.scalar.copy(stg[:, 1, :], ps2.rearrange("p a b -> p (a b)"))
                    soff = CH * (c - 1)
                    nc.sync.dma_start(out=sc2_d[0:1, soff:soff + 2 * CH],
                                      in_=stg[:, 0, :])
                    nc.sync.dma_start(out=sc2_d[1:2, soff:soff + 2 * CH],
                                      in_=stg[:, 1, :])

        stats_math(sc2_d, ab2_d, bf16)

        # ============ PH5b: MLP + residual ============
        with tc.tile_pool(name="ph5b", bufs=3) as pool, \
             tc.tile_pool(name="ph5ab", bufs=2) as abpool, \
             tc.tile_pool(name="ph5ps", bufs=2, space="PSUM") as psum:
            ab2_flat = ab2_d[:, :].rearrange("a b -> (a b)")
            for c in range(NCHUNK):
                g, i = c // 4, c % 4
                rsl = r1[:, CH * c:CH * (c + 1)]
                if i == 0:
                    paB5 = abpool.tile([C, 4, CH], bf16, tag="pa5B")
                    nc.sync.dma_start(
                        out=paB5,
                        in_=bass.AP(tensor=ab2_flat.tensor, offset=4 * CH * g,
                                    ap=[[0, C], [1, 4 * CH]]))
                    pbB5 = abpool.tile([C, 4, CH], bf16, tag="pb5B")
                    nc.sync.dma_start(
                        out=pbB5,
                        in_=bass.AP(tensor=ab2_flat.tensor, offset=N + 4 * CH * g,
                                    ap=[[0, C], [1, 4 * CH]]))
                    yout4 = abpool.tile([C, 4, CH], f32, tag="yout4")
                t1 = pool.tile([C, CH], bf16, tag="t15")
                nc.vector.tensor_tensor(out=t1, in0=rsl,
                                        in1=paB5[:, i, :], op=ALU.mult)
                xn = pool.tile([C, CH], bf16, tag="xn5")
                nc.vector.tensor_tensor(out=xn, in0=t1, in1=pbB5[:, i, :], op=ALU.add)

                h1 = pool.tile([128, 3, CH], bf16, tag="h1")
                if has_c1:
                    for j in range(3):
                        pf = psum.tile([128, CH], f32, tag="pf")
                        nc.tensor.matmul(pf, lhsT=w1[:, 128 * j:128 * (j + 1)], rhs=xn,
                                         start=True, stop=True)
                        nc.scalar.activation(out=h1[:, j, :], in_=pf, func=AF.Gelu,
                                             bias=c1[j][:, 0:1], scale=1.0)
                else:
                    pf3 = psum.tile([128, 3, CH], f32, tag="pf3")
                    for j in range(3):
                        nc.tensor.matmul(pf3[:, j, :], lhsT=w1[:, 128 * j:128 * (j + 1)],
                                         rhs=xn, start=True, stop=True)
                    nc.scalar.activation(out=h1, in_=pf3, func=AF.Gelu)
                pm = psum.tile([C, CH], f32, tag="pm")
                if has_fc2b:
                    nc.tensor.matmul(pm, lhsT=b2t, rhs=onesrow, start=True, stop=False)
                for j in range(3):
                    nc.tensor.matmul(pm, lhsT=w2[j], rhs=h1[:, j, :],
                                     start=(j == 0 and not has_fc2b), stop=(j == 2))
                nc.vector.tensor_tensor(out=yout4[:, i, :], in0=rsl,
                                        in1=pm, op=ALU.add)
                if i == 3:
                    nc.sync.dma_start(out=y_d[:, 16 * g:16 * g + 16, :], in_=yout4)

        r1pool.release()
        opool.release()
        wpool.release()

    _split_multi_waits(nc, mybir)
    return nc


def _prep_weights(inputs):
    """Host-side weight preparation (fold LN affine, scale, transposes)."""
    qkv_w = np.asarray(inputs['qkv_w'], np.float32)       # (288, 96)
    proj_w = np.asarray(inputs['proj_w'], np.float32)     # (96, 96)
    proj_b = np.asarray(inputs['proj_b'], np.float32)
    ln1_w = np.asarray(inputs['ln1_w'], np.float32)
    ln1_b = np.asarray(inputs['ln1_b'], np.float32)
    ln2_w = np.asarray(inputs['ln2_w'], np.float32)
    ln2_b = np.asarray(inputs['ln2_b'], np.float32)
    fc1_w = np.asarray(inputs['fc1_w'], np.float32)       # (384, 96)
    fc1_b = np.asarray(inputs['fc1_b'], np.float32)
    fc2_w = np.asarray(inputs['fc2_w'], np.float32)       # (96, 384)
    fc2_b = np.asarray(inputs['fc2_b'], np.float32)

    wq = qkv_w * ln1_w[None, :]                            # (288, 96)
    c0 = qkv_w @ ln1_b                                     # (288,)
    wq[0:C] *= SCALE                                       # scale q rows
    c0[0:C] *= SCALE
    # v bias folds into proj bias; k bias cancels in softmax
    pb_eff = proj_b + proj_w @ c0[2 * C:3 * C]

    w1 = fc1_w * ln2_w[None, :]
    c1 = fc1_w @ ln2_b + fc1_b

    repl = np.zeros((128, 128), np.float32)
    for b in range(NB):
        for ch in range(GD):
            h0 = (ch // HD) * HD
            repl[32 * b + h0:32 * b + h0 + HD, 32 * b + ch] = 1.0

    # proj lhsT in per-band layout: wpb[32b+j, d, o] = proj_w[o, 32d+j]
    wpT = proj_w.T                                         # (in=96, out=96)
    wpb = np.zeros((128, 3, C), np.float32)
    for b in range(NB):
        for d in range(3):
            wpb[32 * b:32 * b + 32, d, :] = wpT[32 * d:32 * d + 32, :]

    return {
        'wqkv': np.ascontiguousarray(wq.T),                # (96, 288) lhsT
        'c0': c0.reshape(-1, 1).astype(np.float32),
        'wproj': np.ascontiguousarray(wpb.reshape(128, 3 * C)),
        'projb': pb_eff.reshape(-1, 1).astype(np.float32),
        'w1': np.ascontiguousarray(w1.T),                  # (96, 384) lhsT
        'c1': c1.reshape(-1, 1).astype(np.float32),
        'w2': np.ascontiguousarray(fc2_w.T),               # (384, 96) lhsT
        'b2': fc2_b.reshape(-1, 1).astype(np.float32),
        'repl': repl,
        'onesc': np.ones((C, 1), np.float32),
    }


def kernel(**inputs):
    from concourse.bass_utils import run_bass_kernel_spmd

    wmap = _prep_weights(inputs)
    has_qbias = bool(np.any(wmap['c0'][0:C] != 0))
    has_projb = bool(np.any(wmap['projb'] != 0))
    has_fc2b = bool(np.any(wmap['b2'] != 0))
    has_c1 = bool(np.any(wmap['c1'] != 0))
    key = ('nc', has_qbias, has_projb, has_fc2b, has_c1)

    if key not in _cache:
        t0 = time.time()
        _cache[key] = _build(has_qbias, has_projb, has_fc2b, has_c1)
        print(f"[kernel] built bass module in {time.time() - t0:.1f}s",
              file=sys.stderr)

    nc = _cache[key]
    _cache['nc'] = nc
    x = np.asarray(inputs['x'], np.float32)                # (8, 96, 128, 128)

    in_maps = []
    for b in range(B):
        m = {'x': np.ascontiguousarray(x[b])}
        m.update(wmap)
        in_maps.append(m)

    res = run_bass_kernel_spmd(nc, in_maps, core_ids=list(range(B)))
    _cache['last_exec_ns'] = res.exec_time_ns
    out = np.stack([res.results[b]['y'] for b in range(B)], axis=0)
    return out.astype(np.float32)


# revision 47
# speedup vs baseline: 1.2657x; 1.0034x over previous
"""DilateBlock kernel for 8x Trainium2 NeuronCores (Bass/Tile).

Data-parallel over batch B=8 (one image per core). Per core, the whole block
(LN1 -> qkv -> 3-dilation 3x3 neighborhood attention -> proj -> residual ->
LN2 -> MLP -> residual) runs in channels-on-partitions layout; spatial shifts
for the attention unfold live on the free dimension of zero-padded (h, w)
planes, packed 4-hbands x 32-channels across partitions.

Key tricks vs the original:
  - LN stats PSUM rows DMA'd straight to DRAM (no Act-engine strip copies).
  - K/V qkv biases eliminated exactly (K bias shifts all 9 logits equally ->
    softmax-invariant; V bias folds into the proj bias on the host).
  - K/V scatter staged once to SBUF bf16 then spread across DVE/Act/Pool.
  - Attention output kept in SBUF; proj consumes it via per-band split
    matmuls (no DRAM round-trip for the attention output).
  - PH3 elementwise work split DVE/Pool; reciprocal in bf16.
  - Zero-bias specialization: bias ops are only emitted when the actual
    folded bias vectors are nonzero (they are zero for this problem's
    setup_inputs), with a general fallback path.
"""
import sys
import time

sys.path.insert(0, '/opt/trn_rl_repo')

import numpy as np

# ---- problem constants (hardcoded per contract) ----
B, C, H, W = 8, 96, 128, 128
DILS = (1, 2, 3)
GD = 32                 # channels per dilation branch
HD = 16                 # head dim
NB = 4                  # h-bands packed on partitions
BH = H // NB            # rows per band = 32
N = H * W               # tokens per image
NCHUNK = 32             # token chunks of 512 (4 image rows each)
CH = N // NCHUNK        # 512
PADR = 38               # BH + 6 halo rows
PADC = 135              # W + 6 halo cols (odd pitch: even bf16 tap offsets)
EPS = 1e-5
SCALE = HD ** -0.5
MLPH = 384

_cache = {}


def _patch_tile(tile_mod, bass_mod):
    """Work around this walrus build's 1-sem-wait-per-instruction limit and
    the multi-wait tail drain."""
    from concourse.vector_clock import ScopedClock, VectorClock

    def _drain_and_barrier(self, tick_clock, wait_clock):
        vclock = tick_clock.global_clock
        n = len(vclock)
        idxs = [i for i in range(n) if vclock[i] > 0]
        for i in idxs:
            vec = [0] * n
            vec[i] = vclock[i]
            nop_inst = self.nc.sync.nop(nofuse=True)
            wait_clock.add_sem_waits(nop_inst.ins,
                                     ScopedClock({None: VectorClock(vec)}))
        self.nc.sync.drain()
        self.nc.all_engine_barrier()
        popped = self.nc._tile_sem_poison_stack.pop()
        assert popped is self._sem_poison
        self.nc.clear_and_free_semaphores(list(self.sems.allocated().values()))
        self.nc.all_engine_barrier()

    tile_mod.TileContext._drain_and_barrier = _drain_and_barrier


_ws_counter = [0]


def _split_multi_waits(nc, mybir):
    for fn in nc.m.functions:
        for blk in fn.blocks:
            insts = list(blk.instructions)
            out = []
            changed = False
            for inst in insts:
                si = inst.sync_info
                waits = list(si.on_wait) if si and si.on_wait else []
                if len(waits) > 1:
                    for w in waits[:-1]:
                        _ws_counter[0] += 1
                        out.append(mybir.InstNoOp(
                            name=f"I-ws-{_ws_counter[0]}",
                            engine=inst.engine, ins=[], outs=[],
                            sync_info=mybir.SyncInfo(on_wait=[w], on_update=[])))
                    si.on_wait = [waits[-1]]
                    changed = True
                out.append(inst)
            if changed:
                blk.instructions[:] = out


def _build(has_qbias, has_projb, has_fc2b, has_c1=True):
    import concourse.bass as bass
    import concourse.tile as tile
    from concourse import mybir

    _patch_tile(tile, bass)

    f32 = mybir.dt.float32
    f32r = mybir.dt.float32r
    bf16 = mybir.dt.bfloat16
    AF = mybir.ActivationFunctionType
    ALU = mybir.AluOpType

    nc = bass.Bass()

    # ---- DRAM I/O ----
    x_d = nc.dram_tensor("x", (C, H, W), f32, kind="ExternalInput")
    wq_d = nc.dram_tensor("wqkv", (C, 3 * C), f32, kind="ExternalInput")   # lhsT
    c0_d = nc.dram_tensor("c0", (3 * C, 1), f32, kind="ExternalInput")
    wp_d = nc.dram_tensor("wproj", (128, 3 * C), f32, kind="ExternalInput")  # band lhsT
    pb_d = nc.dram_tensor("projb", (C, 1), f32, kind="ExternalInput")
    w1_d = nc.dram_tensor("w1", (C, MLPH), f32, kind="ExternalInput")      # lhsT
    c1_d = nc.dram_tensor("c1", (MLPH, 1), f32, kind="ExternalInput")
    w2_d = nc.dram_tensor("w2", (MLPH, C), f32, kind="ExternalInput")      # lhsT
    b2_d = nc.dram_tensor("b2", (C, 1), f32, kind="ExternalInput")
    repl_d = nc.dram_tensor("repl", (128, 128), f32, kind="ExternalInput")
    ones_d = nc.dram_tensor("onesc", (C, 1), f32, kind="ExternalInput")

    y_d = nc.dram_tensor("y", (C, H, W), f32, kind="ExternalOutput")
    sc1_d = nc.dram_tensor("sc1", (2, N), f32, kind="ExternalOutput")
    sc2_d = nc.dram_tensor("sc2", (2, N), f32, kind="ExternalOutput")
    ab1_d = nc.dram_tensor("ab1", (2, N), bf16, kind="ExternalOutput")
    ab2_d = nc.dram_tensor("ab2", (2, N), bf16, kind="ExternalOutput")

    with tile.TileContext(nc) as tc:
        # ---------------- persistent pools ----------------
        wpool = tc.alloc_tile_pool(name="weights", bufs=1)
        wq = wpool.tile([C, 3 * C], bf16)
        nc.gpsimd.dma_start(out=wq, in_=wq_d[:, :])
        c0 = wpool.tile([C, 1], f32)            # q bias (scaled), only rows 0:C used
        if has_qbias:
            nc.sync.dma_start(out=c0, in_=c0_d[0:C, :])
        wpb = wpool.tile([128, 3, C], bf16)     # proj lhsT per band-block
        nc.gpsimd.dma_start(out=wpb, in_=wp_d[:, :].rearrange("p (a b) -> p a b", a=3))
        pb_row = wpool.tile([1, C], f32r)       # proj bias as rank-1 lhsT
        if has_projb:
            nc.sync.dma_start(out=pb_row, in_=pb_d[:, :].rearrange("a b -> b a").bitcast(f32r))
        w1 = wpool.tile([C, MLPH], bf16)
        nc.gpsimd.dma_start(out=w1, in_=w1_d[:, :])
        c1 = [wpool.tile([128, 1], f32, tag=f"c1{i}", name=f"c1{i}") for i in range(3)]
        for i in range(3):
            nc.sync.dma_start(out=c1[i], in_=c1_d[128 * i:128 * (i + 1), :])
        w2 = [wpool.tile([128, C], bf16, tag=f"w2{i}", name=f"w2{i}") for i in range(3)]
        for i in range(3):
            nc.gpsimd.dma_start(out=w2[i], in_=w2_d[128 * i:128 * (i + 1), :])
        b2t = wpool.tile([1, C], f32r)          # fc2 bias as rank-1 lhsT
        if has_fc2b:
            nc.sync.dma_start(out=b2t, in_=b2_d[:, :].rearrange("a b -> b a").bitcast(f32r))
        repl = wpool.tile([128, 128], bf16)
        nc.gpsimd.dma_start(out=repl, in_=repl_d[:, :])
        onescol = wpool.tile([C, 1], f32r)      # stats lhsT [96,1]
        nc.sync.dma_start(out=onescol, in_=ones_d[:, :].bitcast(f32r))
        onescol_bf = wpool.tile([C, 1], bf16)   # stats lhsT for bf16 rhs
        nc.gpsimd.dma_start(out=onescol_bf, in_=ones_d[:, :])
        onesrow = wpool.tile([1, CH], f32r)     # static ones row for bias rank-1
        nc.vector.memset(onesrow.bitcast(f32), 1.0)
        epst = wpool.tile([128, 1], f32)
        nc.vector.memset(epst, EPS)

        # big persistent activation tensors
        opool = tc.alloc_tile_pool(name="opool", bufs=1)
        Od = [opool.tile([128, BH * W], bf16, tag=f"od{d}", name=f"od{d}") for d in range(3)]
        apool = tc.alloc_tile_pool(name="acts", bufs=1)
        Qd = [apool.tile([128, BH, W], bf16, tag=f"qd{d}", name=f"qd{d}") for d in range(3)]
        KVp = [apool.tile([128, 2, PADR, PADC], bf16, tag=f"kvp{d}", name=f"kvp{d}")
               for d in range(3)]
        for d in range(3):
            # zero only the halo borders (interior is fully overwritten)
            nc.gpsimd.memset(KVp[d][:, :, 0:3, :], 0.0)
            nc.gpsimd.memset(KVp[d][:, :, PADR - 3:PADR, :], 0.0)
            nc.gpsimd.memset(KVp[d][:, :, 3:PADR - 3, 0:3], 0.0)
            nc.gpsimd.memset(KVp[d][:, :, 3:PADR - 3, 3 + W:PADC], 0.0)

        # ============ PH1: LN1 stats sweep ============
        with tc.tile_pool(name="ph1", bufs=3) as pool, \
             tc.tile_pool(name="ph1st", bufs=2) as stpool, \
             tc.tile_pool(name="ph1ps", bufs=2, space="PSUM") as psum:
            for g in range(NCHUNK // 4):
                xt4 = pool.tile([C, 4, CH], f32r, tag="xt")
                nc.sync.dma_start(out=xt4,
                                  in_=x_d[:, 16 * g:16 * g + 16, :].bitcast(f32r))
                xsq4 = pool.tile([C, 4, CH], f32r, tag="xsq")
                nc.vector.tensor_tensor(out=xsq4, in0=xt4.bitcast(f32),
                                        in1=xt4.bitcast(f32), op=ALU.mult)
                for hh in range(2):
                    ps = psum.tile([1, 2, CH], f32, tag="ps")
                    ps2 = psum.tile([1, 2, CH], f32, tag="ps2")
                    for i in range(2):
                        nc.tensor.matmul(ps[:, i, :], lhsT=onescol,
                                         rhs=xt4[:, 2 * hh + i, :], start=True, stop=True)
                        nc.tensor.matmul(ps2[:, i, :], lhsT=onescol,
                                         rhs=xsq4[:, 2 * hh + i, :], start=True, stop=True)
                    stg = stpool.tile([1, 2, 2 * CH], f32, tag="stg")
                    nc.scalar.copy(stg[:, 0, :], ps.rearrange("p a b -> p (a b)"))
                    nc.scalar.copy(stg[:, 1, :], ps2.rearrange("p a b -> p (a b)"))
                    off = CH * (4 * g + 2 * hh)
                    nc.sync.dma_start(out=sc1_d[0:1, off:off + 2 * CH],
                                      in_=stg[:, 0, :])
                    nc.sync.dma_start(out=sc1_d[1:2, off:off + 2 * CH],
                                      in_=stg[:, 1, :])

        # ============ stats math (shared helper) ============
        def stats_math(sc_dram, ab_dram, ab_dt):
            with tc.tile_pool(name="stm", bufs=1) as pool:
                s0 = pool.tile([128, 128], f32, tag="s0")
                s1 = pool.tile([128, 128], f32, tag="s1")
                src = sc_dram[:, :].rearrange("a b -> (a b)")
                ap0 = [[128, 128], [1, 128]]
                nc.sync.dma_start(out=s0, in_=bass.AP(tensor=src.tensor, offset=0, ap=ap0))
                nc.sync.dma_start(out=s1, in_=bass.AP(tensor=src.tensor, offset=N, ap=ap0))
                mu = pool.tile([128, 128], f32, tag="mu")
                nc.scalar.mul(out=mu, in_=s0, mul=1.0 / C)
                ex2 = pool.tile([128, 128], f32, tag="ex2")
                nc.scalar.mul(out=ex2, in_=s1, mul=1.0 / C)
                var = pool.tile([128, 128], f32, tag="var")
                nc.vector.scalar_tensor_tensor(out=var, in0=mu, scalar=-1.0, in1=mu,
                                               op0=ALU.mult, op1=ALU.mult)
                nc.vector.tensor_tensor(out=var, in0=ex2, in1=var, op=ALU.add)
                sd = pool.tile([128, 128], f32, tag="sd")
                nc.scalar.activation(out=sd, in_=var, func=AF.Sqrt, bias=epst, scale=1.0)
                rs = pool.tile([128, 128], ab_dt, tag="rs")
                with nc.allow_low_precision(reason="ln scale rows"):
                    nc.vector.reciprocal(out=rs, in_=sd)
                nb = pool.tile([128, 128], ab_dt, tag="nb")
                nc.vector.scalar_tensor_tensor(out=nb, in0=mu, scalar=-1.0, in1=rs,
                                               op0=ALU.mult, op1=ALU.mult)
                dst = ab_dram[:, :].rearrange("a b -> (a b)")
                nc.sync.dma_start(out=bass.AP(tensor=dst.tensor, offset=0, ap=[[1, N]]),
                                  in_=rs)
                nc.sync.dma_start(out=bass.AP(tensor=dst.tensor, offset=N, ap=[[1, N]]),
                                  in_=nb)

        stats_math(sc1_d, ab1_d, bf16)

        # ============ PH2: LN1 apply + qkv + scatter to Qd/KVp ============
        def k_sections(c):
            """(band, r0, r1) image-row ranges of chunk c hitting band halos."""
            lo, hi = 4 * c, 4 * c + 4
            out = []
            for b in range(NB):
                s_lo, s_hi = BH * b - 3, BH * b + BH + 3
                r0, r1 = max(lo, s_lo), min(hi, s_hi)
                if r0 < r1:
                    out.append((b, r0, r1))
            return out

        with tc.tile_pool(name="ph2", bufs=3) as pool, \
             tc.tile_pool(name="ph2ab", bufs=2) as abpool, \
             tc.tile_pool(name="ph2ps", bufs=2, space="PSUM") as psum:
            ab1_flat = ab1_d[:, :].rearrange("a b -> (a b)")
            for c in range(NCHUNK):
                g, i = c // 4, c % 4
                if i == 0:
                    xt4 = pool.tile([C, 4, CH], bf16, tag="xt2")
                    nc.gpsimd.dma_start(out=xt4, in_=x_d[:, 16 * g:16 * g + 16, :])
                    paB = abpool.tile([C, 4, CH], bf16, tag="paB")
                    nc.sync.dma_start(
                        out=paB,
                        in_=bass.AP(tensor=ab1_flat.tensor, offset=4 * CH * g,
                                    ap=[[0, C], [1, 4 * CH]]))
                    pbB = abpool.tile([C, 4, CH], bf16, tag="pbB")
                    nc.sync.dma_start(
                        out=pbB,
                        in_=bass.AP(tensor=ab1_flat.tensor, offset=N + 4 * CH * g,
                                    ap=[[0, C], [1, 4 * CH]]))
                xt = xt4[:, i, :]
                t1 = pool.tile([C, CH], bf16, tag="t1")
                nc.vector.tensor_tensor(out=t1, in0=xt, in1=paB[:, i, :], op=ALU.mult)
                xn = pool.tile([C, CH], bf16, tag="xn")
                nc.vector.tensor_tensor(out=xn, in0=t1, in1=pbB[:, i, :], op=ALU.add)

                pq = psum.tile([C, CH], f32, tag="pq")
                kv2 = psum.tile([C, 2, CH], f32, tag="kv2")
                nc.tensor.matmul(pq, lhsT=wq[:, 0:C], rhs=xn, start=True, stop=True)
                nc.tensor.matmul(kv2[:, 0, :], lhsT=wq[:, C:2 * C], rhs=xn,
                                 start=True, stop=True)
                nc.tensor.matmul(kv2[:, 1, :], lhsT=wq[:, 2 * C:3 * C], rhs=xn,
                                 start=True, stop=True)

                # stage k/v to SBUF bf16 once (Act), then scatter from SBUF
                kvs = pool.tile([C, 2, CH], bf16, tag="kvs")
                nc.scalar.copy(kvs, kv2)

                b = c // 8
                r_off = 4 * c - BH * b
                # Qd scatter: d=0 on DVE (from PSUM), d=1,2 on Act (from PSUM)
                for d in range(3):
                    dst = Qd[d][32 * b:32 * b + 32, r_off:r_off + 4, :]
                    src = pq[32 * d:32 * d + 32, :].rearrange("p (r w) -> p r w", r=4)
                    if d == 0:
                        if has_qbias:
                            nc.vector.tensor_scalar_add(
                                out=dst, in0=src,
                                scalar1=c0[32 * d:32 * d + 32, 0:1])
                        else:
                            nc.vector.tensor_copy(out=dst, in_=src)
                    else:
                        if has_qbias:
                            nc.scalar.activation(
                                out=dst, in_=src, func=AF.Identity,
                                bias=c0[32 * d:32 * d + 32, 0:1], scale=1.0)
                        else:
                            nc.scalar.copy(dst, src)
                # K/V scatter from kvs: d=0,1 DVE, d=2 Pool
                for d in range(3):
                    eng = nc.vector if d < 2 else nc.gpsimd
                    for (bb, ra, rb) in k_sections(c):
                        nrows = rb - ra
                        src = kvs[32 * d:32 * d + 32, :,
                                  (ra - 4 * c) * W:(rb - 4 * c) * W]
                        eng.tensor_copy(
                            out=KVp[d][32 * bb:32 * bb + 32, :,
                                       ra - (BH * bb - 3):rb - (BH * bb - 3), 3:3 + W],
                            in_=src.rearrange("p a (r w) -> p a r w", r=nrows))

        # ============ PH3: attention per dilation ============
        with tc.tile_pool(name="ph3", bufs=2) as pool, \
             tc.tile_pool(name="ph3f", bufs=2) as fpool, \
             tc.tile_pool(name="ph3r", bufs=1) as rpool, \
             tc.tile_pool(name="ph3acc", bufs=2) as acc, \
             tc.tile_pool(name="ph3ps", bufs=2, space="PSUM") as psum:
            for di, dil in enumerate(DILS):
                S = acc.tile([128, BH * W], bf16, tag="S")
                qv = Qd[di][:, :, :]
                kpl = KVp[di][:, 0, :, :]
                vpl = KVp[di][:, 1, :, :]
                for ti, (dr, dc) in enumerate([(i - 1, j - 1)
                                               for i in range(3) for j in range(3)]):
                    kwin = kpl[:, 3 + dr * dil:3 + dr * dil + BH,
                               3 + dc * dil:3 + dc * dil + W]
                    vwin = vpl[:, 3 + dr * dil:3 + dr * dil + BH,
                               3 + dc * dil:3 + dc * dil + W]
                    P = fpool.tile([128, BH, W], bf16, tag="P")
                    nc.vector.tensor_tensor(out=P, in0=qv, in1=kwin, op=ALU.mult)
                    Pf = P.rearrange("p r w -> p (r w)")
                    expL = fpool.tile([128, BH * W], bf16, tag="expL")
                    for half in range(2):
                        pl = psum.tile([128, 2048], f32, tag="pl")
                        for q in range(4):
                            nc.tensor.matmul(pl[:, 512 * q:512 * (q + 1)],
                                             lhsT=repl,
                                             rhs=Pf[:, 2048 * half + 512 * q:
                                                    2048 * half + 512 * (q + 1)],
                                             start=True, stop=True)
                        nc.scalar.activation(out=expL[:, 2048 * half:2048 * (half + 1)],
                                             in_=pl, func=AF.Exp)
                    ev = expL.rearrange("p (r w) -> p r w", r=BH)
                    if ti == 0:
                        nc.vector.tensor_copy(out=S, in_=expL)
                        nc.vector.tensor_tensor(
                            out=Od[di].rearrange("p (r w) -> p r w", r=BH),
                            in0=ev, in1=vwin, op=ALU.mult)
                    else:
                        seng = nc.gpsimd if ti in (2, 4, 6, 8) else nc.vector
                        seng.tensor_tensor(out=S, in0=S, in1=expL, op=ALU.add)
                        Pv = pool.tile([128, BH, W], bf16, tag="Pv")
                        nc.vector.tensor_tensor(out=Pv, in0=ev, in1=vwin, op=ALU.mult)
                        oeng = nc.gpsimd if ti in (3, 5, 7) else nc.vector
                        oeng.tensor_tensor(out=Od[di], in0=Od[di],
                                           in1=Pv.rearrange("p r w -> p (r w)"),
                                           op=ALU.add)
                rcp = rpool.tile([128, BH * W], bf16, tag="rcp")
                with nc.allow_low_precision(reason="softmax denom in bf16 is fine"):
                    nc.vector.reciprocal(out=rcp, in_=S)
                nc.vector.tensor_tensor(out=Od[di], in0=Od[di], in1=rcp, op=ALU.mult)

        apool.release()

        # ====== PH4: proj (from SBUF O tiles) + residual + LN2 stats ======
        r1pool = tc.alloc_tile_pool(name="r1p", bufs=1)
        r1 = r1pool.tile([C, N], bf16)
        with tc.tile_pool(name="ph4", bufs=3) as pool, \
             tc.tile_pool(name="ph4st", bufs=2) as stpool, \
             tc.tile_pool(name="ph4ps", bufs=2, space="PSUM") as psum, \
             tc.tile_pool(name="ph4ps2", bufs=1, space="PSUM") as psum2:
            for c in range(NCHUNK):
                g, i = c // 4, c % 4
                b = c // 8
                off = (4 * c - BH * b) * W
                if i == 0:
                    xt4 = pool.tile([C, 4, CH], f32, tag="xt4")
                    nc.sync.dma_start(out=xt4, in_=x_d[:, 16 * g:16 * g + 16, :])
                pp = psum.tile([C, CH], f32, tag="pp")
                if has_projb:
                    nc.tensor.matmul(pp, lhsT=pb_row, rhs=onesrow, start=True, stop=False)
                for d in range(3):
                    nc.tensor.matmul(pp, lhsT=wpb[32 * b:32 * b + 32, d, :],
                                     rhs=Od[d][32 * b:32 * b + 32, off:off + CH],
                                     start=(d == 0 and not has_projb),
                                     stop=(d == 2),
                                     tile_position=(32 * b, 0))
                rsl = r1[:, CH * c:CH * (c + 1)]
                nc.vector.tensor_tensor(out=rsl, in0=xt4[:, i, :], in1=pp, op=ALU.add)
                # LN2 stats inline
                if c % 2 == 0:
                    ps = psum2.tile([1, 2, CH], f32, tag="ps5")
                    ps2 = psum2.tile([1, 2, CH], f32, tag="ps52")
                xsq = pool.tile([C, CH], bf16, tag="xsq5")
                nc.vector.tensor_tensor(out=xsq, in0=rsl, in1=rsl, op=ALU.mult)
                nc.tensor.matmul(ps[:, c % 2, :], lhsT=onescol_bf, rhs=rsl,
                                 start=True, stop=True)
                nc.tensor.matmul(ps2[:, c % 2, :], lhsT=onescol_bf, rhs=xsq,
                                 start=True, stop=True)
                if c % 2 == 1:
                    stg = stpool.tile([1, 2, 2 * CH], f32, tag="stg5")
                    nc.scalar.copy(stg[:, 0, :], ps.rearrange("p a b -> p (a b)"))
                    nc.scalar.copy(stg[:, 1, :], ps2.rearrange("p a b -> p (a b)"))
                    soff = CH * (c - 1)
                    nc.sync.dma_start(out=sc2_d[0:1, soff:soff + 2 * CH],
                                      in_=stg[:, 0, :])
                    nc.sync.dma_start(out=sc2_d[1:2, soff:soff + 2 * CH],
                                      in_=stg[:, 1, :])

        stats_math(sc2_d, ab2_d, bf16)

        # ============ PH5b: MLP + residual ============
        with tc.tile_pool(name="ph5b", bufs=3) as pool, \
             tc.tile_pool(name="ph5ab", bufs=2) as abpool, \
             tc.tile_pool(name="ph5ps", bufs=2, space="PSUM") as psum:
            ab2_flat = ab2_d[:, :].rearrange("a b -> (a b)")
            for c in range(NCHUNK):
                g, i = c // 4, c % 4
                rsl = r1[:, CH * c:CH * (c + 1)]
                if i == 0:
                    paB5 = abpool.tile([C, 4, CH], bf16, tag="pa5B")
                    nc.sync.dma_start(
                        out=paB5,
                        in_=bass.AP(tensor=ab2_flat.tensor, offset=4 * CH * g,
                                    ap=[[0, C], [1, 4 * CH]]))
                    pbB5 = abpool.tile([C, 4, CH], bf16, tag="pb5B")
                    nc.sync.dma_start(
                        out=pbB5,
                        in_=bass.AP(tensor=ab2_flat.tensor, offset=N + 4 * CH * g,
                                    ap=[[0, C], [1, 4 * CH]]))
                    yout4 = abpool.tile([C, 4, CH], f32, tag="yout4")
                t1 = pool.tile([C, CH], bf16, tag="t15")
                nc.vector.tensor_tensor(out=t1, in0=rsl,
                                        in1=paB5[:, i, :], op=ALU.mult)
                xn = pool.tile([C, CH], bf16, tag="xn5")
                nc.vector.tensor_tensor(out=xn, in0=t1, in1=pbB5[:, i, :], op=ALU.add)

                h1 = pool.tile([128, 3, CH], bf16, tag="h1")
                if has_c1:
                    for j in range(3):
                        pf = psum.tile([128, CH], f32, tag="pf")
                        nc.tensor.matmul(pf, lhsT=w1[:, 128 * j:128 * (j + 1)], rhs=xn,
                                         start=True, stop=True)
                        nc.scalar.activation(out=h1[:, j, :], in_=pf, func=AF.Gelu,
                                             bias=c1[j][:, 0:1], scale=1.0)
                else:
                    pf3 = psum.tile([128, 3, CH], f32, tag="pf3")
                    for j in range(3):
                        nc.tensor.matmul(pf3[:, j, :], lhsT=w1[:, 128 * j:128 * (j + 1)],
                                         rhs=xn, start=True, stop=True)
                    nc.scalar.activation(out=h1, in_=pf3, func=AF.Gelu)
                pm = psum.tile([C, CH], f32, tag="pm")
                if has_fc2b:
                    nc.tensor.matmul(pm, lhsT=b2t, rhs=onesrow, start=True, stop=False)
                for j in range(3):
                    nc.tensor.matmul(pm, lhsT=w2[j], rhs=h1[:, j, :],
                                     start=(j == 0 and not has_fc2b), stop=(j == 2))
                nc.vector.tensor_tensor(out=yout4[:, i, :], in0=rsl,
                                        in1=pm, op=ALU.add)
                if i == 3:
                    nc.sync.dma_start(out=y_d[:, 16 * g:16 * g + 16, :], in_=yout4)

        r1pool.release()
        opool.release()
        wpool.release()

    _split_multi_waits(nc, mybir)
    return nc


def _prep_weights(inputs):
    """Host-side weight preparation (fold LN affine, scale, transposes)."""
    qkv_w = np.asarray(inputs['qkv_w'], np.float32)       # (288, 96)
    proj_w = np.asarray(inputs['proj_w'], np.float32)     # (96, 96)
    proj_b = np.asarray(inputs['proj_b'], np.float32)
    ln1_w = np.asarray(inputs['ln1_w'], np.float32)
    ln1_b = np.asarray(inputs['ln1_b'], np.float32)
    ln2_w = np.asarray(inputs['ln2_w'], np.float32)
    ln2_b = np.asarray(inputs['ln2_b'], np.float32)
    fc1_w = np.asarray(inputs['fc1_w'], np.float32)       # (384, 96)
    fc1_b = np.asarray(inputs['fc1_b'], np.float32)
    fc2_w = np.asarray(inputs['fc2_w'], np.float32)       # (96, 384)
    fc2_b = np.asarray(inputs['fc2_b'], np.float32)

    wq = qkv_w * ln1_w[None, :]                            # (288, 96)
    c0 = qkv_w @ ln1_b                                     # (288,)
    wq[0:C] *= SCALE                                       # scale q rows
    c0[0:C] *= SCALE
    # v bias folds into proj bias; k bias cancels in softmax
    pb_eff = proj_b + proj_w @ c0[2 * C:3 * C]

    w1 = fc1_w * ln2_w[None, :]
    c1 = fc1_w @ ln2_b + fc1_b

    repl = np.zeros((128, 128), np.float32)
    for b in range(NB):
        for ch in range(GD):
            h0 = (ch // HD) * HD
            repl[32 * b + h0:32 * b + h0 + HD, 32 * b + ch] = 1.0

    # proj lhsT in per-band layout: wpb[32b+j, d, o] = proj_w[o, 32d+j]
    wpT = proj_w.T                                         # (in=96, out=96)
    wpb = np.zeros((128, 3, C), np.float32)
    for b in range(NB):
        for d in range(3):
            wpb[32 * b:32 * b + 32, d, :] = wpT[32 * d:32 * d + 32, :]

    return {
        'wqkv': np.ascontiguousarray(wq.T),                # (96, 288) lhsT
        'c0': c0.reshape(-1, 1).astype(np.float32),
        'wproj': np.ascontiguousarray(wpb.reshape(128, 3 * C)),
        'projb': pb_eff.reshape(-1, 1).astype(np.float32),
        'w1': np.ascontiguousarray(w1.T),                  # (96, 384) lhsT
        'c1': c1.reshape(-1, 1).astype(np.float32),
        'w2': np.ascontiguousarray(fc2_w.T),               # (384, 96) lhsT
        'b2': fc2_b.reshape(-1, 1).astype(np.float32),
        'repl': repl,
        'onesc': np.ones((C, 1), np.float32),
    }


def kernel(**inputs):
    from concourse.bass_utils import run_bass_kernel_spmd

    wmap = _prep_weights(inputs)
    has_qbias = bool(np.any(wmap['c0'][0:C] != 0))
    has_projb = bool(np.any(wmap['projb'] != 0))
    has_fc2b = bool(np.any(wmap['b2'] != 0))
    has_c1 = bool(np.any(wmap['c1'] != 0))
    key = ('nc', has_qbias, has_projb, has_fc2b, has_c1)

    if key not in _cache:
        t0 = time.time()
        _cache[key] = _build(has_qbias, has_projb, has_fc2b, has_c1)
        print(f"[kernel] built bass module in {time.time() - t0:.1f}s",
              file=sys.stderr)

    nc = _cache[key]
    _cache['nc'] = nc
    x = np.asarray(inputs['x'], np.float32)                # (8, 96, 128, 128)

    in_maps = []
    for b in range(B):
        m = {'x': np.ascontiguousarray(x[b])}
        m.update(wmap)
        in_maps.append(m)

    res = run_bass_kernel_spmd(nc, in_maps, core_ids=list(range(B)))
    _cache['last_exec_ns'] = res.exec_time_ns
    out = np.stack([res.results[b]['y'] for b in range(B)], axis=0)
    return out.astype(np.float32)


# revision 54
# speedup vs baseline: 1.2725x; 1.0054x over previous
"""DilateBlock kernel for 8x Trainium2 NeuronCores (Bass/Tile).

Data-parallel over batch B=8 (one image per core). Per core, the whole block
(LN1 -> qkv -> 3-dilation 3x3 neighborhood attention -> proj -> residual ->
LN2 -> MLP -> residual) runs in channels-on-partitions layout; spatial shifts
for the attention unfold live on the free dimension of zero-padded (h, w)
planes, packed 4-hbands x 32-channels across partitions.

Key tricks vs the original:
  - LN stats PSUM rows DMA'd straight to DRAM (no Act-engine strip copies).
  - K/V qkv biases eliminated exactly (K bias shifts all 9 logits equally ->
    softmax-invariant; V bias folds into the proj bias on the host).
  - K/V scatter staged once to SBUF bf16 then spread across DVE/Act/Pool.
  - Attention output kept in SBUF; proj consumes it via per-band split
    matmuls (no DRAM round-trip for the attention output).
  - PH3 elementwise work split DVE/Pool; reciprocal in bf16.
  - Zero-bias specialization: bias ops are only emitted when the actual
    folded bias vectors are nonzero (they are zero for this problem's
    setup_inputs), with a general fallback path.
"""
import sys
import time

sys.path.insert(0, '/opt/trn_rl_repo')

import numpy as np

# ---- problem constants (hardcoded per contract) ----
B, C, H, W = 8, 96, 128, 128
DILS = (1, 2, 3)
GD = 32                 # channels per dilation branch
HD = 16                 # head dim
NB = 4                  # h-bands packed on partitions
BH = H // NB            # rows per band = 32
N = H * W               # tokens per image
NCHUNK = 32             # token chunks of 512 (4 image rows each)
CH = N // NCHUNK        # 512
PADR = 38               # BH + 6 halo rows
PADC = 135              # W + 6 halo cols (odd pitch: even bf16 tap offsets)
EPS = 1e-5
SCALE = HD ** -0.5
MLPH = 384

_cache = {}


def _patch_tile(tile_mod, bass_mod):
    """Work around this walrus build's 1-sem-wait-per-instruction limit and
    the multi-wait tail drain."""
    from concourse.vector_clock import ScopedClock, VectorClock

    def _drain_and_barrier(self, tick_clock, wait_clock):
        vclock = tick_clock.global_clock
        n = len(vclock)
        idxs = [i for i in range(n) if vclock[i] > 0]
        for i in idxs:
            vec = [0] * n
            vec[i] = vclock[i]
            nop_inst = self.nc.sync.nop(nofuse=True)
            wait_clock.add_sem_waits(nop_inst.ins,
                                     ScopedClock({None: VectorClock(vec)}))
        self.nc.sync.drain()
        self.nc.all_engine_barrier()
        popped = self.nc._tile_sem_poison_stack.pop()
        assert popped is self._sem_poison
        self.nc.clear_and_free_semaphores(list(self.sems.allocated().values()))
        self.nc.all_engine_barrier()

    tile_mod.TileContext._drain_and_barrier = _drain_and_barrier


_ws_counter = [0]


def _split_multi_waits(nc, mybir):
    for fn in nc.m.functions:
        for blk in fn.blocks:
            insts = list(blk.instructions)
            out = []
            changed = False
            for inst in insts:
                si = inst.sync_info
                waits = list(si.on_wait) if si and si.on_wait else []
                if len(waits) > 1:
                    for w in waits[:-1]:
                        _ws_counter[0] += 1
                        out.append(mybir.InstNoOp(
                            name=f"I-ws-{_ws_counter[0]}",
                            engine=inst.engine, ins=[], outs=[],
                            sync_info=mybir.SyncInfo(on_wait=[w], on_update=[])))
                    si.on_wait = [waits[-1]]
                    changed = True
                out.append(inst)
            if changed:
                blk.instructions[:] = out


def _build(has_qbias, has_projb, has_fc2b, has_c1=True):
    import concourse.bass as bass
    import concourse.tile as tile
    from concourse import mybir

    _patch_tile(tile, bass)

    f32 = mybir.dt.float32
    f32r = mybir.dt.float32r
    bf16 = mybir.dt.bfloat16
    AF = mybir.ActivationFunctionType
    ALU = mybir.AluOpType

    nc = bass.Bass()

    # ---- DRAM I/O ----
    x_d = nc.dram_tensor("x", (C, H, W), f32, kind="ExternalInput")
    wq_d = nc.dram_tensor("wqkv", (C, 3 * C), f32, kind="ExternalInput")   # lhsT
    c0_d = nc.dram_tensor("c0", (3 * C, 1), f32, kind="ExternalInput")
    wp_d = nc.dram_tensor("wproj", (128, 3 * C), f32, kind="ExternalInput")  # band lhsT
    pb_d = nc.dram_tensor("projb", (C, 1), f32, kind="ExternalInput")
    w1_d = nc.dram_tensor("w1", (C, MLPH), f32, kind="ExternalInput")      # lhsT
    c1_d = nc.dram_tensor("c1", (MLPH, 1), f32, kind="ExternalInput")
    w2_d = nc.dram_tensor("w2", (MLPH, C), f32, kind="ExternalInput")      # lhsT
    b2_d = nc.dram_tensor("b2", (C, 1), f32, kind="ExternalInput")
    repl_d = nc.dram_tensor("repl", (128, 128), f32, kind="ExternalInput")
    ones_d = nc.dram_tensor("onesc", (C, 1), f32, kind="ExternalInput")

    y_d = nc.dram_tensor("y", (C, H, W), f32, kind="ExternalOutput")
    sc1_d = nc.dram_tensor("sc1", (2, N), f32, kind="ExternalOutput")
    sc2_d = nc.dram_tensor("sc2", (2, N), f32, kind="ExternalOutput")
    ab1_d = nc.dram_tensor("ab1", (2, N), bf16, kind="ExternalOutput")
    ab2_d = nc.dram_tensor("ab2", (2, N), bf16, kind="ExternalOutput")

    with tile.TileContext(nc) as tc:
        # ---------------- persistent pools ----------------
        wpool = tc.alloc_tile_pool(name="weights", bufs=1)
        wq = wpool.tile([C, 3 * C], bf16)
        nc.gpsimd.dma_start(out=wq, in_=wq_d[:, :])
        c0 = wpool.tile([C, 1], f32)            # q bias (scaled), only rows 0:C used
        if has_qbias:
            nc.sync.dma_start(out=c0, in_=c0_d[0:C, :])
        wpb = wpool.tile([128, 3, C], bf16)     # proj lhsT per band-block
        nc.gpsimd.dma_start(out=wpb, in_=wp_d[:, :].rearrange("p (a b) -> p a b", a=3))
        pb_row = wpool.tile([1, C], f32r)       # proj bias as rank-1 lhsT
        if has_projb:
            nc.sync.dma_start(out=pb_row, in_=pb_d[:, :].rearrange("a b -> b a").bitcast(f32r))
        w1 = wpool.tile([C, MLPH], bf16)
        nc.gpsimd.dma_start(out=w1, in_=w1_d[:, :])
        c1 = [wpool.tile([128, 1], f32, tag=f"c1{i}", name=f"c1{i}") for i in range(3)]
        for i in range(3):
            nc.sync.dma_start(out=c1[i], in_=c1_d[128 * i:128 * (i + 1), :])
        w2 = [wpool.tile([128, C], bf16, tag=f"w2{i}", name=f"w2{i}") for i in range(3)]
        for i in range(3):
            nc.gpsimd.dma_start(out=w2[i], in_=w2_d[128 * i:128 * (i + 1), :])
        b2t = wpool.tile([1, C], f32r)          # fc2 bias as rank-1 lhsT
        if has_fc2b:
            nc.sync.dma_start(out=b2t, in_=b2_d[:, :].rearrange("a b -> b a").bitcast(f32r))
        repl = wpool.tile([128, 128], bf16)
        nc.gpsimd.dma_start(out=repl, in_=repl_d[:, :])
        onescol = wpool.tile([C, 1], f32r)      # stats lhsT [96,1]
        nc.sync.dma_start(out=onescol, in_=ones_d[:, :].bitcast(f32r))
        onescol_bf = wpool.tile([C, 1], bf16)   # stats lhsT for bf16 rhs
        nc.gpsimd.dma_start(out=onescol_bf, in_=ones_d[:, :])
        onesrow = wpool.tile([1, CH], f32r)     # static ones row for bias rank-1
        nc.vector.memset(onesrow.bitcast(f32), 1.0)
        epst = wpool.tile([128, 1], f32)
        nc.vector.memset(epst, EPS)

        # big persistent activation tensors
        opool = tc.alloc_tile_pool(name="opool", bufs=1)
        Od = [opool.tile([128, BH * W], bf16, tag=f"od{d}", name=f"od{d}") for d in range(3)]
        apool = tc.alloc_tile_pool(name="acts", bufs=1)
        Qd = [apool.tile([128, BH, W], bf16, tag=f"qd{d}", name=f"qd{d}") for d in range(3)]
        KVp = [apool.tile([128, 2, PADR, PADC], bf16, tag=f"kvp{d}", name=f"kvp{d}")
               for d in range(3)]
        for d in range(3):
            # zero only the halo borders (interior is fully overwritten)
            nc.gpsimd.memset(KVp[d][:, :, 0:3, :], 0.0)
            nc.gpsimd.memset(KVp[d][:, :, PADR - 3:PADR, :], 0.0)
            nc.gpsimd.memset(KVp[d][:, :, 3:PADR - 3, 0:3], 0.0)
            nc.gpsimd.memset(KVp[d][:, :, 3:PADR - 3, 3 + W:PADC], 0.0)

        # ============ PH1: LN1 stats sweep ============
        with tc.tile_pool(name="ph1", bufs=3) as pool, \
             tc.tile_pool(name="ph1st", bufs=2) as stpool, \
             tc.tile_pool(name="ph1ps", bufs=2, space="PSUM") as psum:
            for g in range(NCHUNK // 4):
                xt4 = pool.tile([C, 4, CH], bf16, tag="xt")
                nc.gpsimd.dma_start(out=xt4, in_=x_d[:, 16 * g:16 * g + 16, :])
                xsq4 = pool.tile([C, 4, CH], bf16, tag="xsq")
                nc.vector.tensor_tensor(out=xsq4, in0=xt4, in1=xt4, op=ALU.mult)
                for hh in range(2):
                    ps = psum.tile([1, 2, CH], f32, tag="ps")
                    ps2 = psum.tile([1, 2, CH], f32, tag="ps2")
                    for i in range(2):
                        nc.tensor.matmul(ps[:, i, :], lhsT=onescol_bf,
                                         rhs=xt4[:, 2 * hh + i, :], start=True, stop=True)
                        nc.tensor.matmul(ps2[:, i, :], lhsT=onescol_bf,
                                         rhs=xsq4[:, 2 * hh + i, :], start=True, stop=True)
                    stg = stpool.tile([1, 2, 2 * CH], f32, tag="stg")
                    nc.scalar.copy(stg[:, 0, :], ps.rearrange("p a b -> p (a b)"))
                    nc.scalar.copy(stg[:, 1, :], ps2.rearrange("p a b -> p (a b)"))
                    off = CH * (4 * g + 2 * hh)
                    nc.sync.dma_start(out=sc1_d[0:1, off:off + 2 * CH],
                                      in_=stg[:, 0, :])
                    nc.sync.dma_start(out=sc1_d[1:2, off:off + 2 * CH],
                                      in_=stg[:, 1, :])

        # ============ stats math (shared helper) ============
        def stats_math(sc_dram, ab_dram, ab_dt):
            with tc.tile_pool(name="stm", bufs=1) as pool:
                s0 = pool.tile([128, 128], f32, tag="s0")
                s1 = pool.tile([128, 128], f32, tag="s1")
                src = sc_dram[:, :].rearrange("a b -> (a b)")
                ap0 = [[128, 128], [1, 128]]
                nc.sync.dma_start(out=s0, in_=bass.AP(tensor=src.tensor, offset=0, ap=ap0))
                nc.sync.dma_start(out=s1, in_=bass.AP(tensor=src.tensor, offset=N, ap=ap0))
                mu = pool.tile([128, 128], f32, tag="mu")
                nc.scalar.mul(out=mu, in_=s0, mul=1.0 / C)
                ex2 = pool.tile([128, 128], f32, tag="ex2")
                nc.scalar.mul(out=ex2, in_=s1, mul=1.0 / C)
                var = pool.tile([128, 128], f32, tag="var")
                nc.vector.scalar_tensor_tensor(out=var, in0=mu, scalar=-1.0, in1=mu,
                                               op0=ALU.mult, op1=ALU.mult)
                nc.vector.tensor_tensor(out=var, in0=ex2, in1=var, op=ALU.add)
                sd = pool.tile([128, 128], f32, tag="sd")
                nc.scalar.activation(out=sd, in_=var, func=AF.Sqrt, bias=epst, scale=1.0)
                rs = pool.tile([128, 128], ab_dt, tag="rs")
                with nc.allow_low_precision(reason="ln scale rows"):
                    nc.vector.reciprocal(out=rs, in_=sd)
                nb = pool.tile([128, 128], ab_dt, tag="nb")
                nc.vector.scalar_tensor_tensor(out=nb, in0=mu, scalar=-1.0, in1=rs,
                                               op0=ALU.mult, op1=ALU.mult)
                dst = ab_dram[:, :].rearrange("a b -> (a b)")
                nc.sync.dma_start(out=bass.AP(tensor=dst.tensor, offset=0, ap=[[1, N]]),
                                  in_=rs)
                nc.sync.dma_start(out=bass.AP(tensor=dst.tensor, offset=N, ap=[[1, N]]),
                                  in_=nb)

        stats_math(sc1_d, ab1_d, bf16)

        # ============ PH2: LN1 apply + qkv + scatter to Qd/KVp ============
        def k_sections(c):
            """(band, r0, r1) image-row ranges of chunk c hitting band halos."""
            lo, hi = 4 * c, 4 * c + 4
            out = []
            for b in range(NB):
                s_lo, s_hi = BH * b - 3, BH * b + BH + 3
                r0, r1 = max(lo, s_lo), min(hi, s_hi)
                if r0 < r1:
                    out.append((b, r0, r1))
            return out

        with tc.tile_pool(name="ph2", bufs=3) as pool, \
             tc.tile_pool(name="ph2ab", bufs=2) as abpool, \
             tc.tile_pool(name="ph2ps", bufs=2, space="PSUM") as psum:
            ab1_flat = ab1_d[:, :].rearrange("a b -> (a b)")
            for c in range(NCHUNK):
                g, i = c // 4, c % 4
                if i == 0:
                    xt4 = pool.tile([C, 4, CH], bf16, tag="xt2")
                    nc.gpsimd.dma_start(out=xt4, in_=x_d[:, 16 * g:16 * g + 16, :])
                    paB = abpool.tile([C, 4, CH], bf16, tag="paB")
                    nc.sync.dma_start(
                        out=paB,
                        in_=bass.AP(tensor=ab1_flat.tensor, offset=4 * CH * g,
                                    ap=[[0, C], [1, 4 * CH]]))
                    pbB = abpool.tile([C, 4, CH], bf16, tag="pbB")
                    nc.sync.dma_start(
                        out=pbB,
                        in_=bass.AP(tensor=ab1_flat.tensor, offset=N + 4 * CH * g,
                                    ap=[[0, C], [1, 4 * CH]]))
                xt = xt4[:, i, :]
                t1 = pool.tile([C, CH], bf16, tag="t1")
                nc.vector.tensor_tensor(out=t1, in0=xt, in1=paB[:, i, :], op=ALU.mult)
                xn = pool.tile([C, CH], bf16, tag="xn")
                nc.vector.tensor_tensor(out=xn, in0=t1, in1=pbB[:, i, :], op=ALU.add)

                pq = psum.tile([C, CH], f32, tag="pq")
                kv2 = psum.tile([C, 2, CH], f32, tag="kv2")
                nc.tensor.matmul(pq, lhsT=wq[:, 0:C], rhs=xn, start=True, stop=True)
                nc.tensor.matmul(kv2[:, 0, :], lhsT=wq[:, C:2 * C], rhs=xn,
                                 start=True, stop=True)
                nc.tensor.matmul(kv2[:, 1, :], lhsT=wq[:, 2 * C:3 * C], rhs=xn,
                                 start=True, stop=True)

                # stage k/v to SBUF bf16 once (Act), then scatter from SBUF
                kvs = pool.tile([C, 2, CH], bf16, tag="kvs")
                nc.scalar.copy(kvs, kv2)

                b = c // 8
                r_off = 4 * c - BH * b
                # Qd scatter: d=0 on DVE (from PSUM), d=1,2 on Act (from PSUM)
                for d in range(3):
                    dst = Qd[d][32 * b:32 * b + 32, r_off:r_off + 4, :]
                    src = pq[32 * d:32 * d + 32, :].rearrange("p (r w) -> p r w", r=4)
                    if d == 0:
                        if has_qbias:
                            nc.vector.tensor_scalar_add(
                                out=dst, in0=src,
                                scalar1=c0[32 * d:32 * d + 32, 0:1])
                        else:
                            nc.vector.tensor_copy(out=dst, in_=src)
                    else:
                        if has_qbias:
                            nc.scalar.activation(
                                out=dst, in_=src, func=AF.Identity,
                                bias=c0[32 * d:32 * d + 32, 0:1], scale=1.0)
                        else:
                            nc.scalar.copy(dst, src)
                # K/V scatter from kvs: d=0 DVE, d=1 split K->Act V->Pool,
                # d=2 Pool
                for d in range(3):
                    for (bb, ra, rb) in k_sections(c):
                        nrows = rb - ra
                        src = kvs[32 * d:32 * d + 32, :,
                                  (ra - 4 * c) * W:(rb - 4 * c) * W]
                        dst = KVp[d][32 * bb:32 * bb + 32, :,
                                     ra - (BH * bb - 3):rb - (BH * bb - 3), 3:3 + W]
                        srcr = src.rearrange("p a (r w) -> p a r w", r=nrows)
                        if d < 2:
                            nc.vector.tensor_copy(out=dst, in_=srcr)
                        else:
                            nc.gpsimd.tensor_copy(out=dst, in_=srcr)

        # ============ PH3: attention per dilation ============
        with tc.tile_pool(name="ph3", bufs=2) as pool, \
             tc.tile_pool(name="ph3f", bufs=2) as fpool, \
             tc.tile_pool(name="ph3r", bufs=1) as rpool, \
             tc.tile_pool(name="ph3acc", bufs=2) as acc, \
             tc.tile_pool(name="ph3ps", bufs=2, space="PSUM") as psum:
            for di, dil in enumerate(DILS):
                S = acc.tile([128, BH * W], bf16, tag="S")
                qv = Qd[di][:, :, :]
                kpl = KVp[di][:, 0, :, :]
                vpl = KVp[di][:, 1, :, :]
                for ti, (dr, dc) in enumerate([(i - 1, j - 1)
                                               for i in range(3) for j in range(3)]):
                    kwin = kpl[:, 3 + dr * dil:3 + dr * dil + BH,
                               3 + dc * dil:3 + dc * dil + W]
                    vwin = vpl[:, 3 + dr * dil:3 + dr * dil + BH,
                               3 + dc * dil:3 + dc * dil + W]
                    P = fpool.tile([128, BH, W], bf16, tag="P")
                    nc.vector.tensor_tensor(out=P, in0=qv, in1=kwin, op=ALU.mult)
                    Pf = P.rearrange("p r w -> p (r w)")
                    expL = fpool.tile([128, BH * W], bf16, tag="expL")
                    for half in range(2):
                        pl = psum.tile([128, 2048], f32, tag="pl")
                        for q in range(4):
                            nc.tensor.matmul(pl[:, 512 * q:512 * (q + 1)],
                                             lhsT=repl,
                                             rhs=Pf[:, 2048 * half + 512 * q:
                                                    2048 * half + 512 * (q + 1)],
                                             start=True, stop=True)
                        nc.scalar.activation(out=expL[:, 2048 * half:2048 * (half + 1)],
                                             in_=pl, func=AF.Exp)
                    ev = expL.rearrange("p (r w) -> p r w", r=BH)
                    if ti == 0:
                        nc.vector.tensor_copy(out=S, in_=expL)
                        nc.vector.tensor_tensor(
                            out=Od[di].rearrange("p (r w) -> p r w", r=BH),
                            in0=ev, in1=vwin, op=ALU.mult)
                    else:
                        # accumulation chains (S, O) stay on DVE; Pool gets
                        # only off-chain Pv products so slow ops don't extend
                        # the serial dependency chain
                        nc.vector.tensor_tensor(out=S, in0=S, in1=expL, op=ALU.add)
                        Pv = pool.tile([128, BH, W], bf16, tag="Pv")
                        peng = nc.gpsimd if ti in (2, 4, 6, 8) else nc.vector
                        peng.tensor_tensor(out=Pv, in0=ev, in1=vwin, op=ALU.mult)
                        nc.vector.tensor_tensor(out=Od[di], in0=Od[di],
                                                in1=Pv.rearrange("p r w -> p (r w)"),
                                                op=ALU.add)
                rcp = rpool.tile([128, BH * W], bf16, tag="rcp")
                with nc.allow_low_precision(reason="softmax denom in bf16 is fine"):
                    nc.vector.reciprocal(out=rcp, in_=S)
                nc.vector.tensor_tensor(out=Od[di], in0=Od[di], in1=rcp, op=ALU.mult)

        apool.release()

        # ====== PH4: proj (from SBUF O tiles) + residual + LN2 stats ======
        r1pool = tc.alloc_tile_pool(name="r1p", bufs=1)
        r1 = r1pool.tile([C, N], bf16)
        with tc.tile_pool(name="ph4", bufs=3) as pool, \
             tc.tile_pool(name="ph4st", bufs=2) as stpool, \
             tc.tile_pool(name="ph4ps", bufs=2, space="PSUM") as psum, \
             tc.tile_pool(name="ph4ps2", bufs=1, space="PSUM") as psum2:
            for c in range(NCHUNK):
                g, i = c // 4, c % 4
                b = c // 8
                off = (4 * c - BH * b) * W
                if i == 0:
                    xt4 = pool.tile([C, 4, CH], f32, tag="xt4")
                    nc.sync.dma_start(out=xt4, in_=x_d[:, 16 * g:16 * g + 16, :])
                pp = psum.tile([C, CH], f32, tag="pp")
                if has_projb:
                    nc.tensor.matmul(pp, lhsT=pb_row, rhs=onesrow, start=True, stop=False)
                for d in range(3):
                    nc.tensor.matmul(pp, lhsT=wpb[32 * b:32 * b + 32, d, :],
                                     rhs=Od[d][32 * b:32 * b + 32, off:off + CH],
                                     start=(d == 0 and not has_projb),
                                     stop=(d == 2),
                                     tile_position=(32 * b, 0))
                rsl = r1[:, CH * c:CH * (c + 1)]
                nc.vector.tensor_tensor(out=rsl, in0=xt4[:, i, :], in1=pp, op=ALU.add)
                # LN2 stats inline
                if c % 2 == 0:
                    ps = psum2.tile([1, 2, CH], f32, tag="ps5")
                    ps2 = psum2.tile([1, 2, CH], f32, tag="ps52")
                xsq = pool.tile([C, CH], bf16, tag="xsq5")
                nc.vector.tensor_tensor(out=xsq, in0=rsl, in1=rsl, op=ALU.mult)
                nc.tensor.matmul(ps[:, c % 2, :], lhsT=onescol_bf, rhs=rsl,
                                 start=True, stop=True)
                nc.tensor.matmul(ps2[:, c % 2, :], lhsT=onescol_bf, rhs=xsq,
                                 start=True, stop=True)
                if c % 2 == 1:
                    stg = stpool.tile([1, 2, 2 * CH], f32, tag="stg5")
                    nc.scalar.copy(stg[:, 0, :], ps.rearrange("p a b -> p (a b)"))
                    nc.scalar.copy(stg[:, 1, :], ps2.rearrange("p a b -> p (a b)"))
                    soff = CH * (c - 1)
                    nc.sync.dma_start(out=sc2_d[0:1, soff:soff + 2 * CH],
                                      in_=stg[:, 0, :])
                    nc.sync.dma_start(out=sc2_d[1:2, soff:soff + 2 * CH],
                                      in_=stg[:, 1, :])

        stats_math(sc2_d, ab2_d, bf16)

        # ============ PH5b: MLP + residual ============
        with tc.tile_pool(name="ph5b", bufs=3) as pool, \
             tc.tile_pool(name="ph5ab", bufs=2) as abpool, \
             tc.tile_pool(name="ph5ps", bufs=2, space="PSUM") as psum:
            ab2_flat = ab2_d[:, :].rearrange("a b -> (a b)")
            for c in range(NCHUNK):
                g, i = c // 4, c % 4
                rsl = r1[:, CH * c:CH * (c + 1)]
                if i == 0:
                    paB5 = abpool.tile([C, 4, CH], bf16, tag="pa5B")
                    nc.sync.dma_start(
                        out=paB5,
                        in_=bass.AP(tensor=ab2_flat.tensor, offset=4 * CH * g,
                                    ap=[[0, C], [1, 4 * CH]]))
                    pbB5 = abpool.tile([C, 4, CH], bf16, tag="pb5B")
                    nc.sync.dma_start(
                        out=pbB5,
                        in_=bass.AP(tensor=ab2_flat.tensor, offset=N + 4 * CH * g,
                                    ap=[[0, C], [1, 4 * CH]]))
                    yout4 = abpool.tile([C, 4, CH], f32, tag="yout4")
                t1 = pool.tile([C, CH], bf16, tag="t15")
                nc.gpsimd.tensor_tensor(out=t1, in0=rsl,
                                        in1=paB5[:, i, :], op=ALU.mult)
                xn = pool.tile([C, CH], bf16, tag="xn5")
                nc.gpsimd.tensor_tensor(out=xn, in0=t1, in1=pbB5[:, i, :], op=ALU.add)

                h1 = pool.tile([128, 3, CH], bf16, tag="h1")
                if has_c1:
                    for j in range(3):
                        pf = psum.tile([128, CH], f32, tag="pf")
                        nc.tensor.matmul(pf, lhsT=w1[:, 128 * j:128 * (j + 1)], rhs=xn,
                                         start=True, stop=True)
                        nc.scalar.activation(out=h1[:, j, :], in_=pf, func=AF.Gelu,
                                             bias=c1[j][:, 0:1], scale=1.0)
                else:
                    pf3 = psum.tile([128, 3, CH], f32, tag="pf3")
                    for j in range(3):
                        nc.tensor.matmul(pf3[:, j, :], lhsT=w1[:, 128 * j:128 * (j + 1)],
                                         rhs=xn, start=True, stop=True)
                    nc.scalar.activation(out=h1, in_=pf3, func=AF.Gelu)
                pm = psum.tile([C, CH], f32, tag="pm")
                if has_fc2b:
                    nc.tensor.matmul(pm, lhsT=b2t, rhs=onesrow, start=True, stop=False)
                for j in range(3):
                    nc.tensor.matmul(pm, lhsT=w2[j], rhs=h1[:, j, :],
                                     start=(j == 0 and not has_fc2b), stop=(j == 2))
                nc.vector.tensor_tensor(out=yout4[:, i, :], in0=rsl,
                                        in1=pm, op=ALU.add)
                if i == 3:
                    nc.sync.dma_start(out=y_d[:, 16 * g:16 * g + 16, :], in_=yout4)

        r1pool.release()
        opool.release()
        wpool.release()

    _split_multi_waits(nc, mybir)
    return nc


def _prep_weights(inputs):
    """Host-side weight preparation (fold LN affine, scale, transposes)."""
    qkv_w = np.asarray(inputs['qkv_w'], np.float32)       # (288, 96)
    proj_w = np.asarray(inputs['proj_w'], np.float32)     # (96, 96)
    proj_b = np.asarray(inputs['proj_b'], np.float32)
    ln1_w = np.asarray(inputs['ln1_w'], np.float32)
    ln1_b = np.asarray(inputs['ln1_b'], np.float32)
    ln2_w = np.asarray(inputs['ln2_w'], np.float32)
    ln2_b = np.asarray(inputs['ln2_b'], np.float32)
    fc1_w = np.asarray(inputs['fc1_w'], np.float32)       # (384, 96)
    fc1_b = np.asarray(inputs['fc1_b'], np.float32)
    fc2_w = np.asarray(inputs['fc2_w'], np.float32)       # (96, 384)
    fc2_b = np.asarray(inputs['fc2_b'], np.float32)

    wq = qkv_w * ln1_w[None, :]                            # (288, 96)
    c0 = qkv_w @ ln1_b                                     # (288,)
    wq[0:C] *= SCALE                                       # scale q rows
    c0[0:C] *= SCALE
    # v bias folds into proj bias; k bias cancels in softmax
    pb_eff = proj_b + proj_w @ c0[2 * C:3 * C]

    w1 = fc1_w * ln2_w[None, :]
    c1 = fc1_w @ ln2_b + fc1_b

    repl = np.zeros((128, 128), np.float32)
    for b in range(NB):
        for ch in range(GD):
            h0 = (ch // HD) * HD
            repl[32 * b + h0:32 * b + h0 + HD, 32 * b + ch] = 1.0

    # proj lhsT in per-band layout: wpb[32b+j, d, o] = proj_w[o, 32d+j]
    wpT = proj_w.T                                         # (in=96, out=96)
    wpb = np.zeros((128, 3, C), np.float32)
    for b in range(NB):
        for d in range(3):
            wpb[32 * b:32 * b + 32, d, :] = wpT[32 * d:32 * d + 32, :]

    return {
        'wqkv': np.ascontiguousarray(wq.T),                # (96, 288) lhsT
        'c0': c0.reshape(-1, 1).astype(np.float32),
        'wproj': np.ascontiguousarray(wpb.reshape(128, 3 * C)),
        'projb': pb_eff.reshape(-1, 1).astype(np.float32),
        'w1': np.ascontiguousarray(w1.T),                  # (96, 384) lhsT
        'c1': c1.reshape(-1, 1).astype(np.float32),
        'w2': np.ascontiguousarray(fc2_w.T),               # (384, 96) lhsT
        'b2': fc2_b.reshape(-1, 1).astype(np.float32),
        'repl': repl,
        'onesc': np.ones((C, 1), np.float32),
    }


def kernel(**inputs):
    from concourse.bass_utils import run_bass_kernel_spmd

    wmap = _prep_weights(inputs)
    has_qbias = bool(np.any(wmap['c0'][0:C] != 0))
    has_projb = bool(np.any(wmap['projb'] != 0))
    has_fc2b = bool(np.any(wmap['b2'] != 0))
    has_c1 = bool(np.any(wmap['c1'] != 0))
    key = ('nc', has_qbias, has_projb, has_fc2b, has_c1)

    if key not in _cache:
        t0 = time.time()
        _cache[key] = _build(has_qbias, has_projb, has_fc2b, has_c1)
        print(f"[kernel] built bass module in {time.time() - t0:.1f}s",
              file=sys.stderr)

    nc = _cache[key]
    _cache['nc'] = nc
    x = np.asarray(inputs['x'], np.float32)                # (8, 96, 128, 128)

    in_maps = []
    for b in range(B):
        m = {'x': np.ascontiguousarray(x[b])}
        m.update(wmap)
        in_maps.append(m)

    res = run_bass_kernel_spmd(nc, in_maps, core_ids=list(range(B)))
    _cache['last_exec_ns'] = res.exec_time_ns
    out = np.stack([res.results[b]['y'] for b in range(B)], axis=0)
    return out.astype(np.float32)


# revision 64
# speedup vs baseline: 1.3090x; 1.0287x over previous
"""DilateBlock kernel for 8x Trainium2 NeuronCores (Bass/Tile).

Data-parallel over batch B=8 (one image per core). Per core, the whole block
(LN1 -> qkv -> 3-dilation 3x3 neighborhood attention -> proj -> residual ->
LN2 -> MLP -> residual) runs in channels-on-partitions layout; spatial shifts
for the attention unfold live on the free dimension of zero-padded (h, w)
planes, packed 4-hbands x 32-channels across partitions.

Key tricks vs the original:
  - LN stats PSUM rows DMA'd straight to DRAM (no Act-engine strip copies).
  - K/V qkv biases eliminated exactly (K bias shifts all 9 logits equally ->
    softmax-invariant; V bias folds into the proj bias on the host).
  - K/V scatter staged once to SBUF bf16 then spread across DVE/Act/Pool.
  - Attention output kept in SBUF; proj consumes it via per-band split
    matmuls (no DRAM round-trip for the attention output).
  - PH3 elementwise work split DVE/Pool; reciprocal in bf16.
  - Zero-bias specialization: bias ops are only emitted when the actual
    folded bias vectors are nonzero (they are zero for this problem's
    setup_inputs), with a general fallback path.
"""
import sys
import time

sys.path.insert(0, '/opt/trn_rl_repo')

import numpy as np

# ---- problem constants (hardcoded per contract) ----
B, C, H, W = 8, 96, 128, 128
DILS = (1, 2, 3)
GD = 32                 # channels per dilation branch
HD = 16                 # head dim
NB = 4                  # h-bands packed on partitions
BH = H // NB            # rows per band = 32
N = H * W               # tokens per image
NCHUNK = 32             # token chunks of 512 (4 image rows each)
CH = N // NCHUNK        # 512
PADR = 38               # BH + 6 halo rows
PADC = 135              # W + 6 halo cols (odd pitch: even bf16 tap offsets)
EPS = 1e-5
SCALE = HD ** -0.5
MLPH = 384

_cache = {}


def _patch_tile(tile_mod, bass_mod):
    """Work around this walrus build's 1-sem-wait-per-instruction limit and
    the multi-wait tail drain."""
    from concourse.vector_clock import ScopedClock, VectorClock

    def _drain_and_barrier(self, tick_clock, wait_clock):
        vclock = tick_clock.global_clock
        n = len(vclock)
        idxs = [i for i in range(n) if vclock[i] > 0]
        for i in idxs:
            vec = [0] * n
            vec[i] = vclock[i]
            nop_inst = self.nc.sync.nop(nofuse=True)
            wait_clock.add_sem_waits(nop_inst.ins,
                                     ScopedClock({None: VectorClock(vec)}))
        self.nc.sync.drain()
        self.nc.all_engine_barrier()
        popped = self.nc._tile_sem_poison_stack.pop()
        assert popped is self._sem_poison
        self.nc.clear_and_free_semaphores(list(self.sems.allocated().values()))
        self.nc.all_engine_barrier()

    tile_mod.TileContext._drain_and_barrier = _drain_and_barrier


_ws_counter = [0]


def _split_multi_waits(nc, mybir):
    for fn in nc.m.functions:
        for blk in fn.blocks:
            insts = list(blk.instructions)
            out = []
            changed = False
            for inst in insts:
                si = inst.sync_info
                waits = list(si.on_wait) if si and si.on_wait else []
                if len(waits) > 1:
                    for w in waits[:-1]:
                        _ws_counter[0] += 1
                        out.append(mybir.InstNoOp(
                            name=f"I-ws-{_ws_counter[0]}",
                            engine=inst.engine, ins=[], outs=[],
                            sync_info=mybir.SyncInfo(on_wait=[w], on_update=[])))
                    si.on_wait = [waits[-1]]
                    changed = True
                out.append(inst)
            if changed:
                blk.instructions[:] = out


def _build(has_qbias, has_projb, has_fc2b, has_c1=True):
    import concourse.bass as bass
    import concourse.tile as tile
    from concourse import mybir

    _patch_tile(tile, bass)

    f32 = mybir.dt.float32
    f32r = mybir.dt.float32r
    bf16 = mybir.dt.bfloat16
    AF = mybir.ActivationFunctionType
    ALU = mybir.AluOpType

    nc = bass.Bass()

    # ---- DRAM I/O ----
    x_d = nc.dram_tensor("x", (C, H, W), f32, kind="ExternalInput")
    wq_d = nc.dram_tensor("wqkv", (C, 3 * C), f32, kind="ExternalInput")   # lhsT
    c0_d = nc.dram_tensor("c0", (3 * C, 1), f32, kind="ExternalInput")
    wp_d = nc.dram_tensor("wproj", (128, 3 * (C + 1)), f32, kind="ExternalInput")
    pb_d = nc.dram_tensor("projb", (C + 1, 1), f32, kind="ExternalInput")
    w1_d = nc.dram_tensor("w1", (C, MLPH), f32, kind="ExternalInput")      # lhsT
    c1_d = nc.dram_tensor("c1", (MLPH, 1), f32, kind="ExternalInput")
    w2_d = nc.dram_tensor("w2", (MLPH, C), f32, kind="ExternalInput")      # lhsT
    b2_d = nc.dram_tensor("b2", (C, 1), f32, kind="ExternalInput")
    repl_d = nc.dram_tensor("repl", (128, 128), f32, kind="ExternalInput")
    ones_d = nc.dram_tensor("onesc", (C, 1), f32, kind="ExternalInput")

    y_d = nc.dram_tensor("y", (C, H, W), f32, kind="ExternalOutput")
    sc1_d = nc.dram_tensor("sc1", (2, N), f32, kind="ExternalOutput")
    sc2_d = nc.dram_tensor("sc2", (2, N), f32, kind="ExternalOutput")
    ab1_d = nc.dram_tensor("ab1", (2, N), bf16, kind="ExternalOutput")
    ab2_d = nc.dram_tensor("ab2", (2, N), bf16, kind="ExternalOutput")

    with tile.TileContext(nc) as tc:
        # ---------------- persistent pools ----------------
        # Allocate weight tiles up front; only PH1's inputs are DMA'd now.
        # The heavy cast-DMAs are deferred until after PH1's x loads so the
        # gpsimd DMA queue starts streaming x immediately.
        wpool = tc.alloc_tile_pool(name="weights", bufs=1)
        wq = wpool.tile([C, 3 * C], bf16)
        c0 = wpool.tile([C, 1], f32)            # q bias (scaled), only rows 0:C used
        wpb = wpool.tile([128, 3, C + 1], bf16)  # proj lhsT per band + sum row
        pb_row = wpool.tile([1, C + 1], f32r)   # proj bias as rank-1 lhsT (+sum col)
        w1 = wpool.tile([C, MLPH], bf16)
        c1 = [wpool.tile([128, 1], f32, tag=f"c1{i}", name=f"c1{i}") for i in range(3)]
        w2 = [wpool.tile([128, C], bf16, tag=f"w2{i}", name=f"w2{i}") for i in range(3)]
        b2t = wpool.tile([1, C], f32r)          # fc2 bias as rank-1 lhsT
        repl = wpool.tile([128, 128], bf16)
        onescol = wpool.tile([C, 1], f32r)      # stats lhsT [96,1]
        nc.sync.dma_start(out=onescol, in_=ones_d[:, :].bitcast(f32r))
        onescol_bf = wpool.tile([C, 1], bf16)   # stats lhsT for bf16 rhs
        nc.gpsimd.dma_start(out=onescol_bf, in_=ones_d[:, :])
        onesrow = wpool.tile([1, CH], f32r)     # static ones row for bias rank-1
        nc.vector.memset(onesrow.bitcast(f32), 1.0)
        epst = wpool.tile([128, 1], f32)
        nc.vector.memset(epst, EPS)

        def load_weights():
            nc.gpsimd.dma_start(out=wq, in_=wq_d[:, :])
            if has_qbias:
                nc.sync.dma_start(out=c0, in_=c0_d[0:C, :])
            nc.gpsimd.dma_start(out=wpb,
                                in_=wp_d[:, :].rearrange("p (a b) -> p a b", a=3))
            if has_projb:
                nc.sync.dma_start(
                    out=pb_row,
                    in_=pb_d[:, :].rearrange("a b -> b a").bitcast(f32r))
            nc.gpsimd.dma_start(out=w1, in_=w1_d[:, :])
            for i in range(3):
                nc.sync.dma_start(out=c1[i], in_=c1_d[128 * i:128 * (i + 1), :])
            for i in range(3):
                nc.gpsimd.dma_start(out=w2[i], in_=w2_d[128 * i:128 * (i + 1), :])
            if has_fc2b:
                nc.sync.dma_start(
                    out=b2t, in_=b2_d[:, :].rearrange("a b -> b a").bitcast(f32r))
            nc.gpsimd.dma_start(out=repl, in_=repl_d[:, :])

        # big persistent activation tensors
        opool = tc.alloc_tile_pool(name="opool", bufs=1)
        Od = [opool.tile([128, BH * W], bf16, tag=f"od{d}", name=f"od{d}") for d in range(3)]
        apool = tc.alloc_tile_pool(name="acts", bufs=1)
        Qd = [apool.tile([128, BH, W], bf16, tag=f"qd{d}", name=f"qd{d}") for d in range(3)]
        KVp = [apool.tile([128, 2, PADR, PADC], bf16, tag=f"kvp{d}", name=f"kvp{d}")
               for d in range(3)]
        for d in range(3):
            # zero only the halo borders (interior is fully overwritten)
            nc.gpsimd.memset(KVp[d][:, :, 0:3, :], 0.0)
            nc.gpsimd.memset(KVp[d][:, :, PADR - 3:PADR, :], 0.0)
            nc.gpsimd.memset(KVp[d][:, :, 3:PADR - 3, 0:3], 0.0)
            nc.gpsimd.memset(KVp[d][:, :, 3:PADR - 3, 3 + W:PADC], 0.0)

        # ============ PH1: LN1 stats sweep ============
        with tc.tile_pool(name="ph1", bufs=3) as pool, \
             tc.tile_pool(name="ph1st", bufs=2) as stpool, \
             tc.tile_pool(name="ph1ps", bufs=2, space="PSUM") as psum:
            for g in range(NCHUNK // 4):
                xt4 = pool.tile([C, 4, CH], bf16, tag="xt")
                nc.gpsimd.dma_start(out=xt4, in_=x_d[:, 16 * g:16 * g + 16, :])
                xsq4 = pool.tile([C, 4, CH], bf16, tag="xsq")
                nc.vector.tensor_tensor(out=xsq4, in0=xt4, in1=xt4, op=ALU.mult)
                for hh in range(2):
                    ps = psum.tile([1, 2, CH], f32, tag="ps")
                    ps2 = psum.tile([1, 2, CH], f32, tag="ps2")
                    for i in range(2):
                        nc.tensor.matmul(ps[:, i, :], lhsT=onescol_bf,
                                         rhs=xt4[:, 2 * hh + i, :], start=True, stop=True)
                        nc.tensor.matmul(ps2[:, i, :], lhsT=onescol_bf,
                                         rhs=xsq4[:, 2 * hh + i, :], start=True, stop=True)
                    stg = stpool.tile([1, 2, 2 * CH], f32, tag="stg")
                    nc.scalar.copy(stg[:, 0, :], ps.rearrange("p a b -> p (a b)"))
                    nc.scalar.copy(stg[:, 1, :], ps2.rearrange("p a b -> p (a b)"))
                    off = CH * (4 * g + 2 * hh)
                    nc.sync.dma_start(out=sc1_d[0:1, off:off + 2 * CH],
                                      in_=stg[:, 0, :])
                    nc.sync.dma_start(out=sc1_d[1:2, off:off + 2 * CH],
                                      in_=stg[:, 1, :])

        # ============ stats math (shared helper) ============
        def stats_math(sc_dram, ab_dram, ab_dt, extra_sum=None):
            with tc.tile_pool(name="stm", bufs=1) as pool:
                s0 = pool.tile([128, 128], f32, tag="s0")
                s1 = pool.tile([128, 128], f32, tag="s1")
                src = sc_dram[:, :].rearrange("a b -> (a b)")
                ap0 = [[128, 128], [1, 128]]
                nc.sync.dma_start(out=s0, in_=bass.AP(tensor=src.tensor, offset=0, ap=ap0))
                nc.sync.dma_start(out=s1, in_=bass.AP(tensor=src.tensor, offset=N, ap=ap0))
                if extra_sum is not None:
                    sx = pool.tile([128, 128], f32, tag="sx")
                    esrc = extra_sum[:, :].rearrange("a b -> (a b)")
                    nc.sync.dma_start(out=sx, in_=bass.AP(tensor=esrc.tensor,
                                                          offset=0, ap=ap0))
                    nc.vector.tensor_tensor(out=s0, in0=s0, in1=sx, op=ALU.add)
                mu = pool.tile([128, 128], f32, tag="mu")
                nc.scalar.mul(out=mu, in_=s0, mul=1.0 / C)
                ex2 = pool.tile([128, 128], f32, tag="ex2")
                nc.scalar.mul(out=ex2, in_=s1, mul=1.0 / C)
                var = pool.tile([128, 128], f32, tag="var")
                nc.vector.scalar_tensor_tensor(out=var, in0=mu, scalar=-1.0, in1=mu,
                                               op0=ALU.mult, op1=ALU.mult)
                nc.vector.tensor_tensor(out=var, in0=ex2, in1=var, op=ALU.add)
                sd = pool.tile([128, 128], f32, tag="sd")
                nc.scalar.activation(out=sd, in_=var, func=AF.Sqrt, bias=epst, scale=1.0)
                rs = pool.tile([128, 128], ab_dt, tag="rs")
                with nc.allow_low_precision(reason="ln scale rows"):
                    nc.vector.reciprocal(out=rs, in_=sd)
                nb = pool.tile([128, 128], ab_dt, tag="nb")
                nc.vector.scalar_tensor_tensor(out=nb, in0=mu, scalar=-1.0, in1=rs,
                                               op0=ALU.mult, op1=ALU.mult)
                dst = ab_dram[:, :].rearrange("a b -> (a b)")
                nc.sync.dma_start(out=bass.AP(tensor=dst.tensor, offset=0, ap=[[1, N]]),
                                  in_=rs)
                nc.sync.dma_start(out=bass.AP(tensor=dst.tensor, offset=N, ap=[[1, N]]),
                                  in_=nb)

        load_weights()
        stats_math(sc1_d, ab1_d, bf16)

        # ============ PH2: LN1 apply + qkv + scatter to Qd/KVp ============
        def k_sections(c):
            """(band, r0, r1) image-row ranges of chunk c hitting band halos."""
            lo, hi = 4 * c, 4 * c + 4
            out = []
            for b in range(NB):
                s_lo, s_hi = BH * b - 3, BH * b + BH + 3
                r0, r1 = max(lo, s_lo), min(hi, s_hi)
                if r0 < r1:
                    out.append((b, r0, r1))
            return out

        with tc.tile_pool(name="ph2", bufs=3) as pool, \
             tc.tile_pool(name="ph2ab", bufs=2) as abpool, \
             tc.tile_pool(name="ph2ps", bufs=2, space="PSUM") as psum:
            ab1_flat = ab1_d[:, :].rearrange("a b -> (a b)")
            for c in range(NCHUNK):
                g, i = c // 4, c % 4
                if i == 0:
                    xt4 = pool.tile([C, 4, CH], bf16, tag="xt2")
                    nc.gpsimd.dma_start(out=xt4, in_=x_d[:, 16 * g:16 * g + 16, :])
                    paB = abpool.tile([C, 4, CH], bf16, tag="paB")
                    nc.sync.dma_start(
                        out=paB,
                        in_=bass.AP(tensor=ab1_flat.tensor, offset=4 * CH * g,
                                    ap=[[0, C], [1, 4 * CH]]))
                    pbB = abpool.tile([C, 4, CH], bf16, tag="pbB")
                    nc.sync.dma_start(
                        out=pbB,
                        in_=bass.AP(tensor=ab1_flat.tensor, offset=N + 4 * CH * g,
                                    ap=[[0, C], [1, 4 * CH]]))
                xt = xt4[:, i, :]
                t1 = pool.tile([C, CH], bf16, tag="t1")
                nc.vector.tensor_tensor(out=t1, in0=xt, in1=paB[:, i, :], op=ALU.mult)
                xn = pool.tile([C, CH], bf16, tag="xn")
                nc.vector.tensor_tensor(out=xn, in0=t1, in1=pbB[:, i, :], op=ALU.add)

                pq = psum.tile([C, CH], f32, tag="pq")
                kv2 = psum.tile([C, 2, CH], f32, tag="kv2")
                nc.tensor.matmul(pq, lhsT=wq[:, 0:C], rhs=xn, start=True, stop=True)
                nc.tensor.matmul(kv2[:, 0, :], lhsT=wq[:, C:2 * C], rhs=xn,
                                 start=True, stop=True)
                nc.tensor.matmul(kv2[:, 1, :], lhsT=wq[:, 2 * C:3 * C], rhs=xn,
                                 start=True, stop=True)

                # stage k/v to SBUF bf16 once (Act), then scatter from SBUF
                kvs = pool.tile([C, 2, CH], bf16, tag="kvs")
                nc.scalar.copy(kvs, kv2)

                b = c // 8
                r_off = 4 * c - BH * b
                # Qd scatter: d=0 on DVE (from PSUM), d=1,2 on Act (from PSUM)
                for d in range(3):
                    dst = Qd[d][32 * b:32 * b + 32, r_off:r_off + 4, :]
                    src = pq[32 * d:32 * d + 32, :].rearrange("p (r w) -> p r w", r=4)
                    if d == 0:
                        if has_qbias:
                            nc.vector.tensor_scalar_add(
                                out=dst, in0=src,
                                scalar1=c0[32 * d:32 * d + 32, 0:1])
                        else:
                            nc.vector.tensor_copy(out=dst, in_=src)
                    else:
                        if has_qbias:
                            nc.scalar.activation(
                                out=dst, in_=src, func=AF.Identity,
                                bias=c0[32 * d:32 * d + 32, 0:1], scale=1.0)
                        else:
                            nc.scalar.copy(dst, src)
                # K/V scatter from kvs: d=0 DVE, d=1 split K->Act V->Pool,
                # d=2 Pool
                for d in range(3):
                    for (bb, ra, rb) in k_sections(c):
                        nrows = rb - ra
                        src = kvs[32 * d:32 * d + 32, :,
                                  (ra - 4 * c) * W:(rb - 4 * c) * W]
                        dst = KVp[d][32 * bb:32 * bb + 32, :,
                                     ra - (BH * bb - 3):rb - (BH * bb - 3), 3:3 + W]
                        srcr = src.rearrange("p a (r w) -> p a r w", r=nrows)
                        if d < 2:
                            nc.vector.tensor_copy(out=dst, in_=srcr)
                        else:
                            nc.gpsimd.tensor_copy(out=dst, in_=srcr)

        # ============ PH3: attention per dilation ============
        with tc.tile_pool(name="ph3", bufs=2) as pool, \
             tc.tile_pool(name="ph3p", bufs=3) as ppool, \
             tc.tile_pool(name="ph3f", bufs=2) as fpool, \
             tc.tile_pool(name="ph3r", bufs=1) as rpool, \
             tc.tile_pool(name="ph3acc", bufs=1) as acc, \
             tc.tile_pool(name="ph3ps", bufs=2, space="PSUM") as psum:
            for di, dil in enumerate(DILS):
                S = acc.tile([128, BH * W], bf16, tag="S")
                qv = Qd[di][:, :, :]
                kpl = KVp[di][:, 0, :, :]
                vpl = KVp[di][:, 1, :, :]
                for ti, (dr, dc) in enumerate([(i - 1, j - 1)
                                               for i in range(3) for j in range(3)]):
                    kwin = kpl[:, 3 + dr * dil:3 + dr * dil + BH,
                               3 + dc * dil:3 + dc * dil + W]
                    vwin = vpl[:, 3 + dr * dil:3 + dr * dil + BH,
                               3 + dc * dil:3 + dc * dil + W]
                    P = ppool.tile([128, BH, W], bf16, tag="P")
                    nc.vector.tensor_tensor(out=P, in0=qv, in1=kwin, op=ALU.mult)
                    Pf = P.rearrange("p r w -> p (r w)")
                    expL = fpool.tile([128, BH * W], bf16, tag="expL")
                    for half in range(2):
                        pl = psum.tile([128, 2048], f32, tag="pl")
                        for q in range(4):
                            nc.tensor.matmul(pl[:, 512 * q:512 * (q + 1)],
                                             lhsT=repl,
                                             rhs=Pf[:, 2048 * half + 512 * q:
                                                    2048 * half + 512 * (q + 1)],
                                             start=True, stop=True)
                        nc.scalar.activation(out=expL[:, 2048 * half:2048 * (half + 1)],
                                             in_=pl, func=AF.Exp)
                    ev = expL.rearrange("p (r w) -> p r w", r=BH)
                    if ti == 0:
                        nc.vector.tensor_copy(out=S, in_=expL)
                        nc.vector.tensor_tensor(
                            out=Od[di].rearrange("p (r w) -> p r w", r=BH),
                            in0=ev, in1=vwin, op=ALU.mult)
                    else:
                        # accumulation chains (S, O) stay on DVE; Pool gets
                        # only off-chain Pv products so slow ops don't extend
                        # the serial dependency chain
                        nc.vector.tensor_tensor(out=S, in0=S, in1=expL, op=ALU.add)
                        Pv = pool.tile([128, BH, W], bf16, tag="Pv")
                        peng = nc.gpsimd if ti in (2, 4, 6, 8) else nc.vector
                        peng.tensor_tensor(out=Pv, in0=ev, in1=vwin, op=ALU.mult)
                        nc.vector.tensor_tensor(out=Od[di], in0=Od[di],
                                                in1=Pv.rearrange("p r w -> p (r w)"),
                                                op=ALU.add)
                rcp = rpool.tile([128, BH * W], bf16, tag="rcp")
                with nc.allow_low_precision(reason="softmax denom in bf16 is fine"):
                    nc.vector.reciprocal(out=rcp, in_=S)
                nc.vector.tensor_tensor(out=Od[di], in0=Od[di], in1=rcp, op=ALU.mult)

        apool.release()

        # ====== PH4: proj (from SBUF O tiles) + residual + LN2 stats ======
        r1pool = tc.alloc_tile_pool(name="r1p", bufs=1)
        r1 = r1pool.tile([C, N], bf16)
        with tc.tile_pool(name="ph4", bufs=3) as pool, \
             tc.tile_pool(name="ph4st", bufs=2) as stpool, \
             tc.tile_pool(name="ph4ps", bufs=2, space="PSUM") as psum, \
             tc.tile_pool(name="ph4ps2", bufs=1, space="PSUM") as psum2:
            for c in range(NCHUNK):
                g, i = c // 4, c % 4
                b = c // 8
                off = (4 * c - BH * b) * W
                if i == 0:
                    xt4 = pool.tile([C, 4, CH], f32, tag="xt4")
                    nc.sync.dma_start(out=xt4, in_=x_d[:, 16 * g:16 * g + 16, :])
                # proj matmul carries an extra output row: the column-sums of
                # proj_w, so row C of pp = sum_c proj_out[c, n]. Combined with
                # PH1's x-sums (sc1 row 0) in stats_math this gives the LN2
                # token sums without a dedicated matmul.
                pp = psum.tile([C + 1, CH], f32, tag="pp")
                if has_projb:
                    nc.tensor.matmul(pp, lhsT=pb_row, rhs=onesrow, start=True, stop=False)
                for d in range(3):
                    nc.tensor.matmul(pp, lhsT=wpb[32 * b:32 * b + 32, d, :],
                                     rhs=Od[d][32 * b:32 * b + 32, off:off + CH],
                                     start=(d == 0 and not has_projb),
                                     stop=(d == 2),
                                     tile_position=(32 * b, 0))
                rsl = r1[:, CH * c:CH * (c + 1)]
                nc.vector.tensor_tensor(out=rsl, in0=xt4[:, i, :], in1=pp[0:C, :],
                                        op=ALU.add)
                # LN2 stats inline
                if c % 2 == 0:
                    ps2 = psum2.tile([1, 2, CH], f32, tag="ps52")
                    stg = stpool.tile([1, 2, 2 * CH], f32, tag="stg5")
                nc.scalar.copy(stg[:, 0, (c % 2) * CH:(c % 2 + 1) * CH],
                               pp[C:C + 1, :])
                xsq = pool.tile([C, CH], bf16, tag="xsq5")
                nc.vector.tensor_tensor(out=xsq, in0=rsl, in1=rsl, op=ALU.mult)
                nc.tensor.matmul(ps2[:, c % 2, :], lhsT=onescol_bf, rhs=xsq,
                                 start=True, stop=True)
                if c % 2 == 1:
                    nc.scalar.copy(stg[:, 1, :], ps2.rearrange("p a b -> p (a b)"))
                    soff = CH * (c - 1)
                    nc.sync.dma_start(out=sc2_d[0:1, soff:soff + 2 * CH],
                                      in_=stg[:, 0, :])
                    nc.sync.dma_start(out=sc2_d[1:2, soff:soff + 2 * CH],
                                      in_=stg[:, 1, :])

        stats_math(sc2_d, ab2_d, bf16, extra_sum=sc1_d)

        # ============ PH5b: MLP + residual ============
        with tc.tile_pool(name="ph5b", bufs=3) as pool, \
             tc.tile_pool(name="ph5ab", bufs=2) as abpool, \
             tc.tile_pool(name="ph5ps", bufs=2, space="PSUM") as psum:
            ab2_flat = ab2_d[:, :].rearrange("a b -> (a b)")
            for c in range(NCHUNK):
                g, i = c // 4, c % 4
                rsl = r1[:, CH * c:CH * (c + 1)]
                if i == 0:
                    paB5 = abpool.tile([C, 4, CH], bf16, tag="pa5B")
                    nc.sync.dma_start(
                        out=paB5,
                        in_=bass.AP(tensor=ab2_flat.tensor, offset=4 * CH * g,
                                    ap=[[0, C], [1, 4 * CH]]))
                    pbB5 = abpool.tile([C, 4, CH], bf16, tag="pb5B")
                    nc.sync.dma_start(
                        out=pbB5,
                        in_=bass.AP(tensor=ab2_flat.tensor, offset=N + 4 * CH * g,
                                    ap=[[0, C], [1, 4 * CH]]))
                    yout4 = abpool.tile([C, 4, CH], f32, tag="yout4")
                t1 = pool.tile([C, CH], bf16, tag="t15")
                nc.gpsimd.tensor_tensor(out=t1, in0=rsl,
                                        in1=paB5[:, i, :], op=ALU.mult)
                xn = pool.tile([C, CH], bf16, tag="xn5")
                nc.gpsimd.tensor_tensor(out=xn, in0=t1, in1=pbB5[:, i, :], op=ALU.add)

                h1 = pool.tile([128, 3, CH], bf16, tag="h1")
                if has_c1:
                    for j in range(3):
                        pf = psum.tile([128, CH], f32, tag="pf")
                        nc.tensor.matmul(pf, lhsT=w1[:, 128 * j:128 * (j + 1)], rhs=xn,
                                         start=True, stop=True)
                        nc.scalar.activation(out=h1[:, j, :], in_=pf, func=AF.Gelu,
                                             bias=c1[j][:, 0:1], scale=1.0)
                else:
                    pf3 = psum.tile([128, 3, CH], f32, tag="pf3")
                    for j in range(3):
                        nc.tensor.matmul(pf3[:, j, :], lhsT=w1[:, 128 * j:128 * (j + 1)],
                                         rhs=xn, start=True, stop=True)
                    nc.scalar.activation(out=h1, in_=pf3, func=AF.Gelu)
                pm = psum.tile([C, CH], f32, tag="pm")
                if has_fc2b:
                    nc.tensor.matmul(pm, lhsT=b2t, rhs=onesrow, start=True, stop=False)
                for j in range(3):
                    nc.tensor.matmul(pm, lhsT=w2[j], rhs=h1[:, j, :],
                                     start=(j == 0 and not has_fc2b), stop=(j == 2))
                nc.vector.tensor_tensor(out=yout4[:, i, :], in0=rsl,
                                        in1=pm, op=ALU.add)
                if i == 3:
                    nc.sync.dma_start(out=y_d[:, 16 * g:16 * g + 16, :], in_=yout4)

        r1pool.release()
        opool.release()
        wpool.release()

    _split_multi_waits(nc, mybir)
    return nc


def _prep_weights(inputs):
    """Host-side weight preparation (fold LN affine, scale, transposes)."""
    qkv_w = np.asarray(inputs['qkv_w'], np.float32)       # (288, 96)
    proj_w = np.asarray(inputs['proj_w'], np.float32)     # (96, 96)
    proj_b = np.asarray(inputs['proj_b'], np.float32)
    ln1_w = np.asarray(inputs['ln1_w'], np.float32)
    ln1_b = np.asarray(inputs['ln1_b'], np.float32)
    ln2_w = np.asarray(inputs['ln2_w'], np.float32)
    ln2_b = np.asarray(inputs['ln2_b'], np.float32)
    fc1_w = np.asarray(inputs['fc1_w'], np.float32)       # (384, 96)
    fc1_b = np.asarray(inputs['fc1_b'], np.float32)
    fc2_w = np.asarray(inputs['fc2_w'], np.float32)       # (96, 384)
    fc2_b = np.asarray(inputs['fc2_b'], np.float32)

    wq = qkv_w * ln1_w[None, :]                            # (288, 96)
    c0 = qkv_w @ ln1_b                                     # (288,)
    wq[0:C] *= SCALE                                       # scale q rows
    c0[0:C] *= SCALE
    # v bias folds into proj bias; k bias cancels in softmax
    pb_eff = proj_b + proj_w @ c0[2 * C:3 * C]

    w1 = fc1_w * ln2_w[None, :]
    c1 = fc1_w @ ln2_b + fc1_b

    repl = np.zeros((128, 128), np.float32)
    for b in range(NB):
        for ch in range(GD):
            h0 = (ch // HD) * HD
            repl[32 * b + h0:32 * b + h0 + HD, 32 * b + ch] = 1.0

    # proj lhsT in per-band layout: wpb[32b+j, d, o] = proj_w[o, 32d+j];
    # extra column C holds proj_w column-sums so the matmul also emits the
    # per-token sum of the proj output (feeds LN2 stats).
    wpT = proj_w.T                                         # (in=96, out=96)
    colsum = proj_w.sum(axis=0)                            # (96,)
    wpb = np.zeros((128, 3, C + 1), np.float32)
    for b in range(NB):
        for d in range(3):
            wpb[32 * b:32 * b + 32, d, 0:C] = wpT[32 * d:32 * d + 32, :]
            wpb[32 * b:32 * b + 32, d, C] = colsum[32 * d:32 * d + 32]
    pb_ext = np.concatenate([pb_eff, [pb_eff.sum()]]).astype(np.float32)

    return {
        'wqkv': np.ascontiguousarray(wq.T),                # (96, 288) lhsT
        'c0': c0.reshape(-1, 1).astype(np.float32),
        'wproj': np.ascontiguousarray(wpb.reshape(128, 3 * (C + 1))),
        'projb': pb_ext.reshape(-1, 1),
        'w1': np.ascontiguousarray(w1.T),                  # (96, 384) lhsT
        'c1': c1.reshape(-1, 1).astype(np.float32),
        'w2': np.ascontiguousarray(fc2_w.T),               # (384, 96) lhsT
        'b2': fc2_b.reshape(-1, 1).astype(np.float32),
        'repl': repl,
        'onesc': np.ones((C, 1), np.float32),
    }


def kernel(**inputs):
    from concourse.bass_utils import run_bass_kernel_spmd

    wmap = _prep_weights(inputs)
    has_qbias = bool(np.any(wmap['c0'][0:C] != 0))
    has_projb = bool(np.any(wmap['projb'] != 0))
    has_fc2b = bool(np.any(wmap['b2'] != 0))
    has_c1 = bool(np.any(wmap['c1'] != 0))
    key = ('nc', has_qbias, has_projb, has_fc2b, has_c1)

    if key not in _cache:
        t0 = time.time()
        _cache[key] = _build(has_qbias, has_projb, has_fc2b, has_c1)
        print(f"[kernel] built bass module in {time.time() - t0:.1f}s",
              file=sys.stderr)

    nc = _cache[key]
    _cache['nc'] = nc
    x = np.asarray(inputs['x'], np.float32)                # (8, 96, 128, 128)

    in_maps = []
    for b in range(B):
        m = {'x': np.ascontiguousarray(x[b])}
        m.update(wmap)
        in_maps.append(m)

    res = run_bass_kernel_spmd(nc, in_maps, core_ids=list(range(B)))
    _cache['last_exec_ns'] = res.exec_time_ns
    out = np.stack([res.results[b]['y'] for b in range(B)], axis=0)
    return out.astype(np.float32)


# revision 70
# speedup vs baseline: 1.3324x; 1.0179x over previous
"""DilateBlock kernel for 8x Trainium2 NeuronCores (Bass/Tile).

Data-parallel over batch B=8 (one image per core). Per core, the whole block
(LN1 -> qkv -> 3-dilation 3x3 neighborhood attention -> proj -> residual ->
LN2 -> MLP -> residual) runs in channels-on-partitions layout; spatial shifts
for the attention unfold live on the free dimension of zero-padded (h, w)
planes, packed 4-hbands x 32-channels across partitions.

Key tricks vs the original:
  - LN stats PSUM rows DMA'd straight to DRAM (no Act-engine strip copies).
  - K/V qkv biases eliminated exactly (K bias shifts all 9 logits equally ->
    softmax-invariant; V bias folds into the proj bias on the host).
  - K/V scatter staged once to SBUF bf16 then spread across DVE/Act/Pool.
  - Attention output kept in SBUF; proj consumes it via per-band split
    matmuls (no DRAM round-trip for the attention output).
  - PH3 elementwise work split DVE/Pool; reciprocal in bf16.
  - Zero-bias specialization: bias ops are only emitted when the actual
    folded bias vectors are nonzero (they are zero for this problem's
    setup_inputs), with a general fallback path.
"""
import sys
import time

sys.path.insert(0, '/opt/trn_rl_repo')

import numpy as np

# ---- problem constants (hardcoded per contract) ----
B, C, H, W = 8, 96, 128, 128
DILS = (1, 2, 3)
GD = 32                 # channels per dilation branch
HD = 16                 # head dim
NB = 4                  # h-bands packed on partitions
BH = H // NB            # rows per band = 32
N = H * W               # tokens per image
NCHUNK = 32             # token chunks of 512 (4 image rows each)
CH = N // NCHUNK        # 512
PADR = 38               # BH + 6 halo rows
PADC = 135              # W + 6 halo cols (odd pitch: even bf16 tap offsets)
EPS = 1e-5
SCALE = HD ** -0.5
MLPH = 384

_cache = {}


def _patch_tile(tile_mod, bass_mod):
    """Work around this walrus build's 1-sem-wait-per-instruction limit and
    the multi-wait tail drain."""
    from concourse.vector_clock import ScopedClock, VectorClock

    def _drain_and_barrier(self, tick_clock, wait_clock):
        vclock = tick_clock.global_clock
        n = len(vclock)
        idxs = [i for i in range(n) if vclock[i] > 0]
        for i in idxs:
            vec = [0] * n
            vec[i] = vclock[i]
            nop_inst = self.nc.sync.nop(nofuse=True)
            wait_clock.add_sem_waits(nop_inst.ins,
                                     ScopedClock({None: VectorClock(vec)}))
        self.nc.sync.drain()
        self.nc.all_engine_barrier()
        popped = self.nc._tile_sem_poison_stack.pop()
        assert popped is self._sem_poison
        self.nc.clear_and_free_semaphores(list(self.sems.allocated().values()))
        self.nc.all_engine_barrier()

    tile_mod.TileContext._drain_and_barrier = _drain_and_barrier


_ws_counter = [0]


def _split_multi_waits(nc, mybir):
    for fn in nc.m.functions:
        for blk in fn.blocks:
            insts = list(blk.instructions)
            out = []
            changed = False
            for inst in insts:
                si = inst.sync_info
                waits = list(si.on_wait) if si and si.on_wait else []
                if len(waits) > 1:
                    for w in waits[:-1]:
                        _ws_counter[0] += 1
                        out.append(mybir.InstNoOp(
                            name=f"I-ws-{_ws_counter[0]}",
                            engine=inst.engine, ins=[], outs=[],
                            sync_info=mybir.SyncInfo(on_wait=[w], on_update=[])))
                    si.on_wait = [waits[-1]]
                    changed = True
                out.append(inst)
            if changed:
                blk.instructions[:] = out


def _build(has_qbias, has_projb, has_fc2b, has_c1=True):
    import concourse.bass as bass
    import concourse.tile as tile
    from concourse import mybir

    _patch_tile(tile, bass)

    f32 = mybir.dt.float32
    f32r = mybir.dt.float32r
    bf16 = mybir.dt.bfloat16
    AF = mybir.ActivationFunctionType
    ALU = mybir.AluOpType

    nc = bass.Bass()

    # ---- DRAM I/O ----
    x_d = nc.dram_tensor("x", (C, H, W), f32, kind="ExternalInput")
    wq_d = nc.dram_tensor("wqkv", (C, 3 * C), f32, kind="ExternalInput")   # lhsT
    c0_d = nc.dram_tensor("c0", (3 * C, 1), f32, kind="ExternalInput")
    wp_d = nc.dram_tensor("wproj", (128, 3 * (C + 1)), f32, kind="ExternalInput")
    pb_d = nc.dram_tensor("projb", (C + 1, 1), f32, kind="ExternalInput")
    w1_d = nc.dram_tensor("w1", (C, MLPH), f32, kind="ExternalInput")      # lhsT
    c1_d = nc.dram_tensor("c1", (MLPH, 1), f32, kind="ExternalInput")
    w2_d = nc.dram_tensor("w2", (MLPH, C), f32, kind="ExternalInput")      # lhsT
    b2_d = nc.dram_tensor("b2", (C, 1), f32, kind="ExternalInput")
    repl_d = nc.dram_tensor("repl", (128, 128), f32, kind="ExternalInput")
    ones_d = nc.dram_tensor("onesc", (C, 1), f32, kind="ExternalInput")

    y_d = nc.dram_tensor("y", (C, H, W), f32, kind="ExternalOutput")
    sc1_d = nc.dram_tensor("sc1", (2, N), f32, kind="ExternalOutput")
    sc2_d = nc.dram_tensor("sc2", (2, N), f32, kind="ExternalOutput")
    ab1_d = nc.dram_tensor("ab1", (2, N), bf16, kind="ExternalOutput")
    ab2_d = nc.dram_tensor("ab2", (2, N), bf16, kind="ExternalOutput")

    with tile.TileContext(nc) as tc:
        # ---------------- persistent pools ----------------
        # Allocate weight tiles up front; only PH1's inputs are DMA'd now.
        # The heavy cast-DMAs are deferred until after PH1's x loads so the
        # gpsimd DMA queue starts streaming x immediately.
        wpool = tc.alloc_tile_pool(name="weights", bufs=1)
        wq = wpool.tile([C, 3 * C], bf16)
        c0 = wpool.tile([C, 1], f32)            # q bias (scaled), only rows 0:C used
        wpb = wpool.tile([128, 3, C + 1], bf16)  # proj lhsT per band + sum row
        pb_row = wpool.tile([1, C + 1], f32r)   # proj bias as rank-1 lhsT (+sum col)
        w1 = wpool.tile([C, MLPH], bf16)
        c1 = [wpool.tile([128, 1], f32, tag=f"c1{i}", name=f"c1{i}") for i in range(3)]
        w2 = [wpool.tile([128, C], bf16, tag=f"w2{i}", name=f"w2{i}") for i in range(3)]
        b2t = wpool.tile([1, C], f32r)          # fc2 bias as rank-1 lhsT
        repl = wpool.tile([128, 128], bf16)
        onescol = wpool.tile([C, 1], f32r)      # stats lhsT [96,1]
        nc.sync.dma_start(out=onescol, in_=ones_d[:, :].bitcast(f32r))
        onescol_bf = wpool.tile([C, 1], bf16)   # stats lhsT for bf16 rhs
        nc.gpsimd.dma_start(out=onescol_bf, in_=ones_d[:, :])
        onesrow = wpool.tile([1, CH], f32r)     # static ones row for bias rank-1
        nc.vector.memset(onesrow.bitcast(f32), 1.0)
        epst = wpool.tile([128, 1], f32)
        nc.vector.memset(epst, EPS)

        def load_weights():
            nc.gpsimd.dma_start(out=wq, in_=wq_d[:, :])
            if has_qbias:
                nc.sync.dma_start(out=c0, in_=c0_d[0:C, :])
            nc.gpsimd.dma_start(out=wpb,
                                in_=wp_d[:, :].rearrange("p (a b) -> p a b", a=3))
            if has_projb:
                nc.sync.dma_start(
                    out=pb_row,
                    in_=pb_d[:, :].rearrange("a b -> b a").bitcast(f32r))
            nc.gpsimd.dma_start(out=w1, in_=w1_d[:, :])
            for i in range(3):
                nc.sync.dma_start(out=c1[i], in_=c1_d[128 * i:128 * (i + 1), :])
            for i in range(3):
                nc.gpsimd.dma_start(out=w2[i], in_=w2_d[128 * i:128 * (i + 1), :])
            if has_fc2b:
                nc.sync.dma_start(
                    out=b2t, in_=b2_d[:, :].rearrange("a b -> b a").bitcast(f32r))
            nc.gpsimd.dma_start(out=repl, in_=repl_d[:, :])

        # big persistent activation tensors
        opool = tc.alloc_tile_pool(name="opool", bufs=1)
        Od = [opool.tile([128, BH * W], bf16, tag=f"od{d}", name=f"od{d}") for d in range(3)]
        apool = tc.alloc_tile_pool(name="acts", bufs=1)
        Qd = [apool.tile([128, BH, W], bf16, tag=f"qd{d}", name=f"qd{d}") for d in range(3)]
        KVp = [apool.tile([128, 2, PADR, PADC], bf16, tag=f"kvp{d}", name=f"kvp{d}")
               for d in range(3)]
        for d in range(3):
            # zero only the halo borders (interior is fully overwritten)
            nc.gpsimd.memset(KVp[d][:, :, 0:3, :], 0.0)
            nc.gpsimd.memset(KVp[d][:, :, PADR - 3:PADR, :], 0.0)
            nc.gpsimd.memset(KVp[d][:, :, 3:PADR - 3, 0:3], 0.0)
            nc.gpsimd.memset(KVp[d][:, :, 3:PADR - 3, 3 + W:PADC], 0.0)

        # ============ PH1: LN1 stats sweep ============
        with tc.tile_pool(name="ph1", bufs=3) as pool, \
             tc.tile_pool(name="ph1st", bufs=2) as stpool, \
             tc.tile_pool(name="ph1ps", bufs=2, space="PSUM") as psum:
            for g in range(NCHUNK // 4):
                xt4 = pool.tile([C, 4, CH], bf16, tag="xt")
                nc.gpsimd.dma_start(out=xt4, in_=x_d[:, 16 * g:16 * g + 16, :])
                xsq4 = pool.tile([C, 4, CH], bf16, tag="xsq")
                nc.vector.tensor_tensor(out=xsq4, in0=xt4, in1=xt4, op=ALU.mult)
                for hh in range(2):
                    ps = psum.tile([1, 2, CH], f32, tag="ps")
                    ps2 = psum.tile([1, 2, CH], f32, tag="ps2")
                    for i in range(2):
                        nc.tensor.matmul(ps[:, i, :], lhsT=onescol_bf,
                                         rhs=xt4[:, 2 * hh + i, :], start=True, stop=True)
                        nc.tensor.matmul(ps2[:, i, :], lhsT=onescol_bf,
                                         rhs=xsq4[:, 2 * hh + i, :], start=True, stop=True)
                    stg = stpool.tile([1, 2, 2 * CH], f32, tag="stg")
                    nc.scalar.copy(stg[:, 0, :], ps.rearrange("p a b -> p (a b)"))
                    nc.scalar.copy(stg[:, 1, :], ps2.rearrange("p a b -> p (a b)"))
                    off = CH * (4 * g + 2 * hh)
                    nc.sync.dma_start(out=sc1_d[0:1, off:off + 2 * CH],
                                      in_=stg[:, 0, :])
                    nc.sync.dma_start(out=sc1_d[1:2, off:off + 2 * CH],
                                      in_=stg[:, 1, :])

        # ============ stats math (shared helper) ============
        def stats_math(sc_dram, ab_dram, ab_dt, extra_sum=None):
            with tc.tile_pool(name="stm", bufs=1) as pool:
                s0 = pool.tile([128, 128], f32, tag="s0")
                s1 = pool.tile([128, 128], f32, tag="s1")
                src = sc_dram[:, :].rearrange("a b -> (a b)")
                ap0 = [[128, 128], [1, 128]]
                nc.sync.dma_start(out=s0, in_=bass.AP(tensor=src.tensor, offset=0, ap=ap0))
                nc.sync.dma_start(out=s1, in_=bass.AP(tensor=src.tensor, offset=N, ap=ap0))
                if extra_sum is not None:
                    sx = pool.tile([128, 128], f32, tag="sx")
                    esrc = extra_sum[:, :].rearrange("a b -> (a b)")
                    nc.sync.dma_start(out=sx, in_=bass.AP(tensor=esrc.tensor,
                                                          offset=0, ap=ap0))
                    nc.vector.tensor_tensor(out=s0, in0=s0, in1=sx, op=ALU.add)
                mu = pool.tile([128, 128], f32, tag="mu")
                nc.scalar.mul(out=mu, in_=s0, mul=1.0 / C)
                ex2 = pool.tile([128, 128], f32, tag="ex2")
                nc.scalar.mul(out=ex2, in_=s1, mul=1.0 / C)
                var = pool.tile([128, 128], f32, tag="var")
                nc.vector.scalar_tensor_tensor(out=var, in0=mu, scalar=-1.0, in1=mu,
                                               op0=ALU.mult, op1=ALU.mult)
                nc.vector.tensor_tensor(out=var, in0=ex2, in1=var, op=ALU.add)
                sd = pool.tile([128, 128], f32, tag="sd")
                nc.scalar.activation(out=sd, in_=var, func=AF.Sqrt, bias=epst, scale=1.0)
                rs = pool.tile([128, 128], ab_dt, tag="rs")
                with nc.allow_low_precision(reason="ln scale rows"):
                    nc.vector.reciprocal(out=rs, in_=sd)
                nb = pool.tile([128, 128], ab_dt, tag="nb")
                nc.vector.scalar_tensor_tensor(out=nb, in0=mu, scalar=-1.0, in1=rs,
                                               op0=ALU.mult, op1=ALU.mult)
                dst = ab_dram[:, :].rearrange("a b -> (a b)")
                nc.sync.dma_start(out=bass.AP(tensor=dst.tensor, offset=0, ap=[[1, N]]),
                                  in_=rs)
                nc.sync.dma_start(out=bass.AP(tensor=dst.tensor, offset=N, ap=[[1, N]]),
                                  in_=nb)

        load_weights()
        stats_math(sc1_d, ab1_d, bf16)

        # ============ PH2: LN1 apply + qkv + scatter to Qd/KVp ============
        def k_sections(c):
            """(band, r0, r1) image-row ranges of chunk c hitting band halos."""
            lo, hi = 4 * c, 4 * c + 4
            out = []
            for b in range(NB):
                s_lo, s_hi = BH * b - 3, BH * b + BH + 3
                r0, r1 = max(lo, s_lo), min(hi, s_hi)
                if r0 < r1:
                    out.append((b, r0, r1))
            return out

        with tc.tile_pool(name="ph2", bufs=3) as pool, \
             tc.tile_pool(name="ph2ab", bufs=2) as abpool, \
             tc.tile_pool(name="ph2ps", bufs=2, space="PSUM") as psum:
            ab1_flat = ab1_d[:, :].rearrange("a b -> (a b)")
            for c in range(NCHUNK):
                g, i = c // 4, c % 4
                if i == 0:
                    xt4 = pool.tile([C, 4, CH], bf16, tag="xt2")
                    nc.gpsimd.dma_start(out=xt4, in_=x_d[:, 16 * g:16 * g + 16, :])
                    paB = abpool.tile([C, 4, CH], bf16, tag="paB")
                    nc.sync.dma_start(
                        out=paB,
                        in_=bass.AP(tensor=ab1_flat.tensor, offset=4 * CH * g,
                                    ap=[[0, C], [1, 4 * CH]]))
                    pbB = abpool.tile([C, 4, CH], bf16, tag="pbB")
                    nc.sync.dma_start(
                        out=pbB,
                        in_=bass.AP(tensor=ab1_flat.tensor, offset=N + 4 * CH * g,
                                    ap=[[0, C], [1, 4 * CH]]))
                xt = xt4[:, i, :]
                t1 = pool.tile([C, CH], bf16, tag="t1")
                nc.vector.tensor_tensor(out=t1, in0=xt, in1=paB[:, i, :], op=ALU.mult)
                xn = pool.tile([C, CH], bf16, tag="xn")
                nc.vector.tensor_tensor(out=xn, in0=t1, in1=pbB[:, i, :], op=ALU.add)

                pq = psum.tile([C, CH], f32, tag="pq")
                kv2 = psum.tile([C, 2, CH], f32, tag="kv2")
                nc.tensor.matmul(pq, lhsT=wq[:, 0:C], rhs=xn, start=True, stop=True)
                nc.tensor.matmul(kv2[:, 0, :], lhsT=wq[:, C:2 * C], rhs=xn,
                                 start=True, stop=True)
                nc.tensor.matmul(kv2[:, 1, :], lhsT=wq[:, 2 * C:3 * C], rhs=xn,
                                 start=True, stop=True)

                # stage k/v to SBUF bf16 once (Act), then scatter from SBUF
                kvs = pool.tile([C, 2, CH], bf16, tag="kvs")
                nc.scalar.copy(kvs, kv2)

                b = c // 8
                r_off = 4 * c - BH * b
                # Qd scatter: d=0 on DVE (from PSUM), d=1,2 on Act (from PSUM)
                for d in range(3):
                    dst = Qd[d][32 * b:32 * b + 32, r_off:r_off + 4, :]
                    src = pq[32 * d:32 * d + 32, :].rearrange("p (r w) -> p r w", r=4)
                    if d == 0:
                        if has_qbias:
                            nc.vector.tensor_scalar_add(
                                out=dst, in0=src,
                                scalar1=c0[32 * d:32 * d + 32, 0:1])
                        else:
                            nc.vector.tensor_copy(out=dst, in_=src)
                    else:
                        if has_qbias:
                            nc.scalar.activation(
                                out=dst, in_=src, func=AF.Identity,
                                bias=c0[32 * d:32 * d + 32, 0:1], scale=1.0)
                        else:
                            nc.scalar.copy(dst, src)
                # K/V scatter from kvs: d=0 DVE, d=1 split K->Act V->Pool,
                # d=2 Pool
                for d in range(3):
                    for (bb, ra, rb) in k_sections(c):
                        nrows = rb - ra
                        src = kvs[32 * d:32 * d + 32, :,
                                  (ra - 4 * c) * W:(rb - 4 * c) * W]
                        dst = KVp[d][32 * bb:32 * bb + 32, :,
                                     ra - (BH * bb - 3):rb - (BH * bb - 3), 3:3 + W]
                        srcr = src.rearrange("p a (r w) -> p a r w", r=nrows)
                        if d < 2:
                            nc.vector.tensor_copy(out=dst, in_=srcr)
                        else:
                            nc.gpsimd.tensor_copy(out=dst, in_=srcr)

        # ============ PH3: attention per dilation ============
        with tc.tile_pool(name="ph3", bufs=2) as pool, \
             tc.tile_pool(name="ph3p", bufs=3) as ppool, \
             tc.tile_pool(name="ph3f", bufs=3) as fpool, \
             tc.tile_pool(name="ph3r", bufs=1) as rpool, \
             tc.tile_pool(name="ph3acc", bufs=1) as acc, \
             tc.tile_pool(name="ph3ps", bufs=2, space="PSUM") as psum:
            for di, dil in enumerate(DILS):
                S = acc.tile([128, BH * W], bf16, tag="S")
                qv = Qd[di][:, :, :]
                kpl = KVp[di][:, 0, :, :]
                vpl = KVp[di][:, 1, :, :]
                for ti, (dr, dc) in enumerate([(i - 1, j - 1)
                                               for i in range(3) for j in range(3)]):
                    kwin = kpl[:, 3 + dr * dil:3 + dr * dil + BH,
                               3 + dc * dil:3 + dc * dil + W]
                    vwin = vpl[:, 3 + dr * dil:3 + dr * dil + BH,
                               3 + dc * dil:3 + dc * dil + W]
                    P = ppool.tile([128, BH, W], bf16, tag="P")
                    nc.vector.tensor_tensor(out=P, in0=qv, in1=kwin, op=ALU.mult)
                    Pf = P.rearrange("p r w -> p (r w)")
                    expL = fpool.tile([128, BH * W], bf16, tag="expL")
                    for half in range(2):
                        pl = psum.tile([128, 2048], f32, tag="pl")
                        for q in range(4):
                            nc.tensor.matmul(pl[:, 512 * q:512 * (q + 1)],
                                             lhsT=repl,
                                             rhs=Pf[:, 2048 * half + 512 * q:
                                                    2048 * half + 512 * (q + 1)],
                                             start=True, stop=True)
                        nc.scalar.activation(out=expL[:, 2048 * half:2048 * (half + 1)],
                                             in_=pl, func=AF.Exp)
                    ev = expL.rearrange("p (r w) -> p r w", r=BH)
                    if ti == 0:
                        nc.vector.tensor_copy(out=S, in_=expL)
                        nc.vector.tensor_tensor(
                            out=Od[di].rearrange("p (r w) -> p r w", r=BH),
                            in0=ev, in1=vwin, op=ALU.mult)
                    else:
                        # accumulation chains (S, O) stay on DVE; Pool gets
                        # only off-chain Pv products so slow ops don't extend
                        # the serial dependency chain
                        nc.vector.tensor_tensor(out=S, in0=S, in1=expL, op=ALU.add)
                        Pv = pool.tile([128, BH, W], bf16, tag="Pv")
                        peng = nc.gpsimd if ti in (2, 4, 6, 8) else nc.vector
                        peng.tensor_tensor(out=Pv, in0=ev, in1=vwin, op=ALU.mult)
                        nc.vector.tensor_tensor(out=Od[di], in0=Od[di],
                                                in1=Pv.rearrange("p r w -> p (r w)"),
                                                op=ALU.add)
                rcp = rpool.tile([128, BH * W], bf16, tag="rcp")
                with nc.allow_low_precision(reason="softmax denom in bf16 is fine"):
                    nc.vector.reciprocal(out=rcp, in_=S)
                nc.vector.tensor_tensor(out=Od[di], in0=Od[di], in1=rcp, op=ALU.mult)

        apool.release()

        # ====== PH4: proj (from SBUF O tiles) + residual + LN2 stats ======
        r1pool = tc.alloc_tile_pool(name="r1p", bufs=1)
        r1 = r1pool.tile([C, N], bf16)
        with tc.tile_pool(name="ph4", bufs=3) as pool, \
             tc.tile_pool(name="ph4st", bufs=2) as stpool, \
             tc.tile_pool(name="ph4ps", bufs=2, space="PSUM") as psum, \
             tc.tile_pool(name="ph4ps2", bufs=1, space="PSUM") as psum2:
            for c in range(NCHUNK):
                g, i = c // 4, c % 4
                b = c // 8
                off = (4 * c - BH * b) * W
                if i == 0:
                    xt4 = pool.tile([C, 4, CH], f32, tag="xt4")
                    nc.sync.dma_start(out=xt4, in_=x_d[:, 16 * g:16 * g + 16, :])
                # proj matmul carries an extra output row: the column-sums of
                # proj_w, so row C of pp = sum_c proj_out[c, n]. Combined with
                # PH1's x-sums (sc1 row 0) in stats_math this gives the LN2
                # token sums without a dedicated matmul.
                pp = psum.tile([C + 1, CH], f32, tag="pp")
                if has_projb:
                    nc.tensor.matmul(pp, lhsT=pb_row, rhs=onesrow, start=True, stop=False)
                for d in range(3):
                    nc.tensor.matmul(pp, lhsT=wpb[32 * b:32 * b + 32, d, :],
                                     rhs=Od[d][32 * b:32 * b + 32, off:off + CH],
                                     start=(d == 0 and not has_projb),
                                     stop=(d == 2),
                                     tile_position=(32 * b, 0))
                rsl = r1[:, CH * c:CH * (c + 1)]
                nc.vector.tensor_tensor(out=rsl, in0=xt4[:, i, :], in1=pp[0:C, :],
                                        op=ALU.add)
                # LN2 stats inline
                if c % 2 == 0:
                    ps2 = psum2.tile([1, 2, CH], f32, tag="ps52")
                    stg = stpool.tile([1, 2, 2 * CH], f32, tag="stg5")
                nc.scalar.copy(stg[:, 0, (c % 2) * CH:(c % 2 + 1) * CH],
                               pp[C:C + 1, :])
                xsq = pool.tile([C, CH], bf16, tag="xsq5")
                nc.vector.tensor_tensor(out=xsq, in0=rsl, in1=rsl, op=ALU.mult)
                nc.tensor.matmul(ps2[:, c % 2, :], lhsT=onescol_bf, rhs=xsq,
                                 start=True, stop=True)
                if c % 2 == 1:
                    nc.scalar.copy(stg[:, 1, :], ps2.rearrange("p a b -> p (a b)"))
                    soff = CH * (c - 1)
                    nc.sync.dma_start(out=sc2_d[0:1, soff:soff + 2 * CH],
                                      in_=stg[:, 0, :])
                    nc.sync.dma_start(out=sc2_d[1:2, soff:soff + 2 * CH],
                                      in_=stg[:, 1, :])

        stats_math(sc2_d, ab2_d, bf16, extra_sum=sc1_d)

        # ============ PH5b: MLP + residual ============
        with tc.tile_pool(name="ph5b", bufs=3) as pool, \
             tc.tile_pool(name="ph5ab", bufs=2) as abpool, \
             tc.tile_pool(name="ph5ps", bufs=2, space="PSUM") as psum:
            ab2_flat = ab2_d[:, :].rearrange("a b -> (a b)")
            for c in range(NCHUNK):
                g, i = c // 4, c % 4
                rsl = r1[:, CH * c:CH * (c + 1)]
                if i == 0:
                    paB5 = abpool.tile([C, 4, CH], bf16, tag="pa5B")
                    nc.sync.dma_start(
                        out=paB5,
                        in_=bass.AP(tensor=ab2_flat.tensor, offset=4 * CH * g,
                                    ap=[[0, C], [1, 4 * CH]]))
                    pbB5 = abpool.tile([C, 4, CH], bf16, tag="pb5B")
                    nc.sync.dma_start(
                        out=pbB5,
                        in_=bass.AP(tensor=ab2_flat.tensor, offset=N + 4 * CH * g,
                                    ap=[[0, C], [1, 4 * CH]]))
                    yout4 = abpool.tile([C, 4, CH], f32, tag="yout4")
                t1 = pool.tile([C, CH], bf16, tag="t15")
                nc.gpsimd.tensor_tensor(out=t1, in0=rsl,
                                        in1=paB5[:, i, :], op=ALU.mult)
                xn = pool.tile([C, CH], bf16, tag="xn5")
                nc.gpsimd.tensor_tensor(out=xn, in0=t1, in1=pbB5[:, i, :], op=ALU.add)

                h1 = pool.tile([128, 3, CH], bf16, tag="h1")
                if has_c1:
                    for j in range(3):
                        pf = psum.tile([128, CH], f32, tag="pf")
                        nc.tensor.matmul(pf, lhsT=w1[:, 128 * j:128 * (j + 1)], rhs=xn,
                                         start=True, stop=True)
                        nc.scalar.activation(out=h1[:, j, :], in_=pf, func=AF.Gelu,
                                             bias=c1[j][:, 0:1], scale=1.0)
                else:
                    pf3 = psum.tile([128, 3, CH], f32, tag="pf3")
                    for j in range(3):
                        nc.tensor.matmul(pf3[:, j, :], lhsT=w1[:, 128 * j:128 * (j + 1)],
                                         rhs=xn, start=True, stop=True)
                    nc.scalar.activation(out=h1, in_=pf3, func=AF.Gelu)
                pm = psum.tile([C, CH], f32, tag="pm")
                if has_fc2b:
                    nc.tensor.matmul(pm, lhsT=b2t, rhs=onesrow, start=True, stop=False)
                for j in range(3):
                    nc.tensor.matmul(pm, lhsT=w2[j], rhs=h1[:, j, :],
                                     start=(j == 0 and not has_fc2b), stop=(j == 2))
                nc.vector.tensor_tensor(out=yout4[:, i, :], in0=rsl,
                                        in1=pm, op=ALU.add)
                if i == 3:
                    nc.sync.dma_start(out=y_d[:, 16 * g:16 * g + 16, :], in_=yout4)

        r1pool.release()
        opool.release()
        wpool.release()

    _split_multi_waits(nc, mybir)
    return nc


def _prep_weights(inputs):
    """Host-side weight preparation (fold LN affine, scale, transposes)."""
    qkv_w = np.asarray(inputs['qkv_w'], np.float32)       # (288, 96)
    proj_w = np.asarray(inputs['proj_w'], np.float32)     # (96, 96)
    proj_b = np.asarray(inputs['proj_b'], np.float32)
    ln1_w = np.asarray(inputs['ln1_w'], np.float32)
    ln1_b = np.asarray(inputs['ln1_b'], np.float32)
    ln2_w = np.asarray(inputs['ln2_w'], np.float32)
    ln2_b = np.asarray(inputs['ln2_b'], np.float32)
    fc1_w = np.asarray(inputs['fc1_w'], np.float32)       # (384, 96)
    fc1_b = np.asarray(inputs['fc1_b'], np.float32)
    fc2_w = np.asarray(inputs['fc2_w'], np.float32)       # (96, 384)
    fc2_b = np.asarray(inputs['fc2_b'], np.float32)

    wq = qkv_w * ln1_w[None, :]                            # (288, 96)
    c0 = qkv_w @ ln1_b                                     # (288,)
    wq[0:C] *= SCALE                                       # scale q rows
    c0[0:C] *= SCALE
    # v bias folds into proj bias; k bias cancels in softmax
    pb_eff = proj_b + proj_w @ c0[2 * C:3 * C]

    w1 = fc1_w * ln2_w[None, :]
    c1 = fc1_w @ ln2_b + fc1_b

    repl = np.zeros((128, 128), np.float32)
    for b in range(NB):
        for ch in range(GD):
            h0 = (ch // HD) * HD
            repl[32 * b + h0:32 * b + h0 + HD, 32 * b + ch] = 1.0

    # proj lhsT in per-band layout: wpb[32b+j, d, o] = proj_w[o, 32d+j];
    # extra column C holds proj_w column-sums so the matmul also emits the
    # per-token sum of the proj output (feeds LN2 stats).
    wpT = proj_w.T                                         # (in=96, out=96)
    colsum = proj_w.sum(axis=0)                            # (96,)
    wpb = np.zeros((128, 3, C + 1), np.float32)
    for b in range(NB):
        for d in range(3):
            wpb[32 * b:32 * b + 32, d, 0:C] = wpT[32 * d:32 * d + 32, :]
            wpb[32 * b:32 * b + 32, d, C] = colsum[32 * d:32 * d + 32]
    pb_ext = np.concatenate([pb_eff, [pb_eff.sum()]]).astype(np.float32)

    return {
        'wqkv': np.ascontiguousarray(wq.T),                # (96, 288) lhsT
        'c0': c0.reshape(-1, 1).astype(np.float32),
        'wproj': np.ascontiguousarray(wpb.reshape(128, 3 * (C + 1))),
        'projb': pb_ext.reshape(-1, 1),
        'w1': np.ascontiguousarray(w1.T),                  # (96, 384) lhsT
        'c1': c1.reshape(-1, 1).astype(np.float32),
        'w2': np.ascontiguousarray(fc2_w.T),               # (384, 96) lhsT
        'b2': fc2_b.reshape(-1, 1).astype(np.float32),
        'repl': repl,
        'onesc': np.ones((C, 1), np.float32),
    }


def kernel(**inputs):
    from concourse.bass_utils import run_bass_kernel_spmd

    wmap = _prep_weights(inputs)
    has_qbias = bool(np.any(wmap['c0'][0:C] != 0))
    has_projb = bool(np.any(wmap['projb'] != 0))
    has_fc2b = bool(np.any(wmap['b2'] != 0))
    has_c1 = bool(np.any(wmap['c1'] != 0))
    key = ('nc', has_qbias, has_projb, has_fc2b, has_c1)

    if key not in _cache:
        t0 = time.time()
        _cache[key] = _build(has_qbias, has_projb, has_fc2b, has_c1)
        print(f"[kernel] built bass module in {time.time() - t0:.1f}s",
              file=sys.stderr)

    nc = _cache[key]
    _cache['nc'] = nc
    x = np.asarray(inputs['x'], np.float32)                # (8, 96, 128, 128)

    in_maps = []
    for b in range(B):
        m = {'x': np.ascontiguousarray(x[b])}
        m.update(wmap)
        in_maps.append(m)

    res = run_bass_kernel_spmd(nc, in_maps, core_ids=list(range(B)))
    _cache['last_exec_ns'] = res.exec_time_ns
    out = np.stack([res.results[b]['y'] for b in range(B)], axis=0)
    return out.astype(np.float32)


# revision 74
# speedup vs baseline: 1.3350x; 1.0019x over previous
"""DilateBlock kernel for 8x Trainium2 NeuronCores (Bass/Tile).

Data-parallel over batch B=8 (one image per core). Per core, the whole block
(LN1 -> qkv -> 3-dilation 3x3 neighborhood attention -> proj -> residual ->
LN2 -> MLP -> residual) runs in channels-on-partitions layout; spatial shifts
for the attention unfold live on the free dimension of zero-padded (h, w)
planes, packed 4-hbands x 32-channels across partitions.

Key tricks vs the original:
  - LN stats PSUM rows DMA'd straight to DRAM (no Act-engine strip copies).
  - K/V qkv biases eliminated exactly (K bias shifts all 9 logits equally ->
    softmax-invariant; V bias folds into the proj bias on the host).
  - K/V scatter staged once to SBUF bf16 then spread across DVE/Act/Pool.
  - Attention output kept in SBUF; proj consumes it via per-band split
    matmuls (no DRAM round-trip for the attention output).
  - PH3 elementwise work split DVE/Pool; reciprocal in bf16.
  - Zero-bias specialization: bias ops are only emitted when the actual
    folded bias vectors are nonzero (they are zero for this problem's
    setup_inputs), with a general fallback path.
"""
import sys
import time

sys.path.insert(0, '/opt/trn_rl_repo')

import numpy as np

# ---- problem constants (hardcoded per contract) ----
B, C, H, W = 8, 96, 128, 128
DILS = (1, 2, 3)
GD = 32                 # channels per dilation branch
HD = 16                 # head dim
NB = 4                  # h-bands packed on partitions
BH = H // NB            # rows per band = 32
N = H * W               # tokens per image
NCHUNK = 32             # token chunks of 512 (4 image rows each)
CH = N // NCHUNK        # 512
PADR = 38               # BH + 6 halo rows
PADC = 135              # W + 6 halo cols (odd pitch: even bf16 tap offsets)
EPS = 1e-5
SCALE = HD ** -0.5
MLPH = 384

_cache = {}


def _patch_tile(tile_mod, bass_mod):
    """Work around this walrus build's 1-sem-wait-per-instruction limit and
    the multi-wait tail drain."""
    from concourse.vector_clock import ScopedClock, VectorClock

    def _drain_and_barrier(self, tick_clock, wait_clock):
        vclock = tick_clock.global_clock
        n = len(vclock)
        idxs = [i for i in range(n) if vclock[i] > 0]
        for i in idxs:
            vec = [0] * n
            vec[i] = vclock[i]
            nop_inst = self.nc.sync.nop(nofuse=True)
            wait_clock.add_sem_waits(nop_inst.ins,
                                     ScopedClock({None: VectorClock(vec)}))
        self.nc.sync.drain()
        self.nc.all_engine_barrier()
        popped = self.nc._tile_sem_poison_stack.pop()
        assert popped is self._sem_poison
        self.nc.clear_and_free_semaphores(list(self.sems.allocated().values()))
        self.nc.all_engine_barrier()

    tile_mod.TileContext._drain_and_barrier = _drain_and_barrier


_ws_counter = [0]


def _split_multi_waits(nc, mybir):
    for fn in nc.m.functions:
        for blk in fn.blocks:
            insts = list(blk.instructions)
            out = []
            changed = False
            for inst in insts:
                si = inst.sync_info
                waits = list(si.on_wait) if si and si.on_wait else []
                if len(waits) > 1:
                    for w in waits[:-1]:
                        _ws_counter[0] += 1
                        out.append(mybir.InstNoOp(
                            name=f"I-ws-{_ws_counter[0]}",
                            engine=inst.engine, ins=[], outs=[],
                            sync_info=mybir.SyncInfo(on_wait=[w], on_update=[])))
                    si.on_wait = [waits[-1]]
                    changed = True
                out.append(inst)
            if changed:
                blk.instructions[:] = out


def _build(has_qbias, has_projb, has_fc2b, has_c1=True):
    import concourse.bass as bass
    import concourse.tile as tile
    from concourse import mybir

    _patch_tile(tile, bass)

    f32 = mybir.dt.float32
    f32r = mybir.dt.float32r
    bf16 = mybir.dt.bfloat16
    AF = mybir.ActivationFunctionType
    ALU = mybir.AluOpType

    nc = bass.Bass()

    # ---- DRAM I/O ----
    x_d = nc.dram_tensor("x", (C, H, W), f32, kind="ExternalInput")
    wq_d = nc.dram_tensor("wqkv", (C, 3 * C), f32, kind="ExternalInput")   # lhsT
    c0_d = nc.dram_tensor("c0", (3 * C, 1), f32, kind="ExternalInput")
    wp_d = nc.dram_tensor("wproj", (128, 3 * (C + 1)), f32, kind="ExternalInput")
    pb_d = nc.dram_tensor("projb", (C + 1, 1), f32, kind="ExternalInput")
    w1_d = nc.dram_tensor("w1", (C, MLPH), f32, kind="ExternalInput")      # lhsT
    c1_d = nc.dram_tensor("c1", (MLPH, 1), f32, kind="ExternalInput")
    w2_d = nc.dram_tensor("w2", (MLPH, C), f32, kind="ExternalInput")      # lhsT
    b2_d = nc.dram_tensor("b2", (C, 1), f32, kind="ExternalInput")
    repl_d = nc.dram_tensor("repl", (128, 128), f32, kind="ExternalInput")
    ones_d = nc.dram_tensor("onesc", (C, 1), f32, kind="ExternalInput")

    y_d = nc.dram_tensor("y", (C, H, W), f32, kind="ExternalOutput")
    sc1_d = nc.dram_tensor("sc1", (2, N), f32, kind="ExternalOutput")
    sc2_d = nc.dram_tensor("sc2", (2, N), f32, kind="ExternalOutput")
    ab1_d = nc.dram_tensor("ab1", (2, N), bf16, kind="ExternalOutput")
    ab2_d = nc.dram_tensor("ab2", (2, N), bf16, kind="ExternalOutput")

    with tile.TileContext(nc) as tc:
        # ---------------- persistent pools ----------------
        # Allocate weight tiles up front; only PH1's inputs are DMA'd now.
        # The heavy cast-DMAs are deferred until after PH1's x loads so the
        # gpsimd DMA queue starts streaming x immediately.
        wpool = tc.alloc_tile_pool(name="weights", bufs=1)
        wq = wpool.tile([C, 3 * C], bf16)
        c0 = wpool.tile([C, 1], f32)            # q bias (scaled), only rows 0:C used
        wpb = wpool.tile([128, 3, C + 1], bf16)  # proj lhsT per band + sum row
        pb_row = wpool.tile([1, C + 1], f32r, tag="pbr", name="pbr") \
            if has_projb else None
        w1 = wpool.tile([C, MLPH], bf16)
        c1 = [wpool.tile([128, 1], f32, tag=f"c1{i}", name=f"c1{i}") for i in range(3)]
        w2 = [wpool.tile([128, C], bf16, tag=f"w2{i}", name=f"w2{i}") for i in range(3)]
        b2t = wpool.tile([1, C], f32r, tag="b2t", name="b2t") \
            if has_fc2b else None
        repl = wpool.tile([128, 128], bf16)
        onescol = wpool.tile([C, 1], f32r)      # stats lhsT [96,1]
        nc.sync.dma_start(out=onescol, in_=ones_d[:, :].bitcast(f32r))
        onescol_bf = wpool.tile([C, 1], bf16)   # stats lhsT for bf16 rhs
        nc.gpsimd.dma_start(out=onescol_bf, in_=ones_d[:, :])
        onesrow = None
        if has_projb or has_fc2b:
            onesrow = wpool.tile([1, CH], f32r, tag="onesrow", name="onesrow")
            nc.vector.memset(onesrow.bitcast(f32), 1.0)
        epst = wpool.tile([128, 1], f32)
        nc.vector.memset(epst, EPS)

        def load_weights():
            nc.gpsimd.dma_start(out=wq, in_=wq_d[:, :])
            if has_qbias:
                nc.sync.dma_start(out=c0, in_=c0_d[0:C, :])
            nc.gpsimd.dma_start(out=wpb,
                                in_=wp_d[:, :].rearrange("p (a b) -> p a b", a=3))
            if has_projb:
                nc.sync.dma_start(
                    out=pb_row,
                    in_=pb_d[:, :].rearrange("a b -> b a").bitcast(f32r))
            nc.gpsimd.dma_start(out=w1, in_=w1_d[:, :])
            for i in range(3):
                nc.sync.dma_start(out=c1[i], in_=c1_d[128 * i:128 * (i + 1), :])
            for i in range(3):
                nc.gpsimd.dma_start(out=w2[i], in_=w2_d[128 * i:128 * (i + 1), :])
            if has_fc2b:
                nc.sync.dma_start(
                    out=b2t, in_=b2_d[:, :].rearrange("a b -> b a").bitcast(f32r))
            nc.gpsimd.dma_start(out=repl, in_=repl_d[:, :])

        # big persistent activation tensors
        opool = tc.alloc_tile_pool(name="opool", bufs=1)
        Od = [opool.tile([128, BH * W], bf16, tag=f"od{d}", name=f"od{d}") for d in range(3)]
        apool = tc.alloc_tile_pool(name="acts", bufs=1)
        Qd = [apool.tile([128, BH, W], bf16, tag=f"qd{d}", name=f"qd{d}") for d in range(3)]
        KVp = [apool.tile([128, 2, PADR, PADC], bf16, tag=f"kvp{d}", name=f"kvp{d}")
               for d in range(3)]
        for d in range(3):
            # zero only the halo borders (interior is fully overwritten)
            nc.gpsimd.memset(KVp[d][:, :, 0:3, :], 0.0)
            nc.gpsimd.memset(KVp[d][:, :, PADR - 3:PADR, :], 0.0)
            nc.gpsimd.memset(KVp[d][:, :, 3:PADR - 3, 0:3], 0.0)
            nc.gpsimd.memset(KVp[d][:, :, 3:PADR - 3, 3 + W:PADC], 0.0)

        # ============ PH1: LN1 stats sweep ============
        with tc.tile_pool(name="ph1", bufs=3) as pool, \
             tc.tile_pool(name="ph1st", bufs=2) as stpool, \
             tc.tile_pool(name="ph1ps", bufs=2, space="PSUM") as psum:
            for g in range(NCHUNK // 4):
                xt4 = pool.tile([C, 4, CH], bf16, tag="xt")
                nc.gpsimd.dma_start(out=xt4, in_=x_d[:, 16 * g:16 * g + 16, :])
                xsq4 = pool.tile([C, 4, CH], bf16, tag="xsq")
                nc.vector.tensor_tensor(out=xsq4, in0=xt4, in1=xt4, op=ALU.mult)
                for hh in range(2):
                    ps = psum.tile([1, 2, CH], f32, tag="ps")
                    ps2 = psum.tile([1, 2, CH], f32, tag="ps2")
                    for i in range(2):
                        nc.tensor.matmul(ps[:, i, :], lhsT=onescol_bf,
                                         rhs=xt4[:, 2 * hh + i, :], start=True, stop=True)
                        nc.tensor.matmul(ps2[:, i, :], lhsT=onescol_bf,
                                         rhs=xsq4[:, 2 * hh + i, :], start=True, stop=True)
                    stg = stpool.tile([1, 2, 2 * CH], f32, tag="stg")
                    nc.scalar.copy(stg[:, 0, :], ps.rearrange("p a b -> p (a b)"))
                    nc.scalar.copy(stg[:, 1, :], ps2.rearrange("p a b -> p (a b)"))
                    off = CH * (4 * g + 2 * hh)
                    nc.sync.dma_start(out=sc1_d[0:1, off:off + 2 * CH],
                                      in_=stg[:, 0, :])
                    nc.sync.dma_start(out=sc1_d[1:2, off:off + 2 * CH],
                                      in_=stg[:, 1, :])

        # ============ stats math (shared helper) ============
        def stats_math(sc_dram, ab_dram, ab_dt, extra_sum=None):
            with tc.tile_pool(name="stm", bufs=1) as pool:
                s0 = pool.tile([128, 128], f32, tag="s0")
                s1 = pool.tile([128, 128], f32, tag="s1")
                src = sc_dram[:, :].rearrange("a b -> (a b)")
                ap0 = [[128, 128], [1, 128]]
                nc.sync.dma_start(out=s0, in_=bass.AP(tensor=src.tensor, offset=0, ap=ap0))
                nc.sync.dma_start(out=s1, in_=bass.AP(tensor=src.tensor, offset=N, ap=ap0))
                if extra_sum is not None:
                    sx = pool.tile([128, 128], f32, tag="sx")
                    esrc = extra_sum[:, :].rearrange("a b -> (a b)")
                    nc.sync.dma_start(out=sx, in_=bass.AP(tensor=esrc.tensor,
                                                          offset=0, ap=ap0))
                    nc.vector.tensor_tensor(out=s0, in0=s0, in1=sx, op=ALU.add)
                mu = pool.tile([128, 128], f32, tag="mu")
                nc.scalar.mul(out=mu, in_=s0, mul=1.0 / C)
                ex2 = pool.tile([128, 128], f32, tag="ex2")
                nc.scalar.mul(out=ex2, in_=s1, mul=1.0 / C)
                var = pool.tile([128, 128], f32, tag="var")
                nc.vector.scalar_tensor_tensor(out=var, in0=mu, scalar=-1.0, in1=mu,
                                               op0=ALU.mult, op1=ALU.mult)
                nc.vector.tensor_tensor(out=var, in0=ex2, in1=var, op=ALU.add)
                sd = pool.tile([128, 128], f32, tag="sd")
                nc.scalar.activation(out=sd, in_=var, func=AF.Sqrt, bias=epst, scale=1.0)
                rs = pool.tile([128, 128], ab_dt, tag="rs")
                with nc.allow_low_precision(reason="ln scale rows"):
                    nc.vector.reciprocal(out=rs, in_=sd)
                nb = pool.tile([128, 128], ab_dt, tag="nb")
                nc.vector.scalar_tensor_tensor(out=nb, in0=mu, scalar=-1.0, in1=rs,
                                               op0=ALU.mult, op1=ALU.mult)
                dst = ab_dram[:, :].rearrange("a b -> (a b)")
                nc.sync.dma_start(out=bass.AP(tensor=dst.tensor, offset=0, ap=[[1, N]]),
                                  in_=rs)
                nc.sync.dma_start(out=bass.AP(tensor=dst.tensor, offset=N, ap=[[1, N]]),
                                  in_=nb)

        load_weights()
        stats_math(sc1_d, ab1_d, bf16)

        # ============ PH2: LN1 apply + qkv + scatter to Qd/KVp ============
        def k_sections(c):
            """(band, r0, r1) image-row ranges of chunk c hitting band halos."""
            lo, hi = 4 * c, 4 * c + 4
            out = []
            for b in range(NB):
                s_lo, s_hi = BH * b - 3, BH * b + BH + 3
                r0, r1 = max(lo, s_lo), min(hi, s_hi)
                if r0 < r1:
                    out.append((b, r0, r1))
            return out

        with tc.tile_pool(name="ph2", bufs=3) as pool, \
             tc.tile_pool(name="ph2ab", bufs=2) as abpool, \
             tc.tile_pool(name="ph2ps", bufs=2, space="PSUM") as psum:
            ab1_flat = ab1_d[:, :].rearrange("a b -> (a b)")
            for c in range(NCHUNK):
                g, i = c // 4, c % 4
                if i == 0:
                    xt4 = pool.tile([C, 4, CH], bf16, tag="xt2")
                    nc.gpsimd.dma_start(out=xt4, in_=x_d[:, 16 * g:16 * g + 16, :])
                    paB = abpool.tile([C, 4, CH], bf16, tag="paB")
                    nc.sync.dma_start(
                        out=paB,
                        in_=bass.AP(tensor=ab1_flat.tensor, offset=4 * CH * g,
                                    ap=[[0, C], [1, 4 * CH]]))
                    pbB = abpool.tile([C, 4, CH], bf16, tag="pbB")
                    nc.sync.dma_start(
                        out=pbB,
                        in_=bass.AP(tensor=ab1_flat.tensor, offset=N + 4 * CH * g,
                                    ap=[[0, C], [1, 4 * CH]]))
                xt = xt4[:, i, :]
                t1 = pool.tile([C, CH], bf16, tag="t1")
                nc.vector.tensor_tensor(out=t1, in0=xt, in1=paB[:, i, :], op=ALU.mult)
                xn = pool.tile([C, CH], bf16, tag="xn")
                nc.vector.tensor_tensor(out=xn, in0=t1, in1=pbB[:, i, :], op=ALU.add)

                pq = psum.tile([C, CH], f32, tag="pq")
                kv2 = psum.tile([C, 2, CH], f32, tag="kv2")
                nc.tensor.matmul(pq, lhsT=wq[:, 0:C], rhs=xn, start=True, stop=True)
                nc.tensor.matmul(kv2[:, 0, :], lhsT=wq[:, C:2 * C], rhs=xn,
                                 start=True, stop=True)
                nc.tensor.matmul(kv2[:, 1, :], lhsT=wq[:, 2 * C:3 * C], rhs=xn,
                                 start=True, stop=True)

                # stage k/v to SBUF bf16 once (Act), then scatter from SBUF
                kvs = pool.tile([C, 2, CH], bf16, tag="kvs")
                nc.scalar.copy(kvs, kv2)

                b = c // 8
                r_off = 4 * c - BH * b
                # Qd scatter: d=0 on DVE (from PSUM), d=1,2 on Act (from PSUM)
                for d in range(3):
                    dst = Qd[d][32 * b:32 * b + 32, r_off:r_off + 4, :]
                    src = pq[32 * d:32 * d + 32, :].rearrange("p (r w) -> p r w", r=4)
                    if d == 0:
                        if has_qbias:
                            nc.vector.tensor_scalar_add(
                                out=dst, in0=src,
                                scalar1=c0[32 * d:32 * d + 32, 0:1])
                        else:
                            nc.vector.tensor_copy(out=dst, in_=src)
                    else:
                        if has_qbias:
                            nc.scalar.activation(
                                out=dst, in_=src, func=AF.Identity,
                                bias=c0[32 * d:32 * d + 32, 0:1], scale=1.0)
                        else:
                            nc.scalar.copy(dst, src)
                # K/V scatter from kvs: d=0 DVE, d=1 split K->Act V->Pool,
                # d=2 Pool
                for d in range(3):
                    for (bb, ra, rb) in k_sections(c):
                        nrows = rb - ra
                        src = kvs[32 * d:32 * d + 32, :,
                                  (ra - 4 * c) * W:(rb - 4 * c) * W]
                        dst = KVp[d][32 * bb:32 * bb + 32, :,
                                     ra - (BH * bb - 3):rb - (BH * bb - 3), 3:3 + W]
                        srcr = src.rearrange("p a (r w) -> p a r w", r=nrows)
                        if d < 2:
                            nc.vector.tensor_copy(out=dst, in_=srcr)
                        else:
                            nc.gpsimd.tensor_copy(out=dst, in_=srcr)

        # ============ PH3: attention per dilation ============
        with tc.tile_pool(name="ph3", bufs=3) as pool, \
             tc.tile_pool(name="ph3p", bufs=3) as ppool, \
             tc.tile_pool(name="ph3f", bufs=3) as fpool, \
             tc.tile_pool(name="ph3acc", bufs=1) as acc, \
             tc.tile_pool(name="ph3ps", bufs=2, space="PSUM") as psum:
            for di, dil in enumerate(DILS):
                S = acc.tile([128, BH * W], bf16, tag="S")
                qv = Qd[di][:, :, :]
                kpl = KVp[di][:, 0, :, :]
                vpl = KVp[di][:, 1, :, :]
                for ti, (dr, dc) in enumerate([(i - 1, j - 1)
                                               for i in range(3) for j in range(3)]):
                    kwin = kpl[:, 3 + dr * dil:3 + dr * dil + BH,
                               3 + dc * dil:3 + dc * dil + W]
                    vwin = vpl[:, 3 + dr * dil:3 + dr * dil + BH,
                               3 + dc * dil:3 + dc * dil + W]
                    P = ppool.tile([128, BH, W], bf16, tag="P")
                    nc.vector.tensor_tensor(out=P, in0=qv, in1=kwin, op=ALU.mult)
                    Pf = P.rearrange("p r w -> p (r w)")
                    expL = fpool.tile([128, BH * W], bf16, tag="expL")
                    for half in range(2):
                        pl = psum.tile([128, 2048], f32, tag="pl")
                        for q in range(4):
                            nc.tensor.matmul(pl[:, 512 * q:512 * (q + 1)],
                                             lhsT=repl,
                                             rhs=Pf[:, 2048 * half + 512 * q:
                                                    2048 * half + 512 * (q + 1)],
                                             start=True, stop=True)
                        nc.scalar.activation(out=expL[:, 2048 * half:2048 * (half + 1)],
                                             in_=pl, func=AF.Exp)
                    ev = expL.rearrange("p (r w) -> p r w", r=BH)
                    if ti == 0:
                        nc.vector.tensor_copy(out=S, in_=expL)
                        nc.vector.tensor_tensor(
                            out=Od[di].rearrange("p (r w) -> p r w", r=BH),
                            in0=ev, in1=vwin, op=ALU.mult)
                    else:
                        # accumulation chains (S, O) stay on DVE; Pool gets
                        # only off-chain Pv products so slow ops don't extend
                        # the serial dependency chain
                        nc.vector.tensor_tensor(out=S, in0=S, in1=expL, op=ALU.add)
                        Pv = pool.tile([128, BH, W], bf16, tag="Pv")
                        peng = nc.gpsimd if ti in (2, 4, 6, 8) else nc.vector
                        peng.tensor_tensor(out=Pv, in0=ev, in1=vwin, op=ALU.mult)
                        nc.vector.tensor_tensor(out=Od[di], in0=Od[di],
                                                in1=Pv.rearrange("p r w -> p (r w)"),
                                                op=ALU.add)
                rcp = pool.tile([128, BH * W], bf16, tag="Pv", name="rcp")
                with nc.allow_low_precision(reason="softmax denom in bf16 is fine"):
                    nc.vector.reciprocal(out=rcp, in_=S)
                nc.vector.tensor_tensor(out=Od[di], in0=Od[di], in1=rcp, op=ALU.mult)

        apool.release()

        # ====== PH4: proj (from SBUF O tiles) + residual + LN2 stats ======
        r1pool = tc.alloc_tile_pool(name="r1p", bufs=1)
        r1 = r1pool.tile([C, N], bf16)
        with tc.tile_pool(name="ph4", bufs=3) as pool, \
             tc.tile_pool(name="ph4st", bufs=2) as stpool, \
             tc.tile_pool(name="ph4ps", bufs=2, space="PSUM") as psum, \
             tc.tile_pool(name="ph4ps2", bufs=1, space="PSUM") as psum2:
            for c in range(NCHUNK):
                g, i = c // 4, c % 4
                b = c // 8
                off = (4 * c - BH * b) * W
                if i == 0:
                    xt4 = pool.tile([C, 4, CH], f32, tag="xt4")
                    nc.sync.dma_start(out=xt4, in_=x_d[:, 16 * g:16 * g + 16, :])
                # proj matmul carries an extra output row: the column-sums of
                # proj_w, so row C of pp = sum_c proj_out[c, n]. Combined with
                # PH1's x-sums (sc1 row 0) in stats_math this gives the LN2
                # token sums without a dedicated matmul.
                pp = psum.tile([C + 1, CH], f32, tag="pp")
                if has_projb:
                    nc.tensor.matmul(pp, lhsT=pb_row, rhs=onesrow, start=True, stop=False)
                for d in range(3):
                    nc.tensor.matmul(pp, lhsT=wpb[32 * b:32 * b + 32, d, :],
                                     rhs=Od[d][32 * b:32 * b + 32, off:off + CH],
                                     start=(d == 0 and not has_projb),
                                     stop=(d == 2),
                                     tile_position=(32 * b, 0))
                rsl = r1[:, CH * c:CH * (c + 1)]
                nc.vector.tensor_tensor(out=rsl, in0=xt4[:, i, :], in1=pp[0:C, :],
                                        op=ALU.add)
                # LN2 stats inline
                if c % 2 == 0:
                    ps2 = psum2.tile([1, 2, CH], f32, tag="ps52")
                    stg = stpool.tile([1, 2, 2 * CH], f32, tag="stg5")
                nc.scalar.copy(stg[:, 0, (c % 2) * CH:(c % 2 + 1) * CH],
                               pp[C:C + 1, :])
                xsq = pool.tile([C, CH], bf16, tag="xsq5")
                nc.vector.tensor_tensor(out=xsq, in0=rsl, in1=rsl, op=ALU.mult)
                nc.tensor.matmul(ps2[:, c % 2, :], lhsT=onescol_bf, rhs=xsq,
                                 start=True, stop=True)
                if c % 2 == 1:
                    nc.scalar.copy(stg[:, 1, :], ps2.rearrange("p a b -> p (a b)"))
                    soff = CH * (c - 1)
                    nc.sync.dma_start(out=sc2_d[0:1, soff:soff + 2 * CH],
                                      in_=stg[:, 0, :])
                    nc.sync.dma_start(out=sc2_d[1:2, soff:soff + 2 * CH],
                                      in_=stg[:, 1, :])

        stats_math(sc2_d, ab2_d, bf16, extra_sum=sc1_d)

        # ============ PH5b: MLP + residual ============
        with tc.tile_pool(name="ph5b", bufs=3) as pool, \
             tc.tile_pool(name="ph5ab", bufs=2) as abpool, \
             tc.tile_pool(name="ph5ps", bufs=2, space="PSUM") as psum:
            ab2_flat = ab2_d[:, :].rearrange("a b -> (a b)")
            for c in range(NCHUNK):
                g, i = c // 4, c % 4
                rsl = r1[:, CH * c:CH * (c + 1)]
                if i == 0:
                    paB5 = abpool.tile([C, 4, CH], bf16, tag="pa5B")
                    nc.sync.dma_start(
                        out=paB5,
                        in_=bass.AP(tensor=ab2_flat.tensor, offset=4 * CH * g,
                                    ap=[[0, C], [1, 4 * CH]]))
                    pbB5 = abpool.tile([C, 4, CH], bf16, tag="pb5B")
                    nc.sync.dma_start(
                        out=pbB5,
                        in_=bass.AP(tensor=ab2_flat.tensor, offset=N + 4 * CH * g,
                                    ap=[[0, C], [1, 4 * CH]]))
                    yout4 = abpool.tile([C, 4, CH], f32, tag="yout4")
                t1 = pool.tile([C, CH], bf16, tag="t15")
                nc.gpsimd.tensor_tensor(out=t1, in0=rsl,
                                        in1=paB5[:, i, :], op=ALU.mult)
                xn = pool.tile([C, CH], bf16, tag="xn5")
                nc.gpsimd.tensor_tensor(out=xn, in0=t1, in1=pbB5[:, i, :], op=ALU.add)

                h1 = pool.tile([128, 3, CH], bf16, tag="h1")
                if has_c1:
                    for j in range(3):
                        pf = psum.tile([128, CH], f32, tag="pf")
                        nc.tensor.matmul(pf, lhsT=w1[:, 128 * j:128 * (j + 1)], rhs=xn,
                                         start=True, stop=True)
                        nc.scalar.activation(out=h1[:, j, :], in_=pf, func=AF.Gelu,
                                             bias=c1[j][:, 0:1], scale=1.0)
                else:
                    pf3 = psum.tile([128, 3, CH], f32, tag="pf3")
                    for j in range(3):
                        nc.tensor.matmul(pf3[:, j, :], lhsT=w1[:, 128 * j:128 * (j + 1)],
                                         rhs=xn, start=True, stop=True)
                    nc.scalar.activation(out=h1, in_=pf3, func=AF.Gelu)
                pm = psum.tile([C, CH], f32, tag="pm")
                if has_fc2b:
                    nc.tensor.matmul(pm, lhsT=b2t, rhs=onesrow, start=True, stop=False)
                for j in range(3):
                    nc.tensor.matmul(pm, lhsT=w2[j], rhs=h1[:, j, :],
                                     start=(j == 0 and not has_fc2b), stop=(j == 2))
                nc.vector.tensor_tensor(out=yout4[:, i, :], in0=rsl,
                                        in1=pm, op=ALU.add)
                if i == 3:
                    nc.sync.dma_start(out=y_d[:, 16 * g:16 * g + 16, :], in_=yout4)

        r1pool.release()
        opool.release()
        wpool.release()

    _split_multi_waits(nc, mybir)
    return nc


def _prep_weights(inputs):
    """Host-side weight preparation (fold LN affine, scale, transposes)."""
    qkv_w = np.asarray(inputs['qkv_w'], np.float32)       # (288, 96)
    proj_w = np.asarray(inputs['proj_w'], np.float32)     # (96, 96)
    proj_b = np.asarray(inputs['proj_b'], np.float32)
    ln1_w = np.asarray(inputs['ln1_w'], np.float32)
    ln1_b = np.asarray(inputs['ln1_b'], np.float32)
    ln2_w = np.asarray(inputs['ln2_w'], np.float32)
    ln2_b = np.asarray(inputs['ln2_b'], np.float32)
    fc1_w = np.asarray(inputs['fc1_w'], np.float32)       # (384, 96)
    fc1_b = np.asarray(inputs['fc1_b'], np.float32)
    fc2_w = np.asarray(inputs['fc2_w'], np.float32)       # (96, 384)
    fc2_b = np.asarray(inputs['fc2_b'], np.float32)

    wq = qkv_w * ln1_w[None, :]                            # (288, 96)
    c0 = qkv_w @ ln1_b                                     # (288,)
    wq[0:C] *= SCALE                                       # scale q rows
    c0[0:C] *= SCALE
    # v bias folds into proj bias; k bias cancels in softmax
    pb_eff = proj_b + proj_w @ c0[2 * C:3 * C]

    w1 = fc1_w * ln2_w[None, :]
    c1 = fc1_w @ ln2_b + fc1_b

    repl = np.zeros((128, 128), np.float32)
    for b in range(NB):
        for ch in range(GD):
            h0 = (ch // HD) * HD
            repl[32 * b + h0:32 * b + h0 + HD, 32 * b + ch] = 1.0

    # proj lhsT in per-band layout: wpb[32b+j, d, o] = proj_w[o, 32d+j];
    # extra column C holds proj_w column-sums so the matmul also emits the
    # per-token sum of the proj output (feeds LN2 stats).
    wpT = proj_w.T                                         # (in=96, out=96)
    colsum = proj_w.sum(axis=0)                            # (96,)
    wpb = np.zeros((128, 3, C + 1), np.float32)
    for b in range(NB):
        for d in range(3):
            wpb[32 * b:32 * b + 32, d, 0:C] = wpT[32 * d:32 * d + 32, :]
            wpb[32 * b:32 * b + 32, d, C] = colsum[32 * d:32 * d + 32]
    pb_ext = np.concatenate([pb_eff, [pb_eff.sum()]]).astype(np.float32)

    return {
        'wqkv': np.ascontiguousarray(wq.T),                # (96, 288) lhsT
        'c0': c0.reshape(-1, 1).astype(np.float32),
        'wproj': np.ascontiguousarray(wpb.reshape(128, 3 * (C + 1))),
        'projb': pb_ext.reshape(-1, 1),
        'w1': np.ascontiguousarray(w1.T),                  # (96, 384) lhsT
        'c1': c1.reshape(-1, 1).astype(np.float32),
        'w2': np.ascontiguousarray(fc2_w.T),               # (384, 96) lhsT
        'b2': fc2_b.reshape(-1, 1).astype(np.float32),
        'repl': repl,
        'onesc': np.ones((C, 1), np.float32),
    }


def kernel(**inputs):
    from concourse.bass_utils import run_bass_kernel_spmd

    wmap = _prep_weights(inputs)
    has_qbias = bool(np.any(wmap['c0'][0:C] != 0))
    has_projb = bool(np.any(wmap['projb'] != 0))
    has_fc2b = bool(np.any(wmap['b2'] != 0))
    has_c1 = bool(np.any(wmap['c1'] != 0))
    key = ('nc', has_qbias, has_projb, has_fc2b, has_c1)

    if key not in _cache:
        t0 = time.time()
        _cache[key] = _build(has_qbias, has_projb, has_fc2b, has_c1)
        print(f"[kernel] built bass module in {time.time() - t0:.1f}s",
              file=sys.stderr)

    nc = _cache[key]
    _cache['nc'] = nc
    x = np.asarray(inputs['x'], np.float32)                # (8, 96, 128, 128)

    in_maps = []
    for b in range(B):
        m = {'x': np.ascontiguousarray(x[b])}
        m.update(wmap)
        in_maps.append(m)

    res = run_bass_kernel_spmd(nc, in_maps, core_ids=list(range(B)))
    _cache['last_exec_ns'] = res.exec_time_ns
    out = np.stack([res.results[b]['y'] for b in range(B)], axis=0)
    return out.astype(np.float32)


# revision 75
# speedup vs baseline: 1.3534x; 1.0138x over previous
"""DilateBlock kernel for 8x Trainium2 NeuronCores (Bass/Tile).

Data-parallel over batch B=8 (one image per core). Per core, the whole block
(LN1 -> qkv -> 3-dilation 3x3 neighborhood attention -> proj -> residual ->
LN2 -> MLP -> residual) runs in channels-on-partitions layout; spatial shifts
for the attention unfold live on the free dimension of zero-padded (h, w)
planes, packed 4-hbands x 32-channels across partitions.

Key tricks vs the original:
  - LN stats PSUM rows DMA'd straight to DRAM (no Act-engine strip copies).
  - K/V qkv biases eliminated exactly (K bias shifts all 9 logits equally ->
    softmax-invariant; V bias folds into the proj bias on the host).
  - K/V scatter staged once to SBUF bf16 then spread across DVE/Act/Pool.
  - Attention output kept in SBUF; proj consumes it via per-band split
    matmuls (no DRAM round-trip for the attention output).
  - PH3 elementwise work split DVE/Pool; reciprocal in bf16.
  - Zero-bias specialization: bias ops are only emitted when the actual
    folded bias vectors are nonzero (they are zero for this problem's
    setup_inputs), with a general fallback path.
"""
import sys
import time

sys.path.insert(0, '/opt/trn_rl_repo')

import numpy as np

# ---- problem constants (hardcoded per contract) ----
B, C, H, W = 8, 96, 128, 128
DILS = (1, 2, 3)
GD = 32                 # channels per dilation branch
HD = 16                 # head dim
NB = 4                  # h-bands packed on partitions
BH = H // NB            # rows per band = 32
N = H * W               # tokens per image
NCHUNK = 32             # token chunks of 512 (4 image rows each)
CH = N // NCHUNK        # 512
PADR = 38               # BH + 6 halo rows
PADC = 135              # W + 6 halo cols (odd pitch: even bf16 tap offsets)
EPS = 1e-5
SCALE = HD ** -0.5
MLPH = 384

_cache = {}


def _patch_tile(tile_mod, bass_mod):
    """Work around this walrus build's 1-sem-wait-per-instruction limit and
    the multi-wait tail drain."""
    from concourse.vector_clock import ScopedClock, VectorClock

    def _drain_and_barrier(self, tick_clock, wait_clock):
        vclock = tick_clock.global_clock
        n = len(vclock)
        idxs = [i for i in range(n) if vclock[i] > 0]
        for i in idxs:
            vec = [0] * n
            vec[i] = vclock[i]
            nop_inst = self.nc.sync.nop(nofuse=True)
            wait_clock.add_sem_waits(nop_inst.ins,
                                     ScopedClock({None: VectorClock(vec)}))
        self.nc.sync.drain()
        self.nc.all_engine_barrier()
        popped = self.nc._tile_sem_poison_stack.pop()
        assert popped is self._sem_poison
        self.nc.clear_and_free_semaphores(list(self.sems.allocated().values()))
        self.nc.all_engine_barrier()

    tile_mod.TileContext._drain_and_barrier = _drain_and_barrier


_ws_counter = [0]


def _split_multi_waits(nc, mybir):
    for fn in nc.m.functions:
        for blk in fn.blocks:
            insts = list(blk.instructions)
            out = []
            changed = False
            for inst in insts:
                si = inst.sync_info
                waits = list(si.on_wait) if si and si.on_wait else []
                if len(waits) > 1:
                    for w in waits[:-1]:
                        _ws_counter[0] += 1
                        out.append(mybir.InstNoOp(
                            name=f"I-ws-{_ws_counter[0]}",
                            engine=inst.engine, ins=[], outs=[],
                            sync_info=mybir.SyncInfo(on_wait=[w], on_update=[])))
                    si.on_wait = [waits[-1]]
                    changed = True
                out.append(inst)
            if changed:
                blk.instructions[:] = out


def _build(has_qbias, has_projb, has_fc2b, has_c1=True):
    import concourse.bass as bass
    import concourse.tile as tile
    from concourse import mybir

    _patch_tile(tile, bass)

    f32 = mybir.dt.float32
    f32r = mybir.dt.float32r
    bf16 = mybir.dt.bfloat16
    AF = mybir.ActivationFunctionType
    ALU = mybir.AluOpType

    nc = bass.Bass()

    # ---- DRAM I/O ----
    x_d = nc.dram_tensor("x", (C, H, W), f32, kind="ExternalInput")
    wq_d = nc.dram_tensor("wqkv", (C, 3 * C), f32, kind="ExternalInput")   # lhsT
    c0_d = nc.dram_tensor("c0", (3 * C, 1), f32, kind="ExternalInput")
    wp_d = nc.dram_tensor("wproj", (128, 3 * (C + 1)), f32, kind="ExternalInput")
    pb_d = nc.dram_tensor("projb", (C + 1, 1), f32, kind="ExternalInput")
    w1_d = nc.dram_tensor("w1", (C, MLPH), f32, kind="ExternalInput")      # lhsT
    c1_d = nc.dram_tensor("c1", (MLPH, 1), f32, kind="ExternalInput")
    w2_d = nc.dram_tensor("w2", (MLPH, C), f32, kind="ExternalInput")      # lhsT
    b2_d = nc.dram_tensor("b2", (C, 1), f32, kind="ExternalInput")
    repl_d = nc.dram_tensor("repl", (128, 128), f32, kind="ExternalInput")
    ones_d = nc.dram_tensor("onesc", (C, 1), f32, kind="ExternalInput")

    y_d = nc.dram_tensor("y", (C, H, W), f32, kind="ExternalOutput")
    sc1_d = nc.dram_tensor("sc1", (2, N), f32, kind="ExternalOutput")
    sc2_d = nc.dram_tensor("sc2", (2, N), f32, kind="ExternalOutput")
    ab1_d = nc.dram_tensor("ab1", (2, N), bf16, kind="ExternalOutput")
    ab2_d = nc.dram_tensor("ab2", (2, N), bf16, kind="ExternalOutput")

    with tile.TileContext(nc) as tc:
        # ---------------- persistent pools ----------------
        # Allocate weight tiles up front; only PH1's inputs are DMA'd now.
        # The heavy cast-DMAs are deferred until after PH1's x loads so the
        # gpsimd DMA queue starts streaming x immediately.
        wpool = tc.alloc_tile_pool(name="weights", bufs=1)
        wq = wpool.tile([C, 3 * C], bf16)
        c0 = wpool.tile([C, 1], f32)            # q bias (scaled), only rows 0:C used
        wpb = wpool.tile([128, 3, C + 1], bf16)  # proj lhsT per band + sum row
        pb_row = wpool.tile([1, C + 1], f32r, tag="pbr", name="pbr") \
            if has_projb else None
        w1 = wpool.tile([C, MLPH], bf16)
        c1 = [wpool.tile([128, 1], f32, tag=f"c1{i}", name=f"c1{i}") for i in range(3)]
        w2 = [wpool.tile([128, C], bf16, tag=f"w2{i}", name=f"w2{i}") for i in range(3)]
        b2t = wpool.tile([1, C], f32r, tag="b2t", name="b2t") \
            if has_fc2b else None
        repl = wpool.tile([128, 128], bf16)
        onescol = wpool.tile([C, 1], f32r)      # stats lhsT [96,1]
        nc.sync.dma_start(out=onescol, in_=ones_d[:, :].bitcast(f32r))
        onescol_bf = wpool.tile([C, 1], bf16)   # stats lhsT for bf16 rhs
        nc.gpsimd.dma_start(out=onescol_bf, in_=ones_d[:, :])
        onesrow = None
        if has_projb or has_fc2b:
            onesrow = wpool.tile([1, CH], f32r, tag="onesrow", name="onesrow")
            nc.vector.memset(onesrow.bitcast(f32), 1.0)
        epst = wpool.tile([128, 1], f32)
        nc.vector.memset(epst, EPS)

        def load_weights():
            nc.gpsimd.dma_start(out=wq, in_=wq_d[:, :])
            if has_qbias:
                nc.sync.dma_start(out=c0, in_=c0_d[0:C, :])
            nc.gpsimd.dma_start(out=wpb,
                                in_=wp_d[:, :].rearrange("p (a b) -> p a b", a=3))
            if has_projb:
                nc.sync.dma_start(
                    out=pb_row,
                    in_=pb_d[:, :].rearrange("a b -> b a").bitcast(f32r))
            nc.gpsimd.dma_start(out=w1, in_=w1_d[:, :])
            for i in range(3):
                nc.sync.dma_start(out=c1[i], in_=c1_d[128 * i:128 * (i + 1), :])
            for i in range(3):
                nc.gpsimd.dma_start(out=w2[i], in_=w2_d[128 * i:128 * (i + 1), :])
            if has_fc2b:
                nc.sync.dma_start(
                    out=b2t, in_=b2_d[:, :].rearrange("a b -> b a").bitcast(f32r))
            nc.gpsimd.dma_start(out=repl, in_=repl_d[:, :])

        # big persistent activation tensors
        opool = tc.alloc_tile_pool(name="opool", bufs=1)
        Od = [opool.tile([128, BH * W], bf16, tag=f"od{d}", name=f"od{d}") for d in range(3)]
        apool = tc.alloc_tile_pool(name="acts", bufs=1)
        Qd = [apool.tile([128, BH, W], bf16, tag=f"qd{d}", name=f"qd{d}") for d in range(3)]
        KVp = [apool.tile([128, 2, PADR, PADC], bf16, tag=f"kvp{d}", name=f"kvp{d}")
               for d in range(3)]
        for d in range(3):
            # zero only the halo borders (interior is fully overwritten)
            nc.gpsimd.memset(KVp[d][:, :, 0:3, :], 0.0)
            nc.gpsimd.memset(KVp[d][:, :, PADR - 3:PADR, :], 0.0)
            nc.gpsimd.memset(KVp[d][:, :, 3:PADR - 3, 0:3], 0.0)
            nc.gpsimd.memset(KVp[d][:, :, 3:PADR - 3, 3 + W:PADC], 0.0)

        # ============ PH1: LN1 stats sweep ============
        with tc.tile_pool(name="ph1", bufs=4) as pool, \
             tc.tile_pool(name="ph1st", bufs=2) as stpool, \
             tc.tile_pool(name="ph1ps", bufs=2, space="PSUM") as psum:
            for g in range(NCHUNK // 4):
                xt4 = pool.tile([C, 4, CH], bf16, tag="xt")
                nc.gpsimd.dma_start(out=xt4, in_=x_d[:, 16 * g:16 * g + 16, :])
                xsq4 = pool.tile([C, 4, CH], bf16, tag="xsq")
                nc.vector.tensor_tensor(out=xsq4, in0=xt4, in1=xt4, op=ALU.mult)
                for hh in range(2):
                    ps = psum.tile([1, 2, CH], f32, tag="ps")
                    ps2 = psum.tile([1, 2, CH], f32, tag="ps2")
                    for i in range(2):
                        nc.tensor.matmul(ps[:, i, :], lhsT=onescol_bf,
                                         rhs=xt4[:, 2 * hh + i, :], start=True, stop=True)
                        nc.tensor.matmul(ps2[:, i, :], lhsT=onescol_bf,
                                         rhs=xsq4[:, 2 * hh + i, :], start=True, stop=True)
                    stg = stpool.tile([1, 2, 2 * CH], f32, tag="stg")
                    nc.scalar.copy(stg[:, 0, :], ps.rearrange("p a b -> p (a b)"))
                    nc.scalar.copy(stg[:, 1, :], ps2.rearrange("p a b -> p (a b)"))
                    off = CH * (4 * g + 2 * hh)
                    nc.sync.dma_start(out=sc1_d[0:1, off:off + 2 * CH],
                                      in_=stg[:, 0, :])
                    nc.sync.dma_start(out=sc1_d[1:2, off:off + 2 * CH],
                                      in_=stg[:, 1, :])

        # ============ stats math (shared helper) ============
        def stats_math(sc_dram, ab_dram, ab_dt, extra_sum=None):
            with tc.tile_pool(name="stm", bufs=1) as pool:
                s0 = pool.tile([128, 128], f32, tag="s0")
                s1 = pool.tile([128, 128], f32, tag="s1")
                src = sc_dram[:, :].rearrange("a b -> (a b)")
                ap0 = [[128, 128], [1, 128]]
                nc.sync.dma_start(out=s0, in_=bass.AP(tensor=src.tensor, offset=0, ap=ap0))
                nc.sync.dma_start(out=s1, in_=bass.AP(tensor=src.tensor, offset=N, ap=ap0))
                if extra_sum is not None:
                    sx = pool.tile([128, 128], f32, tag="sx")
                    esrc = extra_sum[:, :].rearrange("a b -> (a b)")
                    nc.sync.dma_start(out=sx, in_=bass.AP(tensor=esrc.tensor,
                                                          offset=0, ap=ap0))
                    nc.vector.tensor_tensor(out=s0, in0=s0, in1=sx, op=ALU.add)
                mu = pool.tile([128, 128], f32, tag="mu")
                nc.scalar.mul(out=mu, in_=s0, mul=1.0 / C)
                ex2 = pool.tile([128, 128], f32, tag="ex2")
                nc.scalar.mul(out=ex2, in_=s1, mul=1.0 / C)
                var = pool.tile([128, 128], f32, tag="var")
                nc.vector.scalar_tensor_tensor(out=var, in0=mu, scalar=-1.0, in1=mu,
                                               op0=ALU.mult, op1=ALU.mult)
                nc.vector.tensor_tensor(out=var, in0=ex2, in1=var, op=ALU.add)
                sd = pool.tile([128, 128], f32, tag="sd")
                nc.scalar.activation(out=sd, in_=var, func=AF.Sqrt, bias=epst, scale=1.0)
                rs = pool.tile([128, 128], ab_dt, tag="rs")
                with nc.allow_low_precision(reason="ln scale rows"):
                    nc.vector.reciprocal(out=rs, in_=sd)
                nb = pool.tile([128, 128], ab_dt, tag="nb")
                nc.vector.scalar_tensor_tensor(out=nb, in0=mu, scalar=-1.0, in1=rs,
                                               op0=ALU.mult, op1=ALU.mult)
                dst = ab_dram[:, :].rearrange("a b -> (a b)")
                nc.sync.dma_start(out=bass.AP(tensor=dst.tensor, offset=0, ap=[[1, N]]),
                                  in_=rs)
                nc.sync.dma_start(out=bass.AP(tensor=dst.tensor, offset=N, ap=[[1, N]]),
                                  in_=nb)

        load_weights()
        stats_math(sc1_d, ab1_d, bf16)

        # ============ PH2: LN1 apply + qkv + scatter to Qd/KVp ============
        def k_sections(c):
            """(band, r0, r1) image-row ranges of chunk c hitting band halos."""
            lo, hi = 4 * c, 4 * c + 4
            out = []
            for b in range(NB):
                s_lo, s_hi = BH * b - 3, BH * b + BH + 3
                r0, r1 = max(lo, s_lo), min(hi, s_hi)
                if r0 < r1:
                    out.append((b, r0, r1))
            return out

        with tc.tile_pool(name="ph2", bufs=4) as pool, \
             tc.tile_pool(name="ph2ab", bufs=3) as abpool, \
             tc.tile_pool(name="ph2ps", bufs=2, space="PSUM") as psum:
            ab1_flat = ab1_d[:, :].rearrange("a b -> (a b)")
            for c in range(NCHUNK):
                g, i = c // 4, c % 4
                if i == 0:
                    xt4 = pool.tile([C, 4, CH], bf16, tag="xt2")
                    nc.gpsimd.dma_start(out=xt4, in_=x_d[:, 16 * g:16 * g + 16, :])
                    paB = abpool.tile([C, 4, CH], bf16, tag="paB")
                    nc.sync.dma_start(
                        out=paB,
                        in_=bass.AP(tensor=ab1_flat.tensor, offset=4 * CH * g,
                                    ap=[[0, C], [1, 4 * CH]]))
                    pbB = abpool.tile([C, 4, CH], bf16, tag="pbB")
                    nc.sync.dma_start(
                        out=pbB,
                        in_=bass.AP(tensor=ab1_flat.tensor, offset=N + 4 * CH * g,
                                    ap=[[0, C], [1, 4 * CH]]))
                xt = xt4[:, i, :]
                t1 = pool.tile([C, CH], bf16, tag="t1")
                nc.vector.tensor_tensor(out=t1, in0=xt, in1=paB[:, i, :], op=ALU.mult)
                xn = pool.tile([C, CH], bf16, tag="xn")
                nc.vector.tensor_tensor(out=xn, in0=t1, in1=pbB[:, i, :], op=ALU.add)

                pq = psum.tile([C, CH], f32, tag="pq")
                kv2 = psum.tile([C, 2, CH], f32, tag="kv2")
                nc.tensor.matmul(pq, lhsT=wq[:, 0:C], rhs=xn, start=True, stop=True)
                nc.tensor.matmul(kv2[:, 0, :], lhsT=wq[:, C:2 * C], rhs=xn,
                                 start=True, stop=True)
                nc.tensor.matmul(kv2[:, 1, :], lhsT=wq[:, 2 * C:3 * C], rhs=xn,
                                 start=True, stop=True)

                # stage k/v to SBUF bf16 once (Act), then scatter from SBUF
                kvs = pool.tile([C, 2, CH], bf16, tag="kvs")
                nc.scalar.copy(kvs, kv2)

                b = c // 8
                r_off = 4 * c - BH * b
                # Qd scatter: d=0 on DVE (from PSUM), d=1,2 on Act (from PSUM)
                for d in range(3):
                    dst = Qd[d][32 * b:32 * b + 32, r_off:r_off + 4, :]
                    src = pq[32 * d:32 * d + 32, :].rearrange("p (r w) -> p r w", r=4)
                    if d == 0:
                        if has_qbias:
                            nc.vector.tensor_scalar_add(
                                out=dst, in0=src,
                                scalar1=c0[32 * d:32 * d + 32, 0:1])
                        else:
                            nc.vector.tensor_copy(out=dst, in_=src)
                    else:
                        if has_qbias:
                            nc.scalar.activation(
                                out=dst, in_=src, func=AF.Identity,
                                bias=c0[32 * d:32 * d + 32, 0:1], scale=1.0)
                        else:
                            nc.scalar.copy(dst, src)
                # K/V scatter from kvs: d=0 DVE, d=1 split K->Act V->Pool,
                # d=2 Pool
                for d in range(3):
                    for (bb, ra, rb) in k_sections(c):
                        nrows = rb - ra
                        src = kvs[32 * d:32 * d + 32, :,
                                  (ra - 4 * c) * W:(rb - 4 * c) * W]
                        dst = KVp[d][32 * bb:32 * bb + 32, :,
                                     ra - (BH * bb - 3):rb - (BH * bb - 3), 3:3 + W]
                        srcr = src.rearrange("p a (r w) -> p a r w", r=nrows)
                        if d < 2:
                            nc.vector.tensor_copy(out=dst, in_=srcr)
                        else:
                            nc.gpsimd.tensor_copy(out=dst, in_=srcr)

        # ============ PH3: attention per dilation ============
        with tc.tile_pool(name="ph3", bufs=3) as pool, \
             tc.tile_pool(name="ph3p", bufs=3) as ppool, \
             tc.tile_pool(name="ph3f", bufs=3) as fpool, \
             tc.tile_pool(name="ph3acc", bufs=1) as acc, \
             tc.tile_pool(name="ph3ps", bufs=2, space="PSUM") as psum:
            for di, dil in enumerate(DILS):
                S = acc.tile([128, BH * W], bf16, tag="S")
                qv = Qd[di][:, :, :]
                kpl = KVp[di][:, 0, :, :]
                vpl = KVp[di][:, 1, :, :]
                for ti, (dr, dc) in enumerate([(i - 1, j - 1)
                                               for i in range(3) for j in range(3)]):
                    kwin = kpl[:, 3 + dr * dil:3 + dr * dil + BH,
                               3 + dc * dil:3 + dc * dil + W]
                    vwin = vpl[:, 3 + dr * dil:3 + dr * dil + BH,
                               3 + dc * dil:3 + dc * dil + W]
                    P = ppool.tile([128, BH, W], bf16, tag="P")
                    nc.vector.tensor_tensor(out=P, in0=qv, in1=kwin, op=ALU.mult)
                    Pf = P.rearrange("p r w -> p (r w)")
                    expL = fpool.tile([128, BH * W], bf16, tag="expL")
                    for half in range(2):
                        pl = psum.tile([128, 2048], f32, tag="pl")
                        for q in range(4):
                            nc.tensor.matmul(pl[:, 512 * q:512 * (q + 1)],
                                             lhsT=repl,
                                             rhs=Pf[:, 2048 * half + 512 * q:
                                                    2048 * half + 512 * (q + 1)],
                                             start=True, stop=True)
                        nc.scalar.activation(out=expL[:, 2048 * half:2048 * (half + 1)],
                                             in_=pl, func=AF.Exp)
                    ev = expL.rearrange("p (r w) -> p r w", r=BH)
                    if ti == 0:
                        nc.vector.tensor_copy(out=S, in_=expL)
                        nc.vector.tensor_tensor(
                            out=Od[di].rearrange("p (r w) -> p r w", r=BH),
                            in0=ev, in1=vwin, op=ALU.mult)
                    else:
                        # accumulation chains (S, O) stay on DVE; Pool gets
                        # only off-chain Pv products so slow ops don't extend
                        # the serial dependency chain
                        nc.vector.tensor_tensor(out=S, in0=S, in1=expL, op=ALU.add)
                        Pv = pool.tile([128, BH, W], bf16, tag="Pv")
                        peng = nc.gpsimd if ti in (2, 4, 6, 8) else nc.vector
                        peng.tensor_tensor(out=Pv, in0=ev, in1=vwin, op=ALU.mult)
                        nc.vector.tensor_tensor(out=Od[di], in0=Od[di],
                                                in1=Pv.rearrange("p r w -> p (r w)"),
                                                op=ALU.add)
                rcp = pool.tile([128, BH * W], bf16, tag="Pv", name="rcp")
                with nc.allow_low_precision(reason="softmax denom in bf16 is fine"):
                    nc.vector.reciprocal(out=rcp, in_=S)
                nc.vector.tensor_tensor(out=Od[di], in0=Od[di], in1=rcp, op=ALU.mult)

        apool.release()

        # ====== PH4: proj (from SBUF O tiles) + residual + LN2 stats ======
        r1pool = tc.alloc_tile_pool(name="r1p", bufs=1)
        r1 = r1pool.tile([C, N], bf16)
        with tc.tile_pool(name="ph4", bufs=4) as pool, \
             tc.tile_pool(name="ph4st", bufs=3) as stpool, \
             tc.tile_pool(name="ph4ps", bufs=4, space="PSUM") as psum, \
             tc.tile_pool(name="ph4ps2", bufs=1, space="PSUM") as psum2:
            for c in range(NCHUNK):
                g, i = c // 4, c % 4
                b = c // 8
                off = (4 * c - BH * b) * W
                if i == 0:
                    xt4 = pool.tile([C, 4, CH], f32, tag="xt4")
                    nc.sync.dma_start(out=xt4, in_=x_d[:, 16 * g:16 * g + 16, :])
                # proj matmul carries an extra output row: the column-sums of
                # proj_w, so row C of pp = sum_c proj_out[c, n]. Combined with
                # PH1's x-sums (sc1 row 0) in stats_math this gives the LN2
                # token sums without a dedicated matmul.
                pp = psum.tile([C + 1, CH], f32, tag="pp")
                if has_projb:
                    nc.tensor.matmul(pp, lhsT=pb_row, rhs=onesrow, start=True, stop=False)
                for d in range(3):
                    nc.tensor.matmul(pp, lhsT=wpb[32 * b:32 * b + 32, d, :],
                                     rhs=Od[d][32 * b:32 * b + 32, off:off + CH],
                                     start=(d == 0 and not has_projb),
                                     stop=(d == 2),
                                     tile_position=(32 * b, 0))
                rsl = r1[:, CH * c:CH * (c + 1)]
                nc.vector.tensor_tensor(out=rsl, in0=xt4[:, i, :], in1=pp[0:C, :],
                                        op=ALU.add)
                # LN2 stats inline
                if c % 2 == 0:
                    ps2 = psum2.tile([1, 2, CH], f32, tag="ps52")
                    stg = stpool.tile([1, 2, 2 * CH], f32, tag="stg5")
                nc.scalar.copy(stg[:, 0, (c % 2) * CH:(c % 2 + 1) * CH],
                               pp[C:C + 1, :])
                xsq = pool.tile([C, CH], bf16, tag="xsq5")
                nc.vector.tensor_tensor(out=xsq, in0=rsl, in1=rsl, op=ALU.mult)
                nc.tensor.matmul(ps2[:, c % 2, :], lhsT=onescol_bf, rhs=xsq,
                                 start=True, stop=True)
                if c % 2 == 1:
                    nc.scalar.copy(stg[:, 1, :], ps2.rearrange("p a b -> p (a b)"))
                    soff = CH * (c - 1)
                    nc.sync.dma_start(out=sc2_d[0:1, soff:soff + 2 * CH],
                                      in_=stg[:, 0, :])
                    nc.sync.dma_start(out=sc2_d[1:2, soff:soff + 2 * CH],
                                      in_=stg[:, 1, :])

        stats_math(sc2_d, ab2_d, bf16, extra_sum=sc1_d)

        # ============ PH5b: MLP + residual ============
        with tc.tile_pool(name="ph5b", bufs=4) as pool, \
             tc.tile_pool(name="ph5ab", bufs=3) as abpool, \
             tc.tile_pool(name="ph5ps", bufs=2, space="PSUM") as psum:
            ab2_flat = ab2_d[:, :].rearrange("a b -> (a b)")
            for c in range(NCHUNK):
                g, i = c // 4, c % 4
                rsl = r1[:, CH * c:CH * (c + 1)]
                if i == 0:
                    paB5 = abpool.tile([C, 4, CH], bf16, tag="pa5B")
                    nc.sync.dma_start(
                        out=paB5,
                        in_=bass.AP(tensor=ab2_flat.tensor, offset=4 * CH * g,
                                    ap=[[0, C], [1, 4 * CH]]))
                    pbB5 = abpool.tile([C, 4, CH], bf16, tag="pb5B")
                    nc.sync.dma_start(
                        out=pbB5,
                        in_=bass.AP(tensor=ab2_flat.tensor, offset=N + 4 * CH * g,
                                    ap=[[0, C], [1, 4 * CH]]))
                    yout4 = abpool.tile([C, 4, CH], f32, tag="yout4")
                t1 = pool.tile([C, CH], bf16, tag="t15")
                nc.gpsimd.tensor_tensor(out=t1, in0=rsl,
                                        in1=paB5[:, i, :], op=ALU.mult)
                xn = pool.tile([C, CH], bf16, tag="xn5")
                nc.gpsimd.tensor_tensor(out=xn, in0=t1, in1=pbB5[:, i, :], op=ALU.add)

                h1 = pool.tile([128, 3, CH], bf16, tag="h1")
                if has_c1:
                    for j in range(3):
                        pf = psum.tile([128, CH], f32, tag="pf")
                        nc.tensor.matmul(pf, lhsT=w1[:, 128 * j:128 * (j + 1)], rhs=xn,
                                         start=True, stop=True)
                        nc.scalar.activation(out=h1[:, j, :], in_=pf, func=AF.Gelu,
                                             bias=c1[j][:, 0:1], scale=1.0)
                else:
                    pf3 = psum.tile([128, 3, CH], f32, tag="pf3")
                    for j in range(3):
                        nc.tensor.matmul(pf3[:, j, :], lhsT=w1[:, 128 * j:128 * (j + 1)],
                                         rhs=xn, start=True, stop=True)
                    nc.scalar.activation(out=h1, in_=pf3, func=AF.Gelu)
                pm = psum.tile([C, CH], f32, tag="pm")
                if has_fc2b:
                    nc.tensor.matmul(pm, lhsT=b2t, rhs=onesrow, start=True, stop=False)
                for j in range(3):
                    nc.tensor.matmul(pm, lhsT=w2[j], rhs=h1[:, j, :],
                                     start=(j == 0 and not has_fc2b), stop=(j == 2))
                nc.vector.tensor_tensor(out=yout4[:, i, :], in0=rsl,
                                        in1=pm, op=ALU.add)
                if i == 3:
                    nc.sync.dma_start(out=y_d[:, 16 * g:16 * g + 16, :], in_=yout4)

        r1pool.release()
        opool.release()
        wpool.release()

    _split_multi_waits(nc, mybir)
    return nc


def _prep_weights(inputs):
    """Host-side weight preparation (fold LN affine, scale, transposes)."""
    qkv_w = np.asarray(inputs['qkv_w'], np.float32)       # (288, 96)
    proj_w = np.asarray(inputs['proj_w'], np.float32)     # (96, 96)
    proj_b = np.asarray(inputs['proj_b'], np.float32)
    ln1_w = np.asarray(inputs['ln1_w'], np.float32)
    ln1_b = np.asarray(inputs['ln1_b'], np.float32)
    ln2_w = np.asarray(inputs['ln2_w'], np.float32)
    ln2_b = np.asarray(inputs['ln2_b'], np.float32)
    fc1_w = np.asarray(inputs['fc1_w'], np.float32)       # (384, 96)
    fc1_b = np.asarray(inputs['fc1_b'], np.float32)
    fc2_w = np.asarray(inputs['fc2_w'], np.float32)       # (96, 384)
    fc2_b = np.asarray(inputs['fc2_b'], np.float32)

    wq = qkv_w * ln1_w[None, :]                            # (288, 96)
    c0 = qkv_w @ ln1_b                                     # (288,)
    wq[0:C] *= SCALE                                       # scale q rows
    c0[0:C] *= SCALE
    # v bias folds into proj bias; k bias cancels in softmax
    pb_eff = proj_b + proj_w @ c0[2 * C:3 * C]

    w1 = fc1_w * ln2_w[None, :]
    c1 = fc1_w @ ln2_b + fc1_b

    repl = np.zeros((128, 128), np.float32)
    for b in range(NB):
        for ch in range(GD):
            h0 = (ch // HD) * HD
            repl[32 * b + h0:32 * b + h0 + HD, 32 * b + ch] = 1.0

    # proj lhsT in per-band layout: wpb[32b+j, d, o] = proj_w[o, 32d+j];
    # extra column C holds proj_w column-sums so the matmul also emits the
    # per-token sum of the proj output (feeds LN2 stats).
    wpT = proj_w.T                                         # (in=96, out=96)
    colsum = proj_w.sum(axis=0)                            # (96,)
    wpb = np.zeros((128, 3, C + 1), np.float32)
    for b in range(NB):
        for d in range(3):
            wpb[32 * b:32 * b + 32, d, 0:C] = wpT[32 * d:32 * d + 32, :]
            wpb[32 * b:32 * b + 32, d, C] = colsum[32 * d:32 * d + 32]
    pb_ext = np.concatenate([pb_eff, [pb_eff.sum()]]).astype(np.float32)

    return {
        'wqkv': np.ascontiguousarray(wq.T),                # (96, 288) lhsT
        'c0': c0.reshape(-1, 1).astype(np.float32),
        'wproj': np.ascontiguousarray(wpb.reshape(128, 3 * (C + 1))),
        'projb': pb_ext.reshape(-1, 1),
        'w1': np.ascontiguousarray(w1.T),                  # (96, 384) lhsT
        'c1': c1.reshape(-1, 1).astype(np.float32),
        'w2': np.ascontiguousarray(fc2_w.T),               # (384, 96) lhsT
        'b2': fc2_b.reshape(-1, 1).astype(np.float32),
        'repl': repl,
        'onesc': np.ones((C, 1), np.float32),
    }


def kernel(**inputs):
    from concourse.bass_utils import run_bass_kernel_spmd

    wmap = _prep_weights(inputs)
    has_qbias = bool(np.any(wmap['c0'][0:C] != 0))
    has_projb = bool(np.any(wmap['projb'] != 0))
    has_fc2b = bool(np.any(wmap['b2'] != 0))
    has_c1 = bool(np.any(wmap['c1'] != 0))
    key = ('nc', has_qbias, has_projb, has_fc2b, has_c1)

    if key not in _cache:
        t0 = time.time()
        _cache[key] = _build(has_qbias, has_projb, has_fc2b, has_c1)
        print(f"[kernel] built bass module in {time.time() - t0:.1f}s",
              file=sys.stderr)

    nc = _cache[key]
    _cache['nc'] = nc
    x = np.asarray(inputs['x'], np.float32)                # (8, 96, 128, 128)

    in_maps = []
    for b in range(B):
        m = {'x': np.ascontiguousarray(x[b])}
        m.update(wmap)
        in_maps.append(m)

    res = run_bass_kernel_spmd(nc, in_maps, core_ids=list(range(B)))
    _cache['last_exec_ns'] = res.exec_time_ns
    out = np.stack([res.results[b]['y'] for b in range(B)], axis=0)
    return out.astype(np.float32)


# revision 76
# speedup vs baseline: 1.3583x; 1.0036x over previous
"""DilateBlock kernel for 8x Trainium2 NeuronCores (Bass/Tile).

Data-parallel over batch B=8 (one image per core). Per core, the whole block
(LN1 -> qkv -> 3-dilation 3x3 neighborhood attention -> proj -> residual ->
LN2 -> MLP -> residual) runs in channels-on-partitions layout; spatial shifts
for the attention unfold live on the free dimension of zero-padded (h, w)
planes, packed 4-hbands x 32-channels across partitions.

Key tricks vs the original:
  - LN stats PSUM rows DMA'd straight to DRAM (no Act-engine strip copies).
  - K/V qkv biases eliminated exactly (K bias shifts all 9 logits equally ->
    softmax-invariant; V bias folds into the proj bias on the host).
  - K/V scatter staged once to SBUF bf16 then spread across DVE/Act/Pool.
  - Attention output kept in SBUF; proj consumes it via per-band split
    matmuls (no DRAM round-trip for the attention output).
  - PH3 elementwise work split DVE/Pool; reciprocal in bf16.
  - Zero-bias specialization: bias ops are only emitted when the actual
    folded bias vectors are nonzero (they are zero for this problem's
    setup_inputs), with a general fallback path.
"""
import sys
import time

sys.path.insert(0, '/opt/trn_rl_repo')

import numpy as np

# ---- problem constants (hardcoded per contract) ----
B, C, H, W = 8, 96, 128, 128
DILS = (1, 2, 3)
GD = 32                 # channels per dilation branch
HD = 16                 # head dim
NB = 4                  # h-bands packed on partitions
BH = H // NB            # rows per band = 32
N = H * W               # tokens per image
NCHUNK = 32             # token chunks of 512 (4 image rows each)
CH = N // NCHUNK        # 512
PADR = 38               # BH + 6 halo rows
PADC = 135              # W + 6 halo cols (odd pitch: even bf16 tap offsets)
EPS = 1e-5
SCALE = HD ** -0.5
MLPH = 384

_cache = {}


def _patch_tile(tile_mod, bass_mod):
    """Work around this walrus build's 1-sem-wait-per-instruction limit and
    the multi-wait tail drain."""
    from concourse.vector_clock import ScopedClock, VectorClock

    def _drain_and_barrier(self, tick_clock, wait_clock):
        vclock = tick_clock.global_clock
        n = len(vclock)
        idxs = [i for i in range(n) if vclock[i] > 0]
        for i in idxs:
            vec = [0] * n
            vec[i] = vclock[i]
            nop_inst = self.nc.sync.nop(nofuse=True)
            wait_clock.add_sem_waits(nop_inst.ins,
                                     ScopedClock({None: VectorClock(vec)}))
        self.nc.sync.drain()
        self.nc.all_engine_barrier()
        popped = self.nc._tile_sem_poison_stack.pop()
        assert popped is self._sem_poison
        self.nc.clear_and_free_semaphores(list(self.sems.allocated().values()))
        self.nc.all_engine_barrier()

    tile_mod.TileContext._drain_and_barrier = _drain_and_barrier


_ws_counter = [0]


def _split_multi_waits(nc, mybir):
    for fn in nc.m.functions:
        for blk in fn.blocks:
            insts = list(blk.instructions)
            out = []
            changed = False
            for inst in insts:
                si = inst.sync_info
                waits = list(si.on_wait) if si and si.on_wait else []
                if len(waits) > 1:
                    for w in waits[:-1]:
                        _ws_counter[0] += 1
                        out.append(mybir.InstNoOp(
                            name=f"I-ws-{_ws_counter[0]}",
                            engine=inst.engine, ins=[], outs=[],
                            sync_info=mybir.SyncInfo(on_wait=[w], on_update=[])))
                    si.on_wait = [waits[-1]]
                    changed = True
                out.append(inst)
            if changed:
                blk.instructions[:] = out


def _build(has_qbias, has_projb, has_fc2b, has_c1=True):
    import concourse.bass as bass
    import concourse.tile as tile
    from concourse import mybir

    _patch_tile(tile, bass)

    f32 = mybir.dt.float32
    f32r = mybir.dt.float32r
    bf16 = mybir.dt.bfloat16
    AF = mybir.ActivationFunctionType
    ALU = mybir.AluOpType

    nc = bass.Bass()

    # ---- DRAM I/O ----
    x_d = nc.dram_tensor("x", (C, H, W), f32, kind="ExternalInput")
    wq_d = nc.dram_tensor("wqkv", (C, 3 * C), f32, kind="ExternalInput")   # lhsT
    c0_d = nc.dram_tensor("c0", (3 * C, 1), f32, kind="ExternalInput")
    wp_d = nc.dram_tensor("wproj", (128, 3 * (C + 1)), f32, kind="ExternalInput")
    pb_d = nc.dram_tensor("projb", (C + 1, 1), f32, kind="ExternalInput")
    w1_d = nc.dram_tensor("w1", (C, MLPH), f32, kind="ExternalInput")      # lhsT
    c1_d = nc.dram_tensor("c1", (MLPH, 1), f32, kind="ExternalInput")
    w2_d = nc.dram_tensor("w2", (MLPH, C), f32, kind="ExternalInput")      # lhsT
    b2_d = nc.dram_tensor("b2", (C, 1), f32, kind="ExternalInput")
    repl_d = nc.dram_tensor("repl", (128, 128), f32, kind="ExternalInput")
    ones_d = nc.dram_tensor("onesc", (C, 1), f32, kind="ExternalInput")

    y_d = nc.dram_tensor("y", (C, H, W), f32, kind="ExternalOutput")
    sc1_d = nc.dram_tensor("sc1", (2, N), f32, kind="ExternalOutput")
    sc2_d = nc.dram_tensor("sc2", (2, N), f32, kind="ExternalOutput")
    ab1_d = nc.dram_tensor("ab1", (2, N), bf16, kind="ExternalOutput")
    ab2_d = nc.dram_tensor("ab2", (2, N), bf16, kind="ExternalOutput")

    with tile.TileContext(nc) as tc:
        # ---------------- persistent pools ----------------
        # Allocate weight tiles up front; only PH1's inputs are DMA'd now.
        # The heavy cast-DMAs are deferred until after PH1's x loads so the
        # gpsimd DMA queue starts streaming x immediately.
        wpool = tc.alloc_tile_pool(name="weights", bufs=1)
        wq = wpool.tile([C, 3 * C], bf16)
        c0 = wpool.tile([C, 1], f32)            # q bias (scaled), only rows 0:C used
        wpb = wpool.tile([128, 3, C + 1], bf16)  # proj lhsT per band + sum row
        pb_row = wpool.tile([1, C + 1], f32r, tag="pbr", name="pbr") \
            if has_projb else None
        w1 = wpool.tile([C, MLPH], bf16)
        c1 = [wpool.tile([128, 1], f32, tag=f"c1{i}", name=f"c1{i}") for i in range(3)]
        w2 = [wpool.tile([128, C], bf16, tag=f"w2{i}", name=f"w2{i}") for i in range(3)]
        b2t = wpool.tile([1, C], f32r, tag="b2t", name="b2t") \
            if has_fc2b else None
        repl = wpool.tile([128, 128], bf16)
        onescol = wpool.tile([C, 1], f32r)      # stats lhsT [96,1]
        nc.sync.dma_start(out=onescol, in_=ones_d[:, :].bitcast(f32r))
        onescol_bf = wpool.tile([C, 1], bf16)   # stats lhsT for bf16 rhs
        nc.gpsimd.dma_start(out=onescol_bf, in_=ones_d[:, :])
        onesrow = None
        if has_projb or has_fc2b:
            onesrow = wpool.tile([1, CH], f32r, tag="onesrow", name="onesrow")
            nc.vector.memset(onesrow.bitcast(f32), 1.0)
        epst = wpool.tile([128, 1], f32)
        nc.vector.memset(epst, EPS)

        def load_weights():
            nc.gpsimd.dma_start(out=wq, in_=wq_d[:, :])
            if has_qbias:
                nc.sync.dma_start(out=c0, in_=c0_d[0:C, :])
            nc.gpsimd.dma_start(out=wpb,
                                in_=wp_d[:, :].rearrange("p (a b) -> p a b", a=3))
            if has_projb:
                nc.sync.dma_start(
                    out=pb_row,
                    in_=pb_d[:, :].rearrange("a b -> b a").bitcast(f32r))
            nc.gpsimd.dma_start(out=w1, in_=w1_d[:, :])
            for i in range(3):
                nc.sync.dma_start(out=c1[i], in_=c1_d[128 * i:128 * (i + 1), :])
            for i in range(3):
                nc.gpsimd.dma_start(out=w2[i], in_=w2_d[128 * i:128 * (i + 1), :])
            if has_fc2b:
                nc.sync.dma_start(
                    out=b2t, in_=b2_d[:, :].rearrange("a b -> b a").bitcast(f32r))
            nc.gpsimd.dma_start(out=repl, in_=repl_d[:, :])

        # big persistent activation tensors
        opool = tc.alloc_tile_pool(name="opool", bufs=1)
        Od = [opool.tile([128, BH * W], bf16, tag=f"od{d}", name=f"od{d}") for d in range(3)]
        apool = tc.alloc_tile_pool(name="acts", bufs=1)
        Qd = [apool.tile([128, BH, W], bf16, tag=f"qd{d}", name=f"qd{d}") for d in range(3)]
        KVp = [apool.tile([128, 2, PADR, PADC], bf16, tag=f"kvp{d}", name=f"kvp{d}")
               for d in range(3)]
        for d in range(3):
            # zero only the halo borders (interior is fully overwritten)
            nc.gpsimd.memset(KVp[d][:, :, 0:3, :], 0.0)
            nc.gpsimd.memset(KVp[d][:, :, PADR - 3:PADR, :], 0.0)
            nc.gpsimd.memset(KVp[d][:, :, 3:PADR - 3, 0:3], 0.0)
            nc.gpsimd.memset(KVp[d][:, :, 3:PADR - 3, 3 + W:PADC], 0.0)

        # ============ PH1: LN1 stats sweep ============
        with tc.tile_pool(name="ph1", bufs=4) as pool, \
             tc.tile_pool(name="ph1st", bufs=3) as stpool, \
             tc.tile_pool(name="ph1ps", bufs=2, space="PSUM") as psum:
            for g in range(NCHUNK // 4):
                xt4 = pool.tile([C, 4, CH], bf16, tag="xt")
                nc.gpsimd.dma_start(out=xt4, in_=x_d[:, 16 * g:16 * g + 16, :])
                xsq4 = pool.tile([C, 4, CH], bf16, tag="xsq")
                nc.vector.tensor_tensor(out=xsq4, in0=xt4, in1=xt4, op=ALU.mult)
                for hh in range(2):
                    ps = psum.tile([1, 2, CH], f32, tag="ps")
                    ps2 = psum.tile([1, 2, CH], f32, tag="ps2")
                    for i in range(2):
                        nc.tensor.matmul(ps[:, i, :], lhsT=onescol_bf,
                                         rhs=xt4[:, 2 * hh + i, :], start=True, stop=True)
                        nc.tensor.matmul(ps2[:, i, :], lhsT=onescol_bf,
                                         rhs=xsq4[:, 2 * hh + i, :], start=True, stop=True)
                    stg = stpool.tile([1, 2, 2 * CH], f32, tag="stg")
                    nc.scalar.copy(stg[:, 0, :], ps.rearrange("p a b -> p (a b)"))
                    nc.scalar.copy(stg[:, 1, :], ps2.rearrange("p a b -> p (a b)"))
                    off = CH * (4 * g + 2 * hh)
                    nc.sync.dma_start(out=sc1_d[0:1, off:off + 2 * CH],
                                      in_=stg[:, 0, :])
                    nc.sync.dma_start(out=sc1_d[1:2, off:off + 2 * CH],
                                      in_=stg[:, 1, :])

        # ============ stats math (shared helper) ============
        def stats_math(sc_dram, ab_dram, ab_dt, extra_sum=None):
            with tc.tile_pool(name="stm", bufs=1) as pool:
                s0 = pool.tile([128, 128], f32, tag="s0")
                s1 = pool.tile([128, 128], f32, tag="s1")
                src = sc_dram[:, :].rearrange("a b -> (a b)")
                ap0 = [[128, 128], [1, 128]]
                nc.sync.dma_start(out=s0, in_=bass.AP(tensor=src.tensor, offset=0, ap=ap0))
                nc.sync.dma_start(out=s1, in_=bass.AP(tensor=src.tensor, offset=N, ap=ap0))
                if extra_sum is not None:
                    sx = pool.tile([128, 128], f32, tag="sx")
                    esrc = extra_sum[:, :].rearrange("a b -> (a b)")
                    nc.sync.dma_start(out=sx, in_=bass.AP(tensor=esrc.tensor,
                                                          offset=0, ap=ap0))
                    nc.vector.tensor_tensor(out=s0, in0=s0, in1=sx, op=ALU.add)
                mu = pool.tile([128, 128], f32, tag="mu")
                nc.scalar.mul(out=mu, in_=s0, mul=1.0 / C)
                ex2 = pool.tile([128, 128], f32, tag="ex2")
                nc.scalar.mul(out=ex2, in_=s1, mul=1.0 / C)
                var = pool.tile([128, 128], f32, tag="var")
                nc.vector.scalar_tensor_tensor(out=var, in0=mu, scalar=-1.0, in1=mu,
                                               op0=ALU.mult, op1=ALU.mult)
                nc.vector.tensor_tensor(out=var, in0=ex2, in1=var, op=ALU.add)
                sd = pool.tile([128, 128], f32, tag="sd")
                nc.scalar.activation(out=sd, in_=var, func=AF.Sqrt, bias=epst, scale=1.0)
                rs = pool.tile([128, 128], ab_dt, tag="rs")
                with nc.allow_low_precision(reason="ln scale rows"):
                    nc.vector.reciprocal(out=rs, in_=sd)
                nb = pool.tile([128, 128], ab_dt, tag="nb")
                nc.vector.scalar_tensor_tensor(out=nb, in0=mu, scalar=-1.0, in1=rs,
                                               op0=ALU.mult, op1=ALU.mult)
                dst = ab_dram[:, :].rearrange("a b -> (a b)")
                nc.sync.dma_start(out=bass.AP(tensor=dst.tensor, offset=0, ap=[[1, N]]),
                                  in_=rs)
                nc.sync.dma_start(out=bass.AP(tensor=dst.tensor, offset=N, ap=[[1, N]]),
                                  in_=nb)

        load_weights()
        stats_math(sc1_d, ab1_d, bf16)

        # ============ PH2: LN1 apply + qkv + scatter to Qd/KVp ============
        def k_sections(c):
            """(band, r0, r1) image-row ranges of chunk c hitting band halos."""
            lo, hi = 4 * c, 4 * c + 4
            out = []
            for b in range(NB):
                s_lo, s_hi = BH * b - 3, BH * b + BH + 3
                r0, r1 = max(lo, s_lo), min(hi, s_hi)
                if r0 < r1:
                    out.append((b, r0, r1))
            return out

        with tc.tile_pool(name="ph2", bufs=4) as pool, \
             tc.tile_pool(name="ph2ab", bufs=3) as abpool, \
             tc.tile_pool(name="ph2ps", bufs=4, space="PSUM") as psum, \
             tc.tile_pool(name="ph2kv", bufs=2, space="PSUM") as kvpsum:
            ab1_flat = ab1_d[:, :].rearrange("a b -> (a b)")
            for c in range(NCHUNK):
                g, i = c // 4, c % 4
                if i == 0:
                    xt4 = pool.tile([C, 4, CH], bf16, tag="xt2")
                    nc.gpsimd.dma_start(out=xt4, in_=x_d[:, 16 * g:16 * g + 16, :])
                    paB = abpool.tile([C, 4, CH], bf16, tag="paB")
                    nc.sync.dma_start(
                        out=paB,
                        in_=bass.AP(tensor=ab1_flat.tensor, offset=4 * CH * g,
                                    ap=[[0, C], [1, 4 * CH]]))
                    pbB = abpool.tile([C, 4, CH], bf16, tag="pbB")
                    nc.sync.dma_start(
                        out=pbB,
                        in_=bass.AP(tensor=ab1_flat.tensor, offset=N + 4 * CH * g,
                                    ap=[[0, C], [1, 4 * CH]]))
                xt = xt4[:, i, :]
                t1 = pool.tile([C, CH], bf16, tag="t1")
                nc.vector.tensor_tensor(out=t1, in0=xt, in1=paB[:, i, :], op=ALU.mult)
                xn = pool.tile([C, CH], bf16, tag="xn")
                nc.vector.tensor_tensor(out=xn, in0=t1, in1=pbB[:, i, :], op=ALU.add)

                pq = psum.tile([C, CH], f32, tag="pq")
                kv2 = kvpsum.tile([C, 2, CH], f32, tag="kv2")
                nc.tensor.matmul(pq, lhsT=wq[:, 0:C], rhs=xn, start=True, stop=True)
                nc.tensor.matmul(kv2[:, 0, :], lhsT=wq[:, C:2 * C], rhs=xn,
                                 start=True, stop=True)
                nc.tensor.matmul(kv2[:, 1, :], lhsT=wq[:, 2 * C:3 * C], rhs=xn,
                                 start=True, stop=True)

                # stage k/v to SBUF bf16 once (Act), then scatter from SBUF
                kvs = pool.tile([C, 2, CH], bf16, tag="kvs")
                nc.scalar.copy(kvs, kv2)

                b = c // 8
                r_off = 4 * c - BH * b
                # Qd scatter: d=0 on DVE (from PSUM), d=1,2 on Act (from PSUM)
                for d in range(3):
                    dst = Qd[d][32 * b:32 * b + 32, r_off:r_off + 4, :]
                    src = pq[32 * d:32 * d + 32, :].rearrange("p (r w) -> p r w", r=4)
                    if d == 0:
                        if has_qbias:
                            nc.vector.tensor_scalar_add(
                                out=dst, in0=src,
                                scalar1=c0[32 * d:32 * d + 32, 0:1])
                        else:
                            nc.vector.tensor_copy(out=dst, in_=src)
                    else:
                        if has_qbias:
                            nc.scalar.activation(
                                out=dst, in_=src, func=AF.Identity,
                                bias=c0[32 * d:32 * d + 32, 0:1], scale=1.0)
                        else:
                            nc.scalar.copy(dst, src)
                # K/V scatter from kvs: d=0 DVE, d=1 split K->Act V->Pool,
                # d=2 Pool
                for d in range(3):
                    for (bb, ra, rb) in k_sections(c):
                        nrows = rb - ra
                        src = kvs[32 * d:32 * d + 32, :,
                                  (ra - 4 * c) * W:(rb - 4 * c) * W]
                        dst = KVp[d][32 * bb:32 * bb + 32, :,
                                     ra - (BH * bb - 3):rb - (BH * bb - 3), 3:3 + W]
                        srcr = src.rearrange("p a (r w) -> p a r w", r=nrows)
                        if d < 2:
                            nc.vector.tensor_copy(out=dst, in_=srcr)
                        else:
                            nc.gpsimd.tensor_copy(out=dst, in_=srcr)

        # ============ PH3: attention per dilation ============
        with tc.tile_pool(name="ph3", bufs=3) as pool, \
             tc.tile_pool(name="ph3p", bufs=3) as ppool, \
             tc.tile_pool(name="ph3f", bufs=3) as fpool, \
             tc.tile_pool(name="ph3acc", bufs=1) as acc, \
             tc.tile_pool(name="ph3ps", bufs=2, space="PSUM") as psum:
            for di, dil in enumerate(DILS):
                S = acc.tile([128, BH * W], bf16, tag="S")
                qv = Qd[di][:, :, :]
                kpl = KVp[di][:, 0, :, :]
                vpl = KVp[di][:, 1, :, :]
                for ti, (dr, dc) in enumerate([(i - 1, j - 1)
                                               for i in range(3) for j in range(3)]):
                    kwin = kpl[:, 3 + dr * dil:3 + dr * dil + BH,
                               3 + dc * dil:3 + dc * dil + W]
                    vwin = vpl[:, 3 + dr * dil:3 + dr * dil + BH,
                               3 + dc * dil:3 + dc * dil + W]
                    P = ppool.tile([128, BH, W], bf16, tag="P")
                    nc.vector.tensor_tensor(out=P, in0=qv, in1=kwin, op=ALU.mult)
                    Pf = P.rearrange("p r w -> p (r w)")
                    expL = fpool.tile([128, BH * W], bf16, tag="expL")
                    for half in range(2):
                        pl = psum.tile([128, 2048], f32, tag="pl")
                        for q in range(4):
                            nc.tensor.matmul(pl[:, 512 * q:512 * (q + 1)],
                                             lhsT=repl,
                                             rhs=Pf[:, 2048 * half + 512 * q:
                                                    2048 * half + 512 * (q + 1)],
                                             start=True, stop=True)
                        nc.scalar.activation(out=expL[:, 2048 * half:2048 * (half + 1)],
                                             in_=pl, func=AF.Exp)
                    ev = expL.rearrange("p (r w) -> p r w", r=BH)
                    if ti == 0:
                        nc.vector.tensor_copy(out=S, in_=expL)
                        nc.vector.tensor_tensor(
                            out=Od[di].rearrange("p (r w) -> p r w", r=BH),
                            in0=ev, in1=vwin, op=ALU.mult)
                    else:
                        # accumulation chains (S, O) stay on DVE; Pool gets
                        # only off-chain Pv products so slow ops don't extend
                        # the serial dependency chain
                        nc.vector.tensor_tensor(out=S, in0=S, in1=expL, op=ALU.add)
                        Pv = pool.tile([128, BH, W], bf16, tag="Pv")
                        peng = nc.gpsimd if ti in (2, 4, 6, 8) else nc.vector
                        peng.tensor_tensor(out=Pv, in0=ev, in1=vwin, op=ALU.mult)
                        nc.vector.tensor_tensor(out=Od[di], in0=Od[di],
                                                in1=Pv.rearrange("p r w -> p (r w)"),
                                                op=ALU.add)
                rcp = pool.tile([128, BH * W], bf16, tag="Pv", name="rcp")
                with nc.allow_low_precision(reason="softmax denom in bf16 is fine"):
                    nc.vector.reciprocal(out=rcp, in_=S)
                nc.vector.tensor_tensor(out=Od[di], in0=Od[di], in1=rcp, op=ALU.mult)

        apool.release()

        # ====== PH4: proj (from SBUF O tiles) + residual + LN2 stats ======
        r1pool = tc.alloc_tile_pool(name="r1p", bufs=1)
        r1 = r1pool.tile([C, N], bf16)
        with tc.tile_pool(name="ph4", bufs=4) as pool, \
             tc.tile_pool(name="ph4st", bufs=3) as stpool, \
             tc.tile_pool(name="ph4ps", bufs=4, space="PSUM") as psum, \
             tc.tile_pool(name="ph4ps2", bufs=2, space="PSUM") as psum2:
            for c in range(NCHUNK):
                g, i = c // 4, c % 4
                b = c // 8
                off = (4 * c - BH * b) * W
                if i == 0:
                    xt4 = pool.tile([C, 4, CH], f32, tag="xt4")
                    nc.sync.dma_start(out=xt4, in_=x_d[:, 16 * g:16 * g + 16, :])
                # proj matmul carries an extra output row: the column-sums of
                # proj_w, so row C of pp = sum_c proj_out[c, n]. Combined with
                # PH1's x-sums (sc1 row 0) in stats_math this gives the LN2
                # token sums without a dedicated matmul.
                pp = psum.tile([C + 1, CH], f32, tag="pp")
                if has_projb:
                    nc.tensor.matmul(pp, lhsT=pb_row, rhs=onesrow, start=True, stop=False)
                for d in range(3):
                    nc.tensor.matmul(pp, lhsT=wpb[32 * b:32 * b + 32, d, :],
                                     rhs=Od[d][32 * b:32 * b + 32, off:off + CH],
                                     start=(d == 0 and not has_projb),
                                     stop=(d == 2),
                                     tile_position=(32 * b, 0))
                rsl = r1[:, CH * c:CH * (c + 1)]
                nc.vector.tensor_tensor(out=rsl, in0=xt4[:, i, :], in1=pp[0:C, :],
                                        op=ALU.add)
                # LN2 stats inline
                if c % 2 == 0:
                    ps2 = psum2.tile([1, 2, CH], f32, tag="ps52")
                    stg = stpool.tile([1, 2, 2 * CH], f32, tag="stg5")
                nc.scalar.copy(stg[:, 0, (c % 2) * CH:(c % 2 + 1) * CH],
                               pp[C:C + 1, :])
                xsq = pool.tile([C, CH], bf16, tag="xsq5")
                nc.vector.tensor_tensor(out=xsq, in0=rsl, in1=rsl, op=ALU.mult)
                nc.tensor.matmul(ps2[:, c % 2, :], lhsT=onescol_bf, rhs=xsq,
                                 start=True, stop=True)
                if c % 2 == 1:
                    nc.scalar.copy(stg[:, 1, :], ps2.rearrange("p a b -> p (a b)"))
                    soff = CH * (c - 1)
                    nc.sync.dma_start(out=sc2_d[0:1, soff:soff + 2 * CH],
                                      in_=stg[:, 0, :])
                    nc.sync.dma_start(out=sc2_d[1:2, soff:soff + 2 * CH],
                                      in_=stg[:, 1, :])

        stats_math(sc2_d, ab2_d, bf16, extra_sum=sc1_d)

        # ============ PH5b: MLP + residual ============
        with tc.tile_pool(name="ph5b", bufs=4) as pool, \
             tc.tile_pool(name="ph5ab", bufs=3) as abpool, \
             tc.tile_pool(name="ph5ps", bufs=2, space="PSUM") as psum:
            ab2_flat = ab2_d[:, :].rearrange("a b -> (a b)")
            for c in range(NCHUNK):
                g, i = c // 4, c % 4
                rsl = r1[:, CH * c:CH * (c + 1)]
                if i == 0:
                    paB5 = abpool.tile([C, 4, CH], bf16, tag="pa5B")
                    nc.sync.dma_start(
                        out=paB5,
                        in_=bass.AP(tensor=ab2_flat.tensor, offset=4 * CH * g,
                                    ap=[[0, C], [1, 4 * CH]]))
                    pbB5 = abpool.tile([C, 4, CH], bf16, tag="pb5B")
                    nc.sync.dma_start(
                        out=pbB5,
                        in_=bass.AP(tensor=ab2_flat.tensor, offset=N + 4 * CH * g,
                                    ap=[[0, C], [1, 4 * CH]]))
                    yout4 = abpool.tile([C, 4, CH], f32, tag="yout4")
                t1 = pool.tile([C, CH], bf16, tag="t15")
                nc.gpsimd.tensor_tensor(out=t1, in0=rsl,
                                        in1=paB5[:, i, :], op=ALU.mult)
                xn = pool.tile([C, CH], bf16, tag="xn5")
                nc.gpsimd.tensor_tensor(out=xn, in0=t1, in1=pbB5[:, i, :], op=ALU.add)

                h1 = pool.tile([128, 3, CH], bf16, tag="h1")
                if has_c1:
                    for j in range(3):
                        pf = psum.tile([128, CH], f32, tag="pf")
                        nc.tensor.matmul(pf, lhsT=w1[:, 128 * j:128 * (j + 1)], rhs=xn,
                                         start=True, stop=True)
                        nc.scalar.activation(out=h1[:, j, :], in_=pf, func=AF.Gelu,
                                             bias=c1[j][:, 0:1], scale=1.0)
                else:
                    pf3 = psum.tile([128, 3, CH], f32, tag="pf3")
                    for j in range(3):
                        nc.tensor.matmul(pf3[:, j, :], lhsT=w1[:, 128 * j:128 * (j + 1)],
                                         rhs=xn, start=True, stop=True)
                    nc.scalar.activation(out=h1, in_=pf3, func=AF.Gelu)
                pm = psum.tile([C, CH], f32, tag="pm")
                if has_fc2b:
                    nc.tensor.matmul(pm, lhsT=b2t, rhs=onesrow, start=True, stop=False)
                for j in range(3):
                    nc.tensor.matmul(pm, lhsT=w2[j], rhs=h1[:, j, :],
                                     start=(j == 0 and not has_fc2b), stop=(j == 2))
                nc.vector.tensor_tensor(out=yout4[:, i, :], in0=rsl,
                                        in1=pm, op=ALU.add)
                if i == 3:
                    nc.sync.dma_start(out=y_d[:, 16 * g:16 * g + 16, :], in_=yout4)

        r1pool.release()
        opool.release()
        wpool.release()

    _split_multi_waits(nc, mybir)
    return nc


def _prep_weights(inputs):
    """Host-side weight preparation (fold LN affine, scale, transposes)."""
    qkv_w = np.asarray(inputs['qkv_w'], np.float32)       # (288, 96)
    proj_w = np.asarray(inputs['proj_w'], np.float32)     # (96, 96)
    proj_b = np.asarray(inputs['proj_b'], np.float32)
    ln1_w = np.asarray(inputs['ln1_w'], np.float32)
    ln1_b = np.asarray(inputs['ln1_b'], np.float32)
    ln2_w = np.asarray(inputs['ln2_w'], np.float32)
    ln2_b = np.asarray(inputs['ln2_b'], np.float32)
    fc1_w = np.asarray(inputs['fc1_w'], np.float32)       # (384, 96)
    fc1_b = np.asarray(inputs['fc1_b'], np.float32)
    fc2_w = np.asarray(inputs['fc2_w'], np.float32)       # (96, 384)
    fc2_b = np.asarray(inputs['fc2_b'], np.float32)

    wq = qkv_w * ln1_w[None, :]                            # (288, 96)
    c0 = qkv_w @ ln1_b                                     # (288,)
    wq[0:C] *= SCALE                                       # scale q rows
    c0[0:C] *= SCALE
    # v bias folds into proj bias; k bias cancels in softmax
    pb_eff = proj_b + proj_w @ c0[2 * C:3 * C]

    w1 = fc1_w * ln2_w[None, :]
    c1 = fc1_w @ ln2_b + fc1_b

    repl = np.zeros((128, 128), np.float32)
    for b in range(NB):
        for ch in range(GD):
            h0 = (ch // HD) * HD
            repl[32 * b + h0:32 * b + h0 + HD, 32 * b + ch] = 1.0

    # proj lhsT in per-band layout: wpb[32b+j, d, o] = proj_w[o, 32d+j];
    # extra column C holds proj_w column-sums so the matmul also emits the
    # per-token sum of the proj output (feeds LN2 stats).
    wpT = proj_w.T                                         # (in=96, out=96)
    colsum = proj_w.sum(axis=0)                            # (96,)
    wpb = np.zeros((128, 3, C + 1), np.float32)
    for b in range(NB):
        for d in range(3):
            wpb[32 * b:32 * b + 32, d, 0:C] = wpT[32 * d:32 * d + 32, :]
            wpb[32 * b:32 * b + 32, d, C] = colsum[32 * d:32 * d + 32]
    pb_ext = np.concatenate([pb_eff, [pb_eff.sum()]]).astype(np.float32)

    return {
        'wqkv': np.ascontiguousarray(wq.T),                # (96, 288) lhsT
        'c0': c0.reshape(-1, 1).astype(np.float32),
        'wproj': np.ascontiguousarray(wpb.reshape(128, 3 * (C + 1))),
        'projb': pb_ext.reshape(-1, 1),
        'w1': np.ascontiguousarray(w1.T),                  # (96, 384) lhsT
        'c1': c1.reshape(-1, 1).astype(np.float32),
        'w2': np.ascontiguousarray(fc2_w.T),               # (384, 96) lhsT
        'b2': fc2_b.reshape(-1, 1).astype(np.float32),
        'repl': repl,
        'onesc': np.ones((C, 1), np.float32),
    }


def kernel(**inputs):
    from concourse.bass_utils import run_bass_kernel_spmd

    wmap = _prep_weights(inputs)
    has_qbias = bool(np.any(wmap['c0'][0:C] != 0))
    has_projb = bool(np.any(wmap['projb'] != 0))
    has_fc2b = bool(np.any(wmap['b2'] != 0))
    has_c1 = bool(np.any(wmap['c1'] != 0))
    key = ('nc', has_qbias, has_projb, has_fc2b, has_c1)

    if key not in _cache:
        t0 = time.time()
        _cache[key] = _build(has_qbias, has_projb, has_fc2b, has_c1)
        print(f"[kernel] built bass module in {time.time() - t0:.1f}s",
              file=sys.stderr)

    nc = _cache[key]
    _cache['nc'] = nc
    x = np.asarray(inputs['x'], np.float32)                # (8, 96, 128, 128)

    in_maps = []
    for b in range(B):
        m = {'x': np.ascontiguousarray(x[b])}
        m.update(wmap)
        in_maps.append(m)

    res = run_bass_kernel_spmd(nc, in_maps, core_ids=list(range(B)))
    _cache['last_exec_ns'] = res.exec_time_ns
    out = np.stack([res.results[b]['y'] for b in range(B)], axis=0)
    return out.astype(np.float32)


# revision 79
# speedup vs baseline: 1.3604x; 1.0016x over previous
"""DilateBlock kernel for 8x Trainium2 NeuronCores (Bass/Tile).

Data-parallel over batch B=8 (one image per core). Per core, the whole block
(LN1 -> qkv -> 3-dilation 3x3 neighborhood attention -> proj -> residual ->
LN2 -> MLP -> residual) runs in channels-on-partitions layout; spatial shifts
for the attention unfold live on the free dimension of zero-padded (h, w)
planes, packed 4-hbands x 32-channels across partitions.

Key tricks vs the original:
  - LN stats PSUM rows DMA'd straight to DRAM (no Act-engine strip copies).
  - K/V qkv biases eliminated exactly (K bias shifts all 9 logits equally ->
    softmax-invariant; V bias folds into the proj bias on the host).
  - K/V scatter staged once to SBUF bf16 then spread across DVE/Act/Pool.
  - Attention output kept in SBUF; proj consumes it via per-band split
    matmuls (no DRAM round-trip for the attention output).
  - PH3 elementwise work split DVE/Pool; reciprocal in bf16.
  - Zero-bias specialization: bias ops are only emitted when the actual
    folded bias vectors are nonzero (they are zero for this problem's
    setup_inputs), with a general fallback path.
"""
import sys
import time

sys.path.insert(0, '/opt/trn_rl_repo')

import numpy as np

# ---- problem constants (hardcoded per contract) ----
B, C, H, W = 8, 96, 128, 128
DILS = (1, 2, 3)
GD = 32                 # channels per dilation branch
HD = 16                 # head dim
NB = 4                  # h-bands packed on partitions
BH = H // NB            # rows per band = 32
N = H * W               # tokens per image
NCHUNK = 32             # token chunks of 512 (4 image rows each)
CH = N // NCHUNK        # 512
PADR = 38               # BH + 6 halo rows
PADC = 135              # W + 6 halo cols (odd pitch: even bf16 tap offsets)
EPS = 1e-5
SCALE = HD ** -0.5
MLPH = 384

_cache = {}


def _patch_tile(tile_mod, bass_mod):
    """Work around this walrus build's 1-sem-wait-per-instruction limit and
    the multi-wait tail drain."""
    from concourse.vector_clock import ScopedClock, VectorClock

    def _drain_and_barrier(self, tick_clock, wait_clock):
        vclock = tick_clock.global_clock
        n = len(vclock)
        idxs = [i for i in range(n) if vclock[i] > 0]
        for i in idxs:
            vec = [0] * n
            vec[i] = vclock[i]
            nop_inst = self.nc.sync.nop(nofuse=True)
            wait_clock.add_sem_waits(nop_inst.ins,
                                     ScopedClock({None: VectorClock(vec)}))
        self.nc.sync.drain()
        self.nc.all_engine_barrier()
        popped = self.nc._tile_sem_poison_stack.pop()
        assert popped is self._sem_poison
        self.nc.clear_and_free_semaphores(list(self.sems.allocated().values()))
        self.nc.all_engine_barrier()

    tile_mod.TileContext._drain_and_barrier = _drain_and_barrier


_ws_counter = [0]


def _split_multi_waits(nc, mybir):
    for fn in nc.m.functions:
        for blk in fn.blocks:
            insts = list(blk.instructions)
            out = []
            changed = False
            for inst in insts:
                si = inst.sync_info
                waits = list(si.on_wait) if si and si.on_wait else []
                if len(waits) > 1:
                    for w in waits[:-1]:
                        _ws_counter[0] += 1
                        out.append(mybir.InstNoOp(
                            name=f"I-ws-{_ws_counter[0]}",
                            engine=inst.engine, ins=[], outs=[],
                            sync_info=mybir.SyncInfo(on_wait=[w], on_update=[])))
                    si.on_wait = [waits[-1]]
                    changed = True
                out.append(inst)
            if changed:
                blk.instructions[:] = out


def _build(has_qbias, has_projb, has_fc2b, has_c1=True):
    import concourse.bass as bass
    import concourse.tile as tile
    from concourse import mybir

    _patch_tile(tile, bass)

    f32 = mybir.dt.float32
    f32r = mybir.dt.float32r
    bf16 = mybir.dt.bfloat16
    AF = mybir.ActivationFunctionType
    ALU = mybir.AluOpType

    nc = bass.Bass()

    # ---- DRAM I/O ----
    x_d = nc.dram_tensor("x", (C, H, W), f32, kind="ExternalInput")
    wq_d = nc.dram_tensor("wqkv", (C, 3 * C), f32, kind="ExternalInput")   # lhsT
    c0_d = nc.dram_tensor("c0", (3 * C, 1), f32, kind="ExternalInput")
    wp_d = nc.dram_tensor("wproj", (128, 3 * (C + 1)), f32, kind="ExternalInput")
    pb_d = nc.dram_tensor("projb", (C + 1, 1), f32, kind="ExternalInput")
    w1_d = nc.dram_tensor("w1", (C, MLPH), f32, kind="ExternalInput")      # lhsT
    c1_d = nc.dram_tensor("c1", (MLPH, 1), f32, kind="ExternalInput")
    w2_d = nc.dram_tensor("w2", (MLPH, C), f32, kind="ExternalInput")      # lhsT
    b2_d = nc.dram_tensor("b2", (C, 1), f32, kind="ExternalInput")
    repl_d = nc.dram_tensor("repl", (128, 128), f32, kind="ExternalInput")
    ones_d = nc.dram_tensor("onesc", (C, 1), f32, kind="ExternalInput")

    y_d = nc.dram_tensor("y", (C, H, W), f32, kind="ExternalOutput")
    sc1_d = nc.dram_tensor("sc1", (2, N), f32, kind="ExternalOutput")
    sc2_d = nc.dram_tensor("sc2", (2, N), f32, kind="ExternalOutput")
    ab1_d = nc.dram_tensor("ab1", (2, N), bf16, kind="ExternalOutput")
    ab2_d = nc.dram_tensor("ab2", (2, N), bf16, kind="ExternalOutput")

    with tile.TileContext(nc) as tc:
        # ---------------- persistent pools ----------------
        # Allocate weight tiles up front; only PH1's inputs are DMA'd now.
        # The heavy cast-DMAs are deferred until after PH1's x loads so the
        # gpsimd DMA queue starts streaming x immediately.
        wpool = tc.alloc_tile_pool(name="weights", bufs=1)
        wq = wpool.tile([C, 3 * C], bf16)
        c0 = wpool.tile([C, 1], f32)            # q bias (scaled), only rows 0:C used
        wpb = wpool.tile([128, 3, C + 1], bf16)  # proj lhsT per band + sum row
        pb_row = wpool.tile([1, C + 1], f32r, tag="pbr", name="pbr") \
            if has_projb else None
        w1 = wpool.tile([C, MLPH], bf16)
        c1 = [wpool.tile([128, 1], f32, tag=f"c1{i}", name=f"c1{i}") for i in range(3)]
        w2 = [wpool.tile([128, C], bf16, tag=f"w2{i}", name=f"w2{i}") for i in range(3)]
        b2t = wpool.tile([1, C], f32r, tag="b2t", name="b2t") \
            if has_fc2b else None
        repl = wpool.tile([128, 128], bf16)
        onescol = wpool.tile([C, 1], f32r)      # stats lhsT [96,1]
        nc.sync.dma_start(out=onescol, in_=ones_d[:, :].bitcast(f32r))
        onescol_bf = wpool.tile([C, 1], bf16)   # stats lhsT for bf16 rhs
        nc.gpsimd.dma_start(out=onescol_bf, in_=ones_d[:, :])
        onesrow = None
        if has_projb or has_fc2b:
            onesrow = wpool.tile([1, CH], f32r, tag="onesrow", name="onesrow")
            nc.vector.memset(onesrow.bitcast(f32), 1.0)
        epst = wpool.tile([128, 1], f32)
        nc.vector.memset(epst, EPS)

        def load_weights():
            nc.gpsimd.dma_start(out=wq, in_=wq_d[:, :])
            if has_qbias:
                nc.sync.dma_start(out=c0, in_=c0_d[0:C, :])
            nc.gpsimd.dma_start(out=wpb,
                                in_=wp_d[:, :].rearrange("p (a b) -> p a b", a=3))
            if has_projb:
                nc.sync.dma_start(
                    out=pb_row,
                    in_=pb_d[:, :].rearrange("a b -> b a").bitcast(f32r))
            nc.gpsimd.dma_start(out=w1, in_=w1_d[:, :])
            for i in range(3):
                nc.sync.dma_start(out=c1[i], in_=c1_d[128 * i:128 * (i + 1), :])
            for i in range(3):
                nc.gpsimd.dma_start(out=w2[i], in_=w2_d[128 * i:128 * (i + 1), :])
            if has_fc2b:
                nc.sync.dma_start(
                    out=b2t, in_=b2_d[:, :].rearrange("a b -> b a").bitcast(f32r))
            nc.gpsimd.dma_start(out=repl, in_=repl_d[:, :])

        # big persistent activation tensors
        opool = tc.alloc_tile_pool(name="opool", bufs=1)
        Od = [opool.tile([128, BH * W], bf16, tag=f"od{d}", name=f"od{d}") for d in range(3)]
        apool = tc.alloc_tile_pool(name="acts", bufs=1)
        Qd = [apool.tile([128, BH, W], bf16, tag=f"qd{d}", name=f"qd{d}") for d in range(3)]
        KVp = [apool.tile([128, 2, PADR, PADC], bf16, tag=f"kvp{d}", name=f"kvp{d}")
               for d in range(3)]
        for d in range(3):
            # zero only the halo borders (interior is fully overwritten)
            nc.gpsimd.memset(KVp[d][:, :, 0:3, :], 0.0)
            nc.gpsimd.memset(KVp[d][:, :, PADR - 3:PADR, :], 0.0)
            nc.gpsimd.memset(KVp[d][:, :, 3:PADR - 3, 0:3], 0.0)
            nc.gpsimd.memset(KVp[d][:, :, 3:PADR - 3, 3 + W:PADC], 0.0)

        # ============ PH1: LN1 stats sweep ============
        with tc.tile_pool(name="ph1", bufs=4) as pool, \
             tc.tile_pool(name="ph1st", bufs=3) as stpool, \
             tc.tile_pool(name="ph1ps", bufs=2, space="PSUM") as psum:
            for g in range(NCHUNK // 4):
                xt4 = pool.tile([C, 4, CH], bf16, tag="xt")
                nc.gpsimd.dma_start(out=xt4, in_=x_d[:, 16 * g:16 * g + 16, :])
                xsq4 = pool.tile([C, 4, CH], bf16, tag="xsq")
                nc.vector.tensor_tensor(out=xsq4, in0=xt4, in1=xt4, op=ALU.mult)
                for hh in range(2):
                    ps = psum.tile([1, 2, CH], f32, tag="ps")
                    ps2 = psum.tile([1, 2, CH], f32, tag="ps2")
                    for i in range(2):
                        nc.tensor.matmul(ps[:, i, :], lhsT=onescol_bf,
                                         rhs=xt4[:, 2 * hh + i, :], start=True, stop=True)
                        nc.tensor.matmul(ps2[:, i, :], lhsT=onescol_bf,
                                         rhs=xsq4[:, 2 * hh + i, :], start=True, stop=True)
                    stg = stpool.tile([1, 2, 2 * CH], f32, tag="stg")
                    nc.scalar.copy(stg[:, 0, :], ps.rearrange("p a b -> p (a b)"))
                    nc.scalar.copy(stg[:, 1, :], ps2.rearrange("p a b -> p (a b)"))
                    off = CH * (4 * g + 2 * hh)
                    nc.sync.dma_start(out=sc1_d[0:1, off:off + 2 * CH],
                                      in_=stg[:, 0, :])
                    nc.sync.dma_start(out=sc1_d[1:2, off:off + 2 * CH],
                                      in_=stg[:, 1, :])

        # ============ stats math (shared helper) ============
        def stats_math(sc_dram, ab_dram, ab_dt, extra_sum=None):
            with tc.tile_pool(name="stm", bufs=1) as pool:
                s0 = pool.tile([128, 128], f32, tag="s0")
                s1 = pool.tile([128, 128], f32, tag="s1")
                src = sc_dram[:, :].rearrange("a b -> (a b)")
                ap0 = [[128, 128], [1, 128]]
                nc.sync.dma_start(out=s0, in_=bass.AP(tensor=src.tensor, offset=0, ap=ap0))
                nc.sync.dma_start(out=s1, in_=bass.AP(tensor=src.tensor, offset=N, ap=ap0))
                if extra_sum is not None:
                    sx = pool.tile([128, 128], f32, tag="sx")
                    esrc = extra_sum[:, :].rearrange("a b -> (a b)")
                    nc.sync.dma_start(out=sx, in_=bass.AP(tensor=esrc.tensor,
                                                          offset=0, ap=ap0))
                    nc.vector.tensor_tensor(out=s0, in0=s0, in1=sx, op=ALU.add)
                mu = pool.tile([128, 128], f32, tag="mu")
                nc.scalar.mul(out=mu, in_=s0, mul=1.0 / C)
                ex2 = pool.tile([128, 128], f32, tag="ex2")
                nc.scalar.mul(out=ex2, in_=s1, mul=1.0 / C)
                var = pool.tile([128, 128], f32, tag="var")
                nc.vector.scalar_tensor_tensor(out=var, in0=mu, scalar=-1.0, in1=mu,
                                               op0=ALU.mult, op1=ALU.mult)
                nc.vector.tensor_tensor(out=var, in0=ex2, in1=var, op=ALU.add)
                sd = pool.tile([128, 128], f32, tag="sd")
                nc.scalar.activation(out=sd, in_=var, func=AF.Sqrt, bias=epst, scale=1.0)
                rs = pool.tile([128, 128], ab_dt, tag="rs")
                with nc.allow_low_precision(reason="ln scale rows"):
                    nc.vector.reciprocal(out=rs, in_=sd)
                nb = pool.tile([128, 128], ab_dt, tag="nb")
                nc.vector.scalar_tensor_tensor(out=nb, in0=mu, scalar=-1.0, in1=rs,
                                               op0=ALU.mult, op1=ALU.mult)
                dst = ab_dram[:, :].rearrange("a b -> (a b)")
                nc.sync.dma_start(out=bass.AP(tensor=dst.tensor, offset=0, ap=[[1, N]]),
                                  in_=rs)
                nc.sync.dma_start(out=bass.AP(tensor=dst.tensor, offset=N, ap=[[1, N]]),
                                  in_=nb)

        load_weights()
        stats_math(sc1_d, ab1_d, bf16)

        # ============ PH2: LN1 apply + qkv + scatter to Qd/KVp ============
        def k_sections(c):
            """(band, r0, r1) image-row ranges of chunk c hitting band halos."""
            lo, hi = 4 * c, 4 * c + 4
            out = []
            for b in range(NB):
                s_lo, s_hi = BH * b - 3, BH * b + BH + 3
                r0, r1 = max(lo, s_lo), min(hi, s_hi)
                if r0 < r1:
                    out.append((b, r0, r1))
            return out

        with tc.tile_pool(name="ph2", bufs=4) as pool, \
             tc.tile_pool(name="ph2ab", bufs=3) as abpool, \
             tc.tile_pool(name="ph2ps", bufs=4, space="PSUM") as psum, \
             tc.tile_pool(name="ph2kv", bufs=2, space="PSUM") as kvpsum:
            ab1_flat = ab1_d[:, :].rearrange("a b -> (a b)")
            for c in range(NCHUNK):
                g, i = c // 4, c % 4
                if i == 0:
                    xt4 = pool.tile([C, 4, CH], bf16, tag="xt2")
                    nc.gpsimd.dma_start(out=xt4, in_=x_d[:, 16 * g:16 * g + 16, :])
                    paB = abpool.tile([C, 4, CH], bf16, tag="paB")
                    nc.sync.dma_start(
                        out=paB,
                        in_=bass.AP(tensor=ab1_flat.tensor, offset=4 * CH * g,
                                    ap=[[0, C], [1, 4 * CH]]))
                    pbB = abpool.tile([C, 4, CH], bf16, tag="pbB")
                    nc.sync.dma_start(
                        out=pbB,
                        in_=bass.AP(tensor=ab1_flat.tensor, offset=N + 4 * CH * g,
                                    ap=[[0, C], [1, 4 * CH]]))
                xt = xt4[:, i, :]
                t1 = pool.tile([C, CH], bf16, tag="t1")
                nc.vector.tensor_tensor(out=t1, in0=xt, in1=paB[:, i, :], op=ALU.mult)
                xn = pool.tile([C, CH], bf16, tag="xn")
                nc.vector.tensor_tensor(out=xn, in0=t1, in1=pbB[:, i, :], op=ALU.add)

                pq = psum.tile([C, CH], f32, tag="pq")
                kv2 = kvpsum.tile([C, 2, CH], f32, tag="kv2")
                nc.tensor.matmul(pq, lhsT=wq[:, 0:C], rhs=xn, start=True, stop=True)
                nc.tensor.matmul(kv2[:, 0, :], lhsT=wq[:, C:2 * C], rhs=xn,
                                 start=True, stop=True)
                nc.tensor.matmul(kv2[:, 1, :], lhsT=wq[:, 2 * C:3 * C], rhs=xn,
                                 start=True, stop=True)

                # stage k/v to SBUF bf16 once (Act), then scatter from SBUF
                kvs = pool.tile([C, 2, CH], bf16, tag="kvs")
                nc.scalar.copy(kvs, kv2)

                b = c // 8
                r_off = 4 * c - BH * b
                # Qd scatter: d=0 on DVE (from PSUM), d=1,2 on Act (from PSUM)
                for d in range(3):
                    dst = Qd[d][32 * b:32 * b + 32, r_off:r_off + 4, :]
                    src = pq[32 * d:32 * d + 32, :].rearrange("p (r w) -> p r w", r=4)
                    if d == 0:
                        if has_qbias:
                            nc.vector.tensor_scalar_add(
                                out=dst, in0=src,
                                scalar1=c0[32 * d:32 * d + 32, 0:1])
                        else:
                            nc.vector.tensor_copy(out=dst, in_=src)
                    else:
                        if has_qbias:
                            nc.scalar.activation(
                                out=dst, in_=src, func=AF.Identity,
                                bias=c0[32 * d:32 * d + 32, 0:1], scale=1.0)
                        else:
                            nc.scalar.copy(dst, src)
                # K/V scatter from kvs: d=0 DVE, d=1 split K->Act V->Pool,
                # d=2 Pool
                for d in range(3):
                    for (bb, ra, rb) in k_sections(c):
                        nrows = rb - ra
                        src = kvs[32 * d:32 * d + 32, :,
                                  (ra - 4 * c) * W:(rb - 4 * c) * W]
                        dst = KVp[d][32 * bb:32 * bb + 32, :,
                                     ra - (BH * bb - 3):rb - (BH * bb - 3), 3:3 + W]
                        srcr = src.rearrange("p a (r w) -> p a r w", r=nrows)
                        if d < 2:
                            nc.vector.tensor_copy(out=dst, in_=srcr)
                        else:
                            nc.gpsimd.tensor_copy(out=dst, in_=srcr)

        # ============ PH3: attention per dilation ============
        with tc.tile_pool(name="ph3", bufs=3) as pool, \
             tc.tile_pool(name="ph3p", bufs=3) as ppool, \
             tc.tile_pool(name="ph3f", bufs=3) as fpool, \
             tc.tile_pool(name="ph3acc", bufs=1) as acc, \
             tc.tile_pool(name="ph3ps", bufs=4, space="PSUM") as psum:
            for di, dil in enumerate(DILS):
                S = acc.tile([128, BH * W], bf16, tag="S")
                qv = Qd[di][:, :, :]
                kpl = KVp[di][:, 0, :, :]
                vpl = KVp[di][:, 1, :, :]
                for ti, (dr, dc) in enumerate([(i - 1, j - 1)
                                               for i in range(3) for j in range(3)]):
                    kwin = kpl[:, 3 + dr * dil:3 + dr * dil + BH,
                               3 + dc * dil:3 + dc * dil + W]
                    vwin = vpl[:, 3 + dr * dil:3 + dr * dil + BH,
                               3 + dc * dil:3 + dc * dil + W]
                    P = ppool.tile([128, BH, W], bf16, tag="P")
                    nc.vector.tensor_tensor(out=P, in0=qv, in1=kwin, op=ALU.mult)
                    Pf = P.rearrange("p r w -> p (r w)")
                    expL = fpool.tile([128, BH * W], bf16, tag="expL")
                    for qt in range(4):
                        pl = psum.tile([128, 1024], f32, tag="pl")
                        for q in range(2):
                            nc.tensor.matmul(pl[:, 512 * q:512 * (q + 1)],
                                             lhsT=repl,
                                             rhs=Pf[:, 1024 * qt + 512 * q:
                                                    1024 * qt + 512 * (q + 1)],
                                             start=True, stop=True)
                        nc.scalar.activation(out=expL[:, 1024 * qt:1024 * (qt + 1)],
                                             in_=pl, func=AF.Exp)
                    ev = expL.rearrange("p (r w) -> p r w", r=BH)
                    if ti == 0:
                        nc.vector.tensor_copy(out=S, in_=expL)
                        nc.vector.tensor_tensor(
                            out=Od[di].rearrange("p (r w) -> p r w", r=BH),
                            in0=ev, in1=vwin, op=ALU.mult)
                    else:
                        # accumulation chains (S, O) stay on DVE; Pool gets
                        # only off-chain Pv products so slow ops don't extend
                        # the serial dependency chain
                        nc.vector.tensor_tensor(out=S, in0=S, in1=expL, op=ALU.add)
                        Pv = pool.tile([128, BH, W], bf16, tag="Pv")
                        peng = nc.gpsimd if ti in (2, 4, 6, 8) else nc.vector
                        peng.tensor_tensor(out=Pv, in0=ev, in1=vwin, op=ALU.mult)
                        nc.vector.tensor_tensor(out=Od[di], in0=Od[di],
                                                in1=Pv.rearrange("p r w -> p (r w)"),
                                                op=ALU.add)
                rcp = pool.tile([128, BH * W], bf16, tag="Pv", name="rcp")
                with nc.allow_low_precision(reason="softmax denom in bf16 is fine"):
                    nc.vector.reciprocal(out=rcp, in_=S)
                nc.vector.tensor_tensor(out=Od[di], in0=Od[di], in1=rcp, op=ALU.mult)

        apool.release()

        # ====== PH4: proj (from SBUF O tiles) + residual + LN2 stats ======
        r1pool = tc.alloc_tile_pool(name="r1p", bufs=1)
        r1 = r1pool.tile([C, N], bf16)
        with tc.tile_pool(name="ph4", bufs=4) as pool, \
             tc.tile_pool(name="ph4st", bufs=3) as stpool, \
             tc.tile_pool(name="ph4ps", bufs=4, space="PSUM") as psum, \
             tc.tile_pool(name="ph4ps2", bufs=2, space="PSUM") as psum2:
            for c in range(NCHUNK):
                g, i = c // 4, c % 4
                b = c // 8
                off = (4 * c - BH * b) * W
                if i == 0:
                    xt4 = pool.tile([C, 4, CH], f32, tag="xt4")
                    nc.sync.dma_start(out=xt4, in_=x_d[:, 16 * g:16 * g + 16, :])
                # proj matmul carries an extra output row: the column-sums of
                # proj_w, so row C of pp = sum_c proj_out[c, n]. Combined with
                # PH1's x-sums (sc1 row 0) in stats_math this gives the LN2
                # token sums without a dedicated matmul.
                pp = psum.tile([C + 1, CH], f32, tag="pp")
                if has_projb:
                    nc.tensor.matmul(pp, lhsT=pb_row, rhs=onesrow, start=True, stop=False)
                for d in range(3):
                    nc.tensor.matmul(pp, lhsT=wpb[32 * b:32 * b + 32, d, :],
                                     rhs=Od[d][32 * b:32 * b + 32, off:off + CH],
                                     start=(d == 0 and not has_projb),
                                     stop=(d == 2),
                                     tile_position=(32 * b, 0))
                rsl = r1[:, CH * c:CH * (c + 1)]
                nc.vector.tensor_tensor(out=rsl, in0=xt4[:, i, :], in1=pp[0:C, :],
                                        op=ALU.add)
                # LN2 stats inline
                if c % 2 == 0:
                    ps2 = psum2.tile([1, 2, CH], f32, tag="ps52")
                    stg = stpool.tile([1, 2, 2 * CH], f32, tag="stg5")
                nc.scalar.copy(stg[:, 0, (c % 2) * CH:(c % 2 + 1) * CH],
                               pp[C:C + 1, :])
                xsq = pool.tile([C, CH], bf16, tag="xsq5")
                nc.vector.tensor_tensor(out=xsq, in0=rsl, in1=rsl, op=ALU.mult)
                nc.tensor.matmul(ps2[:, c % 2, :], lhsT=onescol_bf, rhs=xsq,
                                 start=True, stop=True)
                if c % 2 == 1:
                    nc.scalar.copy(stg[:, 1, :], ps2.rearrange("p a b -> p (a b)"))
                    soff = CH * (c - 1)
                    nc.sync.dma_start(out=sc2_d[0:1, soff:soff + 2 * CH],
                                      in_=stg[:, 0, :])
                    nc.sync.dma_start(out=sc2_d[1:2, soff:soff + 2 * CH],
                                      in_=stg[:, 1, :])

        stats_math(sc2_d, ab2_d, bf16, extra_sum=sc1_d)

        # ============ PH5b: MLP + residual ============
        with tc.tile_pool(name="ph5b", bufs=4) as pool, \
             tc.tile_pool(name="ph5ab", bufs=3) as abpool, \
             tc.tile_pool(name="ph5ps", bufs=2, space="PSUM") as psum:
            ab2_flat = ab2_d[:, :].rearrange("a b -> (a b)")
            for c in range(NCHUNK):
                g, i = c // 4, c % 4
                rsl = r1[:, CH * c:CH * (c + 1)]
                if i == 0:
                    paB5 = abpool.tile([C, 4, CH], bf16, tag="pa5B")
                    nc.sync.dma_start(
                        out=paB5,
                        in_=bass.AP(tensor=ab2_flat.tensor, offset=4 * CH * g,
                                    ap=[[0, C], [1, 4 * CH]]))
                    pbB5 = abpool.tile([C, 4, CH], bf16, tag="pb5B")
                    nc.sync.dma_start(
                        out=pbB5,
                        in_=bass.AP(tensor=ab2_flat.tensor, offset=N + 4 * CH * g,
                                    ap=[[0, C], [1, 4 * CH]]))
                    yout4 = abpool.tile([C, 4, CH], f32, tag="yout4")
                t1 = pool.tile([C, CH], bf16, tag="t15")
                nc.gpsimd.tensor_tensor(out=t1, in0=rsl,
                                        in1=paB5[:, i, :], op=ALU.mult)
                xn = pool.tile([C, CH], bf16, tag="xn5")
                nc.gpsimd.tensor_tensor(out=xn, in0=t1, in1=pbB5[:, i, :], op=ALU.add)

                h1 = pool.tile([128, 3, CH], bf16, tag="h1")
                if has_c1:
                    for j in range(3):
                        pf = psum.tile([128, CH], f32, tag="pf")
                        nc.tensor.matmul(pf, lhsT=w1[:, 128 * j:128 * (j + 1)], rhs=xn,
                                         start=True, stop=True)
                        nc.scalar.activation(out=h1[:, j, :], in_=pf, func=AF.Gelu,
                                             bias=c1[j][:, 0:1], scale=1.0)
                else:
                    pf3 = psum.tile([128, 3, CH], f32, tag="pf3")
                    for j in range(3):
                        nc.tensor.matmul(pf3[:, j, :], lhsT=w1[:, 128 * j:128 * (j + 1)],
                                         rhs=xn, start=True, stop=True)
                    nc.scalar.activation(out=h1, in_=pf3, func=AF.Gelu)
                pm = psum.tile([C, CH], f32, tag="pm")
                if has_fc2b:
                    nc.tensor.matmul(pm, lhsT=b2t, rhs=onesrow, start=True, stop=False)
                for j in range(3):
                    nc.tensor.matmul(pm, lhsT=w2[j], rhs=h1[:, j, :],
                                     start=(j == 0 and not has_fc2b), stop=(j == 2))
                nc.vector.tensor_tensor(out=yout4[:, i, :], in0=rsl,
                                        in1=pm, op=ALU.add)
                if i == 3:
                    nc.sync.dma_start(out=y_d[:, 16 * g:16 * g + 16, :], in_=yout4)

        r1pool.release()
        opool.release()
        wpool.release()

    _split_multi_waits(nc, mybir)
    return nc


def _prep_weights(inputs):
    """Host-side weight preparation (fold LN affine, scale, transposes)."""
    qkv_w = np.asarray(inputs['qkv_w'], np.float32)       # (288, 96)
    proj_w = np.asarray(inputs['proj_w'], np.float32)     # (96, 96)
    proj_b = np.asarray(inputs['proj_b'], np.float32)
    ln1_w = np.asarray(inputs['ln1_w'], np.float32)
    ln1_b = np.asarray(inputs['ln1_b'], np.float32)
    ln2_w = np.asarray(inputs['ln2_w'], np.float32)
    ln2_b = np.asarray(inputs['ln2_b'], np.float32)
    fc1_w = np.asarray(inputs['fc1_w'], np.float32)       # (384, 96)
    fc1_b = np.asarray(inputs['fc1_b'], np.float32)
    fc2_w = np.asarray(inputs['fc2_w'], np.float32)       # (96, 384)
    fc2_b = np.asarray(inputs['fc2_b'], np.float32)

    wq = qkv_w * ln1_w[None, :]                            # (288, 96)
    c0 = qkv_w @ ln1_b                                     # (288,)
    wq[0:C] *= SCALE                                       # scale q rows
    c0[0:C] *= SCALE
    # v bias folds into proj bias; k bias cancels in softmax
    pb_eff = proj_b + proj_w @ c0[2 * C:3 * C]

    w1 = fc1_w * ln2_w[None, :]
    c1 = fc1_w @ ln2_b + fc1_b

    repl = np.zeros((128, 128), np.float32)
    for b in range(NB):
        for ch in range(GD):
            h0 = (ch // HD) * HD
            repl[32 * b + h0:32 * b + h0 + HD, 32 * b + ch] = 1.0

    # proj lhsT in per-band layout: wpb[32b+j, d, o] = proj_w[o, 32d+j];
    # extra column C holds proj_w column-sums so the matmul also emits the
    # per-token sum of the proj output (feeds LN2 stats).
    wpT = proj_w.T                                         # (in=96, out=96)
    colsum = proj_w.sum(axis=0)                            # (96,)
    wpb = np.zeros((128, 3, C + 1), np.float32)
    for b in range(NB):
        for d in range(3):
            wpb[32 * b:32 * b + 32, d, 0:C] = wpT[32 * d:32 * d + 32, :]
            wpb[32 * b:32 * b + 32, d, C] = colsum[32 * d:32 * d + 32]
    pb_ext = np.concatenate([pb_eff, [pb_eff.sum()]]).astype(np.float32)

    return {
        'wqkv': np.ascontiguousarray(wq.T),                # (96, 288) lhsT
        'c0': c0.reshape(-1, 1).astype(np.float32),
        'wproj': np.ascontiguousarray(wpb.reshape(128, 3 * (C + 1))),
        'projb': pb_ext.reshape(-1, 1),
        'w1': np.ascontiguousarray(w1.T),                  # (96, 384) lhsT
        'c1': c1.reshape(-1, 1).astype(np.float32),
        'w2': np.ascontiguousarray(fc2_w.T),               # (384, 96) lhsT
        'b2': fc2_b.reshape(-1, 1).astype(np.float32),
        'repl': repl,
        'onesc': np.ones((C, 1), np.float32),
    }


def kernel(**inputs):
    from concourse.bass_utils import run_bass_kernel_spmd

    wmap = _prep_weights(inputs)
    has_qbias = bool(np.any(wmap['c0'][0:C] != 0))
    has_projb = bool(np.any(wmap['projb'] != 0))
    has_fc2b = bool(np.any(wmap['b2'] != 0))
    has_c1 = bool(np.any(wmap['c1'] != 0))
    key = ('nc', has_qbias, has_projb, has_fc2b, has_c1)

    if key not in _cache:
        t0 = time.time()
        _cache[key] = _build(has_qbias, has_projb, has_fc2b, has_c1)
        print(f"[kernel] built bass module in {time.time() - t0:.1f}s",
              file=sys.stderr)

    nc = _cache[key]
    _cache['nc'] = nc
    x = np.asarray(inputs['x'], np.float32)                # (8, 96, 128, 128)

    in_maps = []
    for b in range(B):
        m = {'x': np.ascontiguousarray(x[b])}
        m.update(wmap)
        in_maps.append(m)

    res = run_bass_kernel_spmd(nc, in_maps, core_ids=list(range(B)))
    _cache['last_exec_ns'] = res.exec_time_ns
    out = np.stack([res.results[b]['y'] for b in range(B)], axis=0)
    return out.astype(np.float32)
